# revision 13
# baseline (speedup 1.0000x reference)
"""GAT (2-layer graph attention network) Bass kernel for 8 Trainium2 NeuronCores.

Strategy (per spec sharding hint): edges are partitioned by destination-node
block so segment-softmax/sum stay core-local; each core owns N/8 destination
nodes. Layer-1 node features (h = x @ W1 plus attention alphas via an
augmented weight matrix) are computed replicated on every core into a DRAM
table with 512-byte rows: 256 fp8(e4m3) h channels + 4 bf16 alpha_src + 4
bf16 alpha_dst. Per-edge source rows arrive via one SWDGE gather per region
(A/B split for the int16 index range). The per-edge alpha_dst values are NOT
gathered: they are broadcast from the block's own alpha rows through a
transposed one-hot mask on the tensor engine (K tiny matmuls), which removes
one 256-B gather per edge versus the previous build. The segment-softmax +
weighted aggregation are fused into PE matmuls with a one-hot {edge x dst}
mask; PSUM accumulates numerator and denominator together. Layer-2
pre-features are exchanged with one AllGather; layer-2 aggregation repeats
the same structure with 256-B bf16 rows. Global mean-pool partials are
reduced on host along with the tiny fc + log_softmax head.
"""
import os
import sys
import types
import math

import numpy as np
import ml_dtypes


def _setup_paths():
    for p in ("/opt/trn_rl_repo", "/root/.axon_site/_ro/trn_rl_repo"):
        if os.path.isdir(p) and p not in sys.path:
            sys.path.insert(0, p)
    try:
        import concourse.bass  # noqa: F401
    except ImportError as e:
        raise RuntimeError(f"concourse not importable: {e}")


_setup_paths()

import concourse.bass as bass  # noqa: E402
import concourse.mybir as mybir  # noqa: E402
import concourse.tile as tile  # noqa: E402
from concourse import bacc, bass_utils  # noqa: E402

bf16 = ml_dtypes.bfloat16
BF = mybir.dt.bfloat16
F32 = mybir.dt.float32
I16 = mybir.dt.int16
FP8 = mybir.dt.float8e4
AL = mybir.AluOpType
AF = mybir.ActivationFunctionType


class Cfg:
    def __init__(self, N=50000, E=800000, IN_C=128, HID=64, OUT_C=64, HEADS=4,
                 NCLS=40, NEG=0.2, NCORES=8):
        self.N, self.E = N, E
        self.IN_C, self.HID, self.OUT_C, self.HEADS = IN_C, HID, OUT_C, HEADS
        self.NCLS, self.NEG, self.NCORES = NCLS, NEG, NCORES
        assert N % NCORES == 0
        self.NB = N // NCORES                      # owned real nodes per core
        self.NBLK = math.ceil(self.NB / 128)       # dst blocks per core
        self.NDP = self.NBLK * 128                 # padded owned rows per core
        self.RTOT = self.NDP * NCORES              # global padded row space
        assert self.RTOT % 128 == 0
        self.NT1 = self.RTOT // 128                # phase-A node tiles
        # A/B gather split (int16 row-index limit), multiple of 128
        self.SPLIT = min(32768, (self.RTOT // 2 + 127) // 128 * 128)
        assert self.SPLIT % 128 == 0 and self.SPLIT < 32768 + 1
        self.C1 = HEADS * HID                      # 256 layer-1 channels
        self.ROW1 = 512                            # table1 row bytes (fp8 units)
        self.AUX1 = 276                            # used bytes per table1 row
        self.HW1 = self.HID + 1                    # 65: head block + ones col
        self.ROW2 = 128                            # table3 cols (256B rows)
        assert self.OUT_C + 2 <= self.ROW2

    def row_of(self, v):
        return self.NDP * (v // self.NB) + (v % self.NB)


def _pack_idx(vals_2d):
    """vals_2d [G, n] -> dma_gather index layout [G, 128, n//16] int16.

    Index i lives at [i % 16, i // 16]; the 16-row group is replicated 8x
    across the 128 partitions.
    """
    G, n = vals_2d.shape
    assert n % 16 == 0
    a = vals_2d.reshape(G, n // 16, 16).transpose(0, 2, 1)   # [G, 16, n/16]
    return np.tile(a, (1, 8, 1)).astype(np.int16)            # [G, 128, n/16]


def host_prep(cfg, x, edge_index, W1, att_src1, att_dst1, b1, W2, att_src2,
              att_dst2, b2):
    """Build per-core in_maps (everything except the graph-independent consts)."""
    c = cfg
    src = np.asarray(edge_index[0], dtype=np.int64)
    dst = np.asarray(edge_index[1], dtype=np.int64)
    loops = np.arange(c.N, dtype=np.int64)
    src = np.concatenate([src, loops])
    dst = np.concatenate([dst, loops])
    EE = src.shape[0]

    core = dst // c.NB
    # load-balanced permutation of each core's dst nodes into blocks so that
    # per-block A/B-region edge counts are even (minimizes gather slot count K)
    t0core = src // c.NB                           # provisional (pre-perm) side
    rowmap = np.zeros(c.N, np.int64)               # node -> global padded row
    for ci in range(c.NCORES):
        sel = core == ci
        dloc = dst[sel] - ci * c.NB
        srcA = (c.row_of(src[sel]) < c.SPLIT)      # approx region (pre-perm)
        degA = np.bincount(dloc[srcA], minlength=c.NB).astype(np.int64)
        degT = np.bincount(dloc, minlength=c.NB).astype(np.int64)
        degB = degT - degA
        order_d = np.argsort(-degT, kind="stable")
        cnt = np.zeros(c.NBLK, np.int64)
        lA = np.zeros(c.NBLK, np.float64)
        lB = np.zeros(c.NBLK, np.float64)
        perm = np.zeros(c.NB, np.int64)
        for v in order_d:
            score = np.maximum(lA + degA[v], lB + degB[v]) + 1e9 * (cnt >= 128)
            j = int(np.argmin(score))
            perm[v] = j * 128 + cnt[j]
            cnt[j] += 1
            lA[j] += degA[v]
            lB[j] += degB[v]
        rowmap[ci * c.NB:(ci + 1) * c.NB] = ci * c.NDP + perm
    drow = rowmap[dst]
    blk = (drow - core * c.NDP) // 128
    din = (drow - core * c.NDP) % 128              # dst index within block
    srow = rowmap[src]
    isB = (srow >= c.SPLIT).astype(np.int64)

    gid = (core * c.NBLK + blk) * 2 + isB          # group id (A/B separate)
    order = np.argsort(gid, kind="stable")
    gid_s = gid[order]
    counts = np.bincount(gid_s, minlength=c.NCORES * c.NBLK * 2)
    nA = counts[0::2].reshape(c.NCORES, c.NBLK)
    nB = counts[1::2].reshape(c.NCORES, c.NBLK)
    kA = max(1, int(math.ceil(nA.max() / 128)))
    kB = max(1, int(math.ceil(nB.max() / 128)))
    K = kA + kB

    # rank within group
    starts = np.zeros_like(counts)
    starts[1:] = np.cumsum(counts)[:-1]
    rank = np.arange(EE) - starts[gid_s]

    # destination slot within the (core, blk) slot array of length K*128
    slot = np.where(isB[order] == 0, rank, kA * 128 + rank)
    cg = core[order] * c.NBLK + blk[order]          # [EE] group (core, blk)

    srow_slot = np.zeros((c.NCORES * c.NBLK, K * 128), np.int64)
    srow_slot[:, kA * 128:] = c.SPLIT               # B-region pad -> idx 0
    din_slot = np.full((c.NCORES * c.NBLK, K * 128), 128.0, np.float32)
    srow_slot[cg, slot] = srow[order]
    din_slot[cg, slot] = din[order]

    srow_slot = srow_slot.reshape(c.NCORES, c.NBLK, K * 128)
    din_slot = din_slot.reshape(c.NCORES, c.NBLK, K * 128)

    # augmented weights
    W1 = np.asarray(W1, np.float32)
    a_s1 = np.asarray(att_src1, np.float32).reshape(c.HEADS, c.HID)
    a_d1 = np.asarray(att_dst1, np.float32).reshape(c.HEADS, c.HID)
    W1r = W1.reshape(c.IN_C, c.HEADS, c.HID)
    Wa_s = np.einsum("khc,hc->kh", W1r, a_s1)       # [IN_C, HEADS]
    Wa_d = np.einsum("khc,hc->kh", W1r, a_d1)
    w1aug = np.zeros((c.IN_C, c.C1 + 8), np.float32)
    w1aug[:, :c.C1] = W1
    w1aug[:, c.C1:c.C1 + c.HEADS] = Wa_s
    w1aug[:, c.C1 + 4:c.C1 + 4 + c.HEADS] = Wa_d

    W2 = np.asarray(W2, np.float32)
    a_s2 = np.asarray(att_src2, np.float32).reshape(c.OUT_C)
    a_d2 = np.asarray(att_dst2, np.float32).reshape(c.OUT_C)
    w2aug = np.zeros((c.C1, 72), np.float32)
    w2aug[:, :c.OUT_C] = W2
    w2aug[:, c.OUT_C] = W2 @ a_s2
    w2aug[:, c.OUT_C + 1] = W2 @ a_d2

    assert np.allclose(np.asarray(b1), 0) and np.allclose(np.asarray(b2), 0), \
        "nonzero biases not folded in this build"

    # padded, row-mapped, transposed x tiles
    x = np.asarray(x, np.float32)
    x_pad = np.zeros((c.RTOT, c.IN_C), np.float32)
    x_pad[rowmap] = x
    xT = x_pad.reshape(c.NT1, 128, c.IN_C).transpose(0, 2, 1)  # [t, k, n]
    xT = np.ascontiguousarray(xT).astype(bf16)

    iota = np.broadcast_to(np.arange(128, dtype=np.float32),
                           (128, 128)).astype(bf16).copy()
    iota_p = np.arange(128, dtype=np.float32).reshape(128, 1).astype(bf16)
    ones = np.ones((128, 1), np.float32)

    in_maps = []
    meta = dict(kA=kA, kB=kB, K=K)
    for ci in range(c.NCORES):
        idxA = _pack_idx(srow_slot[ci, :, :kA * 128].copy())         # [NBLK,128,kA*8]
        idxB = _pack_idx(srow_slot[ci, :, kA * 128:] - c.SPLIT)
        dr = din_slot[ci].reshape(c.NBLK, K, 128).transpose(2, 0, 1)  # [128,NBLK,K]
        drT = din_slot[ci].reshape(c.NBLK, K * 128)                   # [NBLK,K*128]
        xo = xT[ci * c.NBLK:(ci + 1) * c.NBLK]                       # own tiles
        in_maps.append({
            "x_t3": xT,
            "x_own": np.ascontiguousarray(xo),
            "w1aug": w1aug.astype(bf16),
            "w2aug": np.ascontiguousarray(w2aug.astype(bf16).reshape(c.C1 // 128, 128, 72).transpose(1, 0, 2)),
            "idxA": np.ascontiguousarray(idxA.transpose(1, 0, 2)),   # [128,NBLK,kA*8]
            "idxB": np.ascontiguousarray(idxB.transpose(1, 0, 2)),
            "dstrel": np.ascontiguousarray(dr).astype(bf16),
            "dstrelT": np.ascontiguousarray(drT).astype(bf16),
            "iota": iota,
            "iota_p": iota_p,
            "ones": ones,
        })
    return in_maps, meta


def build(cfg, kA, kB, core_id_split=None, debug=False, stage="F"):
    """stage: truncate program after phase A/B/C/D/E/F (for HW bisection)."""
    c = cfg
    K = kA + kB
    KH = c.C1 // 128                      # k-halves for layer-2 contraction
    nc = bacc.Bacc("TRN2", target_bir_lowering=False, debug=False,
                   num_devices=c.NCORES)

    # ---- IO ----
    x_t3_d = nc.dram_tensor("x_t3", [c.NT1, 128, c.IN_C], BF, kind="ExternalInput").ap()
    x_own_d = nc.dram_tensor("x_own", [c.NBLK, 128, c.IN_C], BF, kind="ExternalInput").ap()
    w1_d = nc.dram_tensor("w1aug", [c.IN_C, c.C1 + 8], BF, kind="ExternalInput").ap()
    w2_d = nc.dram_tensor("w2aug", [128, KH, 72], BF, kind="ExternalInput").ap()
    idxA_d = nc.dram_tensor("idxA", [128, c.NBLK, kA * 8], I16, kind="ExternalInput").ap()
    idxB_d = nc.dram_tensor("idxB", [128, c.NBLK, kB * 8], I16, kind="ExternalInput").ap()
    dstrel_d = nc.dram_tensor("dstrel", [128, c.NBLK, K], BF, kind="ExternalInput").ap()
    dstrelT_d = nc.dram_tensor("dstrelT", [c.NBLK, K * 128], BF, kind="ExternalInput").ap()
    iota_d = nc.dram_tensor("iota", [128, 128], BF, kind="ExternalInput").ap()
    iotap_d = nc.dram_tensor("iota_p", [128, 1], BF, kind="ExternalInput").ap()
    ones_d = nc.dram_tensor("ones", [128, 1], F32, kind="ExternalInput").ap()
    pool_d = nc.dram_tensor("pool64", [c.OUT_C, 1], F32, kind="ExternalOutput").ap()
    if debug:
        h1dbg_d = nc.dram_tensor("h1dbg", [c.NDP, c.C1], F32, kind="ExternalOutput").ap()
        h2dbg_d = nc.dram_tensor("h2dbg", [c.NDP, 72], F32, kind="ExternalOutput").ap()

    # ---- internal DRAM ----
    nsplit_t = c.SPLIT // 128
    t1A = nc.dram_tensor("t1A", [c.SPLIT, c.ROW1], FP8, kind="Internal").ap()
    t1B = nc.dram_tensor("t1B", [c.RTOT - c.SPLIT, c.ROW1], FP8, kind="Internal").ap()
    h1d = nc.dram_tensor("h1d", [c.NDP, c.C1], BF, kind="Internal").ap()
    cc3in = nc.dram_tensor("cc3in", [c.NDP, c.ROW2], BF, kind="Internal").ap()
    cc3 = nc.dram_tensor("cc3", [c.RTOT, c.ROW2], BF, kind="Internal",
                         addr_space="Shared").ap()

    with tile.TileContext(nc) as tc:
        with tc.tile_pool(name="const", bufs=1) as cpool, \
             tc.tile_pool(name="pa", bufs=3) as pa, \
             tc.tile_pool(name="pp", bufs=2, space="PSUM") as pp, \
             tc.tile_pool(name="pg", bufs=2) as pg, \
             tc.tile_pool(name="pe2", bufs=2) as pe2, \
             tc.tile_pool(name="sm", bufs=3) as sm:

            # constants resident
            w1s = cpool.tile_from(w1_d)                     # [128, C1+8]
            w2s = cpool.tile_from(w2_d)                     # [128, KH, 72]
            iota_s = cpool.tile_from(iota_d)
            iotap_s = cpool.tile_from(iotap_d)
            ones_s = cpool.tile_from(ones_d)
            idxA_s = cpool.tile_from(idxA_d)
            idxB_s = cpool.tile_from(idxB_d)
            dstrel_s = cpool.tile_from(dstrel_d)

            # ================= phase A: h table (replicated) =================
            CH = 4
            for t0 in range(0, c.NT1, CH):
                xt = pa.tile([128, CH, c.IN_C], BF, tag="xt")
                nc.sync.dma_start(
                    out=xt[:], in_=x_t3_d[t0:t0 + CH, :, :].rearrange("a k n -> k a n"))
                ob = pa.tile([128, CH, c.AUX1], FP8, tag="ob")
                for i in range(CH):
                    ps = pp.tile([128, c.C1 + 8], F32, tag="A")
                    nc.tensor.matmul(out=ps[:], lhsT=xt[:, i, :], rhs=w1s[:],
                                     start=True, stop=True)
                    obv = ob[:, i, 0:4 * c.HW1].rearrange("p (h q) -> p h q",
                                                          q=c.HW1)
                    psv = ps[:, 0:c.C1].rearrange("p (h q) -> p h q", q=c.HID)
                    if i % 2 == 0:
                        nc.scalar.activation(out=obv[:, :, 0:c.HID], in_=psv,
                                             func=AF.Copy)
                        nc.vector.tensor_copy(
                            out=ob[:, i, 4 * c.HW1:4 * c.HW1 + 16].bitcast(BF),
                            in_=ps[:, c.C1:c.C1 + 8])
                        nc.vector.tensor_scalar(
                            out=obv[:, :, c.HID:c.HW1], in0=psv[:, :, 0:1],
                            scalar1=0.0, scalar2=1.0, op0=AL.mult, op1=AL.add)
                    else:
                        nc.vector.tensor_copy(out=obv[:, :, 0:c.HID], in_=psv)
                        nc.scalar.activation(
                            out=ob[:, i, 4 * c.HW1:4 * c.HW1 + 16].bitcast(BF),
                            in_=ps[:, c.C1:c.C1 + 8], func=AF.Copy)
                        nc.scalar.activation(
                            out=obv[:, :, c.HID:c.HW1], in_=psv[:, :, 0:1],
                            func=AF.Relu, scale=0.0, bias=1.0)
                r0 = t0 * 128
                if t0 < nsplit_t:
                    dstv = t1A[r0:r0 + CH * 128, 0:c.AUX1]
                else:
                    dstv = t1B[r0 - c.SPLIT:r0 - c.SPLIT + CH * 128, 0:c.AUX1]
                nc.sync.dma_start(
                    out=dstv.rearrange("(a p) q -> p a q", p=128), in_=ob[:])

            # ================= phase B: layer-1 edge aggregation =============
            for b in (range(c.NBLK) if stage >= "B" else []):
                hg = pg.tile([128, K, c.ROW1], FP8, tag="hg")
                nc.gpsimd.dma_gather(
                    out_ap=hg[:, 0:kA, :], in_ap=t1A[:, :],
                    idxs_ap=idxA_s[:, b, :], num_idxs=kA * 128,
                    num_idxs_reg=kA * 128, elem_size=c.ROW1, single_packet=False)
                nc.gpsimd.dma_gather(
                    out_ap=hg[:, kA:K, :], in_ap=t1B[:, :],
                    idxs_ap=idxB_s[:, b, :], num_idxs=kB * 128,
                    num_idxs_reg=kB * 128, elem_size=c.ROW1, single_packet=False)

                # transposed one-hot mask (dst-major) for alpha_dst broadcast
                dT = pg.tile([128, K * 128], BF, tag="dT")
                nc.sync.dma_start(
                    out=dT[:], in_=dstrelT_d[b:b + 1, :].to_broadcast([128, K * 128]))
                maskT = pg.tile([128, K, 128], BF, tag="mT")
                nc.vector.tensor_tensor(
                    out=maskT[:],
                    in0=iotap_s[:, :, None].to_broadcast([128, K, 128]),
                    in1=dT[:].rearrange("p (k s) -> p k s", k=K),
                    op=AL.is_equal)
                # block's own alpha_dst rows (on-the-fly from own x tile)
                xo = pa.tile([128, c.IN_C], BF, tag="xo")
                nc.sync.dma_start(out=xo[:], in_=x_own_d[b, :, :])
                pso = pp.tile([128, c.C1 + 8], F32, tag="A")
                nc.tensor.matmul(out=pso[:, 0:8], lhsT=xo[:],
                                 rhs=w1s[:, c.C1:c.C1 + 8],
                                 start=True, stop=True)
                adb = sm.tile([128, 4], BF, tag="adb")
                nc.scalar.activation(out=adb[:], in_=pso[:, 4:8], func=AF.Copy)
                psL = pp.tile([128, K * 4], F32, tag="L")
                for j in range(K):
                    nc.tensor.matmul(out=psL[:, j * 4:(j + 1) * 4],
                                     lhsT=maskT[:, j, :],
                                     rhs=adb[:],
                                     start=True, stop=True)

                z = sm.tile([128, K, c.HEADS], F32, tag="z")
                nc.vector.tensor_tensor(
                    out=z[:],
                    in0=hg[:, :, 4 * c.HW1:4 * c.HW1 + 8].bitcast(BF),
                    in1=psL[:].rearrange("p (k h) -> p k h", k=K), op=AL.add)
                lr = sm.tile([128, K, c.HEADS], F32, tag="lr")
                nc.vector.scalar_tensor_tensor(
                    out=lr[:], in0=z[:], scalar=c.NEG, in1=z[:],
                    op0=AL.mult, op1=AL.max)
                eeb = sm.tile([128, K, c.HEADS], BF, tag="eeb")
                nc.scalar.activation(out=eeb[:], in_=lr[:], func=AF.Exp)

                mask = pg.tile([128, K, 128], BF, tag="mask")
                nc.vector.tensor_tensor(
                    out=mask[:],
                    in0=iota_s[:, None, :].to_broadcast([128, K, 128]),
                    in1=dstrel_s[:, b, :, None].to_broadcast([128, K, 128]),
                    op=AL.is_equal)

                v = pg.tile([128, K, 4 * c.HW1], BF, tag="v")
                nc.vector.tensor_tensor(
                    out=v[:].rearrange("p k (h q) -> p k h q", h=c.HEADS),
                    in0=hg[:, :, 0:4 * c.HW1].rearrange("p k (h q) -> p k h q",
                                                        h=c.HEADS),
                    in1=eeb[:, :, :, None].to_broadcast([128, K, c.HEADS, c.HW1]),
                    op=AL.mult)

                ps = pp.tile([128, 4 * c.HW1], F32, tag="B")
                for j in range(K):
                    nc.tensor.matmul(out=ps[:], lhsT=mask[:, j, :],
                                     rhs=v[:, j, :],
                                     start=(j == 0), stop=(j == K - 1))

                den = sm.tile([128, c.HEADS], F32, tag="den")
                nc.vector.tensor_scalar(
                    out=den[:, :, None],
                    in0=ps[:].rearrange("p (h q) -> p h q",
                                        q=c.HW1)[:, :, c.HID:c.HW1],
                    scalar1=1e-16, scalar2=None, op0=AL.add)
                rec = sm.tile([128, c.HEADS], F32, tag="rec")
                nc.vector.reciprocal(out=rec[:], in_=den[:])
                h1b = sm.tile([128, c.C1], BF, tag="h1b")
                for hh in range(c.HEADS):
                    nc.scalar.activation(
                        out=h1b[:, hh * c.HID:(hh + 1) * c.HID],
                        in_=ps[:, hh * c.HW1:hh * c.HW1 + c.HID],
                        func=AF.Relu, scale=rec[:, hh:hh + 1])
                nc.sync.dma_start(out=h1d[b * 128:(b + 1) * 128, :], in_=h1b[:])
                if debug:
                    h1dbgf = sm.tile([128, c.C1], F32, tag="h1dbgf")
                    nc.vector.tensor_copy(out=h1dbgf[:], in_=h1b[:])
                    nc.sync.dma_start(out=h1dbg_d[b * 128:(b + 1) * 128, :],
                                      in_=h1dbgf[:])

            # ================= phase C: h2_pre = h1 @ W2aug ==================
            for b in (range(c.NBLK) if stage >= "C" else []):
                psc = pp.tile([128, c.C1 + 8], F32, tag="A")
                for kh in range(KH):
                    ht = pa.tile([128, 128], BF, tag="ht")
                    nc.sync.dma_start(
                        out=ht[:], in_=h1d[b * 128:(b + 1) * 128,
                                           kh * 128:(kh + 1) * 128],
                        transpose=True)
                    nc.tensor.matmul(out=psc[:, 0:72], lhsT=ht[:], rhs=w2s[:, kh, :],
                                     start=(kh == 0), stop=(kh == KH - 1))
                hc = pa.tile([128, c.ROW2], BF, tag="hc")
                nc.vector.memset(hc[:, 67:c.ROW2], 0.0)
                nc.vector.memset(hc[:, c.OUT_C:c.OUT_C + 1], 1.0)
                nc.vector.tensor_copy(out=hc[:, 0:c.OUT_C], in_=psc[:, 0:c.OUT_C])
                nc.vector.tensor_copy(out=hc[:, 65:67],
                                      in_=psc[:, c.OUT_C:c.OUT_C + 2])
                nc.sync.dma_start(out=cc3in[b * 128:(b + 1) * 128, :], in_=hc[:])
                if debug:
                    h2f = pa.tile([128, 72], F32, tag="h2f")
                    nc.vector.tensor_copy(out=h2f[:], in_=psc[:, 0:72])
                    nc.sync.dma_start(out=h2dbg_d[b * 128:(b + 1) * 128, :],
                                      in_=h2f[:])

            # ================= phase D: allgather + repack ===================
            if stage >= "D":
                nc.gpsimd.collective_compute(
                    kind="AllGather", op=AL.bypass,
                    replica_groups=[list(range(c.NCORES))],
                    ins=[cc3in[:, :]], outs=[cc3[:, :]])

            # block-own alpha_dst2 column, resident for phase E
            a2dS = cpool.tile([128, c.NBLK], BF)
            if stage >= "D":
                nc.sync.dma_start(
                    out=a2dS[:],
                    in_=cc3in[0:c.NDP, 66:67].rearrange(
                        "(b p) q -> p (b q)", p=128))
            else:
                nc.vector.memset(a2dS[:], 0.0)

            # ================= phase E: layer-2 edge aggregation =============
            pacc = cpool.tile([128, c.OUT_C], F32)
            nc.vector.memset(pacc[:], 0.0)
            for b in (range(c.NBLK) if stage >= "E" else []):
                hg2 = pe2.tile([128, K, c.ROW2], BF, tag="hg2")
                nc.gpsimd.dma_gather(
                    out_ap=hg2[:, 0:kA, :], in_ap=cc3[:, :],
                    idxs_ap=idxA_s[:, b, :], num_idxs=kA * 128,
                    num_idxs_reg=kA * 128, elem_size=c.ROW2, single_packet=False)
                nc.gpsimd.dma_gather(
                    out_ap=hg2[:, kA:K, :], in_ap=cc3[c.SPLIT:c.RTOT, :],
                    idxs_ap=idxB_s[:, b, :], num_idxs=kB * 128,
                    num_idxs_reg=kB * 128, elem_size=c.ROW2, single_packet=False)

                dT2 = pe2.tile([128, K * 128], BF, tag="dT2")
                nc.sync.dma_start(
                    out=dT2[:], in_=dstrelT_d[b:b + 1, :].to_broadcast([128, K * 128]))
                maskT2 = pe2.tile([128, K, 128], BF, tag="mT2")
                nc.vector.tensor_tensor(
                    out=maskT2[:],
                    in0=iotap_s[:, :, None].to_broadcast([128, K, 128]),
                    in1=dT2[:].rearrange("p (k s) -> p k s", k=K),
                    op=AL.is_equal)
                psL2 = pp.tile([128, K * 4], F32, tag="L")
                for j in range(K):
                    nc.tensor.matmul(out=psL2[:, j:j + 1],
                                     lhsT=maskT2[:, j, :],
                                     rhs=a2dS[:, b:b + 1],
                                     start=True, stop=True)

                z2 = sm.tile([128, K], F32, tag="z2")
                nc.vector.tensor_tensor(
                    out=z2[:],
                    in0=hg2[:, :, 65:66].rearrange("p k q -> p (k q)"),
                    in1=psL2[:, 0:K], op=AL.add)
                lr2 = sm.tile([128, K], F32, tag="lr2")
                nc.vector.scalar_tensor_tensor(
                    out=lr2[:], in0=z2[:], scalar=c.NEG, in1=z2[:],
                    op0=AL.mult, op1=AL.max)
                ee2 = sm.tile([128, K], BF, tag="ee2")
                nc.scalar.activation(out=ee2[:], in_=lr2[:], func=AF.Exp)

                mask2 = pe2.tile([128, K, 128], BF, tag="mask2")
                nc.vector.tensor_tensor(
                    out=mask2[:],
                    in0=iota_s[:, None, :].to_broadcast([128, K, 128]),
                    in1=dstrel_s[:, b, :, None].to_broadcast([128, K, 128]),
                    op=AL.is_equal)

                v2 = pe2.tile([128, K, c.OUT_C + 1], BF, tag="v2")
                nc.vector.tensor_tensor(
                    out=v2[:], in0=hg2[:, :, 0:c.OUT_C + 1],
                    in1=ee2[:, :, None].to_broadcast([128, K, c.OUT_C + 1]),
                    op=AL.mult)

                ps2 = pp.tile([128, c.OUT_C + 1], F32, tag="E")
                for j in range(K):
                    nc.tensor.matmul(out=ps2[:], lhsT=mask2[:, j, :],
                                     rhs=v2[:, j, :],
                                     start=(j == 0), stop=(j == K - 1))

                den2 = sm.tile([128, 1], F32, tag="den2")
                nc.vector.tensor_scalar(
                    out=den2[:], in0=ps2[:, c.OUT_C:c.OUT_C + 1], scalar1=1e-16,
                    scalar2=None, op0=AL.add)
                rec2 = sm.tile([128, 1], F32, tag="rec2")
                nc.vector.reciprocal(out=rec2[:], in_=den2[:])
                o2r = sm.tile([128, c.OUT_C], F32, tag="o2r")
                nc.scalar.activation(out=o2r[:], in_=ps2[:, 0:c.OUT_C],
                                     func=AF.Relu, scale=rec2[:])
                nc.vector.tensor_tensor(out=pacc[:], in0=pacc[:], in1=o2r[:],
                                        op=AL.add)

            # ================= phase F: pool partial =========================
            psf = pp.tile([c.OUT_C + 1, 1], F32, tag="E")
            nc.tensor.matmul(out=psf[0:c.OUT_C, :], lhsT=pacc[:], rhs=ones_s[:],
                             start=True, stop=True)
            pf = sm.tile([c.OUT_C, 1], F32, tag="pf")
            nc.vector.tensor_copy(out=pf[:], in_=psf[0:c.OUT_C, :])
            nc.sync.dma_start(out=pool_d[:, :], in_=pf[:])

    nc.compile()
    legalize_waits(nc)
    return nc


def legalize_waits(nc):
    """Walrus encodes at most ONE sync wait per instruction on this toolchain.
    Hoist excess waits onto same-engine NoOps inserted before the instruction."""
    for fn in nc.m.functions:
        for bb in fn.blocks:
            insts = list(bb.instructions)
            out = []
            changed = False
            for inst in insts:
                si = inst.sync_info
                if si is not None and si.on_wait and len(si.on_wait) > 1:
                    waits = list(si.on_wait)
                    for w in waits[:-1]:
                        nop = mybir.InstNoOp(
                            name=nc.get_next_instruction_name(), ins=[], outs=[])
                        nop.engine = inst.engine
                        nop.sync_info = mybir.SyncInfo(on_wait=[w], on_update=[])
                        nc.register_instruction(nop)
                        out.append(nop)
                    inst.sync_info = mybir.SyncInfo(
                        on_wait=waits[-1:], on_update=list(si.on_update))
                    changed = True
                out.append(inst)
            if changed:
                bb.instructions.clear()
                bb.instructions.extend(out)


def host_finish(cfg, pools, fc_w, fc_b):
    c = cfg
    tot = np.zeros(c.OUT_C, np.float64)
    for p in pools:
        tot += p[:, 0].astype(np.float64)
    pooled = (tot / c.N).astype(np.float32)
    logits = pooled @ np.asarray(fc_w, np.float32) + np.asarray(fc_b, np.float32)
    m = logits.max()
    ls = logits - (m + np.log(np.exp(logits - m).sum()))
    return ls.reshape(1, c.NCLS).astype(np.float32)


_BUILD_CACHE = {}


def run(cfg, inputs, debug=False, trace=False, **run_kwargs):
    in_maps, meta = host_prep(
        cfg, inputs["x"], inputs["edge_index"], inputs["W1"], inputs["att_src1"],
        inputs["att_dst1"], inputs["b1"], inputs["W2"], inputs["att_src2"],
        inputs["att_dst2"], inputs["b2"])
    stage = os.environ.get("KSTAGE", "F")
    key = (cfg.N, cfg.E, meta["kA"], meta["kB"], debug, stage)
    if key not in _BUILD_CACHE:
        _BUILD_CACHE[key] = build(cfg, meta["kA"], meta["kB"], debug=debug,
                                  stage=stage)
    nc = _BUILD_CACHE[key]
    res = bass_utils.run_bass_kernel_spmd(
        nc, in_maps, core_ids=list(range(cfg.NCORES)), trace=trace, **run_kwargs)
    out = host_finish(cfg, [r["pool64"] for r in res.results],
                      inputs["fc_w"], inputs["fc_b"])
    return out, res


def kernel(**inputs):
    cfg = Cfg()
    out, _ = run(cfg, inputs)
    return out


# revision 14
# speedup vs baseline: 1.1783x; 1.1783x over previous
"""GAT (2-layer graph attention network) Bass kernel for 8 Trainium2 NeuronCores.

Strategy (per spec sharding hint): edges are partitioned by destination-node
block so segment-softmax/sum stay core-local; each core owns N/8 destination
nodes. Layer-1 node features (h = x @ W1 plus attention alphas via an
augmented weight matrix) are computed replicated on every core into a DRAM
table with 512-byte rows: 256 fp8(e4m3) h channels + 4 bf16 alpha_src + 4
bf16 alpha_dst. Per-edge source rows arrive via one SWDGE gather per region
(A/B split for the int16 index range). The per-edge alpha_dst values are NOT
gathered: they are broadcast from the block's own alpha rows through a
transposed one-hot mask on the tensor engine (K tiny matmuls), which removes
one 256-B gather per edge versus the previous build. The segment-softmax +
weighted aggregation are fused into PE matmuls with a one-hot {edge x dst}
mask; PSUM accumulates numerator and denominator together. Layer-2
pre-features are exchanged with one AllGather; layer-2 aggregation repeats
the same structure with 256-B bf16 rows. Global mean-pool partials are
reduced on host along with the tiny fc + log_softmax head.
"""
import os
import sys
import types
import math

import numpy as np
import ml_dtypes


def _setup_paths():
    for p in ("/opt/trn_rl_repo", "/root/.axon_site/_ro/trn_rl_repo"):
        if os.path.isdir(p) and p not in sys.path:
            sys.path.insert(0, p)
    try:
        import concourse.bass  # noqa: F401
    except ImportError as e:
        raise RuntimeError(f"concourse not importable: {e}")


_setup_paths()

import concourse.bass as bass  # noqa: E402
import concourse.mybir as mybir  # noqa: E402
import concourse.tile as tile  # noqa: E402
from concourse import bacc, bass_utils  # noqa: E402

bf16 = ml_dtypes.bfloat16
BF = mybir.dt.bfloat16
F32 = mybir.dt.float32
I16 = mybir.dt.int16
FP8 = mybir.dt.float8e4
AL = mybir.AluOpType
AF = mybir.ActivationFunctionType


class Cfg:
    def __init__(self, N=50000, E=800000, IN_C=128, HID=64, OUT_C=64, HEADS=4,
                 NCLS=40, NEG=0.2, NCORES=8):
        self.N, self.E = N, E
        self.IN_C, self.HID, self.OUT_C, self.HEADS = IN_C, HID, OUT_C, HEADS
        self.NCLS, self.NEG, self.NCORES = NCLS, NEG, NCORES
        assert N % NCORES == 0
        self.NB = N // NCORES                      # owned real nodes per core
        self.NBLK = math.ceil(self.NB / 128)       # dst blocks per core
        self.NDP = self.NBLK * 128                 # padded owned rows per core
        self.RTOT = self.NDP * NCORES              # global padded row space
        assert self.RTOT % 128 == 0
        self.NT1 = self.RTOT // 128                # phase-A node tiles
        # A/B gather split (int16 row-index limit), multiple of 128
        self.SPLIT = min(32768, (self.RTOT // 2 + 127) // 128 * 128)
        assert self.SPLIT % 128 == 0 and self.SPLIT < 32768 + 1
        self.C1 = HEADS * HID                      # 256 layer-1 channels
        self.ROW1 = 512                            # table1 row bytes (fp8 units)
        self.AUX1 = 276                            # used bytes per table1 row
        self.HW1 = self.HID + 1                    # 65: head block + ones col
        self.ROW2 = 128                            # table3 cols (256B rows)
        assert self.OUT_C + 2 <= self.ROW2

    def row_of(self, v):
        return self.NDP * (v // self.NB) + (v % self.NB)


def _pack_idx(vals_2d):
    """vals_2d [G, n] -> dma_gather index layout [G, 128, n//16] int16.

    Index i lives at [i % 16, i // 16]; the 16-row group is replicated 8x
    across the 128 partitions.
    """
    G, n = vals_2d.shape
    assert n % 16 == 0
    a = vals_2d.reshape(G, n // 16, 16).transpose(0, 2, 1)   # [G, 16, n/16]
    return np.tile(a, (1, 8, 1)).astype(np.int16)            # [G, 128, n/16]


def host_prep(cfg, x, edge_index, W1, att_src1, att_dst1, b1, W2, att_src2,
              att_dst2, b2):
    """Build per-core in_maps (everything except the graph-independent consts)."""
    c = cfg
    src = np.asarray(edge_index[0], dtype=np.int64)
    dst = np.asarray(edge_index[1], dtype=np.int64)
    loops = np.arange(c.N, dtype=np.int64)
    src = np.concatenate([src, loops])
    dst = np.concatenate([dst, loops])
    EE = src.shape[0]

    core = dst // c.NB
    # load-balanced permutation of each core's dst nodes into blocks so that
    # per-block A/B-region edge counts are even (minimizes gather slot count K)
    t0core = src // c.NB                           # provisional (pre-perm) side
    rowmap = np.zeros(c.N, np.int64)               # node -> global padded row
    for ci in range(c.NCORES):
        sel = core == ci
        dloc = dst[sel] - ci * c.NB
        srcA = (c.row_of(src[sel]) < c.SPLIT)      # approx region (pre-perm)
        degA = np.bincount(dloc[srcA], minlength=c.NB).astype(np.int64)
        degT = np.bincount(dloc, minlength=c.NB).astype(np.int64)
        degB = degT - degA
        order_d = np.argsort(-degT, kind="stable")
        cnt = np.zeros(c.NBLK, np.int64)
        lA = np.zeros(c.NBLK, np.float64)
        lB = np.zeros(c.NBLK, np.float64)
        perm = np.zeros(c.NB, np.int64)
        for v in order_d:
            score = np.maximum(lA + degA[v], lB + degB[v]) + 1e9 * (cnt >= 128)
            j = int(np.argmin(score))
            perm[v] = j * 128 + cnt[j]
            cnt[j] += 1
            lA[j] += degA[v]
            lB[j] += degB[v]
        rowmap[ci * c.NB:(ci + 1) * c.NB] = ci * c.NDP + perm
    drow = rowmap[dst]
    blk = (drow - core * c.NDP) // 128
    din = (drow - core * c.NDP) % 128              # dst index within block
    srow = rowmap[src]
    isB = (srow >= c.SPLIT).astype(np.int64)

    gid = (core * c.NBLK + blk) * 2 + isB          # group id (A/B separate)
    order = np.argsort(gid, kind="stable")
    gid_s = gid[order]
    counts = np.bincount(gid_s, minlength=c.NCORES * c.NBLK * 2)
    nA = counts[0::2].reshape(c.NCORES, c.NBLK)
    nB = counts[1::2].reshape(c.NCORES, c.NBLK)
    kA = max(1, int(math.ceil(nA.max() / 128)))
    kB = max(1, int(math.ceil(nB.max() / 128)))
    K = kA + kB

    # rank within group
    starts = np.zeros_like(counts)
    starts[1:] = np.cumsum(counts)[:-1]
    rank = np.arange(EE) - starts[gid_s]

    # destination slot within the (core, blk) slot array of length K*128
    slot = np.where(isB[order] == 0, rank, kA * 128 + rank)
    cg = core[order] * c.NBLK + blk[order]          # [EE] group (core, blk)

    srow_slot = np.zeros((c.NCORES * c.NBLK, K * 128), np.int64)
    srow_slot[:, kA * 128:] = c.SPLIT               # B-region pad -> idx 0
    din_slot = np.full((c.NCORES * c.NBLK, K * 128), 128.0, np.float32)
    srow_slot[cg, slot] = srow[order]
    din_slot[cg, slot] = din[order]

    srow_slot = srow_slot.reshape(c.NCORES, c.NBLK, K * 128)
    din_slot = din_slot.reshape(c.NCORES, c.NBLK, K * 128)

    # augmented weights
    W1 = np.asarray(W1, np.float32)
    a_s1 = np.asarray(att_src1, np.float32).reshape(c.HEADS, c.HID)
    a_d1 = np.asarray(att_dst1, np.float32).reshape(c.HEADS, c.HID)
    W1r = W1.reshape(c.IN_C, c.HEADS, c.HID)
    Wa_s = np.einsum("khc,hc->kh", W1r, a_s1)       # [IN_C, HEADS]
    Wa_d = np.einsum("khc,hc->kh", W1r, a_d1)
    w1aug = np.zeros((c.IN_C, c.C1 + 8), np.float32)
    w1aug[:, :c.C1] = W1
    w1aug[:, c.C1:c.C1 + c.HEADS] = Wa_s
    w1aug[:, c.C1 + 4:c.C1 + 4 + c.HEADS] = Wa_d

    W2 = np.asarray(W2, np.float32)
    a_s2 = np.asarray(att_src2, np.float32).reshape(c.OUT_C)
    a_d2 = np.asarray(att_dst2, np.float32).reshape(c.OUT_C)
    w2aug = np.zeros((c.C1, 72), np.float32)
    w2aug[:, :c.OUT_C] = W2
    w2aug[:, c.OUT_C] = W2 @ a_s2
    w2aug[:, c.OUT_C + 1] = W2 @ a_d2

    assert np.allclose(np.asarray(b1), 0) and np.allclose(np.asarray(b2), 0), \
        "nonzero biases not folded in this build"

    # padded, row-mapped, transposed x tiles
    x = np.asarray(x, np.float32)
    x_pad = np.zeros((c.RTOT, c.IN_C), np.float32)
    x_pad[rowmap] = x
    xT = x_pad.reshape(c.NT1, 128, c.IN_C).transpose(0, 2, 1)  # [t, k, n]
    xT = np.ascontiguousarray(xT).astype(bf16)

    iota = np.broadcast_to(np.arange(128, dtype=np.float32),
                           (128, 128)).astype(bf16).copy()
    iota_p = np.arange(128, dtype=np.float32).reshape(128, 1).astype(bf16)
    ones = np.ones((128, 1), np.float32)

    in_maps = []
    meta = dict(kA=kA, kB=kB, K=K)
    for ci in range(c.NCORES):
        idxA = _pack_idx(srow_slot[ci, :, :kA * 128].copy())         # [NBLK,128,kA*8]
        idxB = _pack_idx(srow_slot[ci, :, kA * 128:] - c.SPLIT)
        dr = din_slot[ci].reshape(c.NBLK, K, 128).transpose(2, 0, 1)  # [128,NBLK,K]
        drT = din_slot[ci].reshape(c.NBLK, K * 128)                   # [NBLK,K*128]
        xo = xT[ci * c.NBLK:(ci + 1) * c.NBLK]                       # own tiles
        in_maps.append({
            "x_t3": xT,
            "x_own": np.ascontiguousarray(xo),
            "w1aug": w1aug.astype(bf16),
            "w2aug": np.ascontiguousarray(w2aug.astype(bf16).reshape(c.C1 // 128, 128, 72).transpose(1, 0, 2)),
            "idxA": np.ascontiguousarray(idxA.transpose(1, 0, 2)),   # [128,NBLK,kA*8]
            "idxB": np.ascontiguousarray(idxB.transpose(1, 0, 2)),
            "dstrel": np.ascontiguousarray(dr).astype(bf16),
            "dstrelT": np.ascontiguousarray(drT).astype(bf16),
            "iota": iota,
            "iota_p": iota_p,
            "ones": ones,
        })
    return in_maps, meta


def build(cfg, kA, kB, core_id_split=None, debug=False, stage="F"):
    """stage: truncate program after phase A/B/C/D/E/F (for HW bisection)."""
    c = cfg
    K = kA + kB
    KH = c.C1 // 128                      # k-halves for layer-2 contraction
    nc = bacc.Bacc("TRN2", target_bir_lowering=False, debug=False,
                   num_devices=c.NCORES)

    # ---- IO ----
    x_t3_d = nc.dram_tensor("x_t3", [c.NT1, 128, c.IN_C], BF, kind="ExternalInput").ap()
    x_own_d = nc.dram_tensor("x_own", [c.NBLK, 128, c.IN_C], BF, kind="ExternalInput").ap()
    w1_d = nc.dram_tensor("w1aug", [c.IN_C, c.C1 + 8], BF, kind="ExternalInput").ap()
    w2_d = nc.dram_tensor("w2aug", [128, KH, 72], BF, kind="ExternalInput").ap()
    idxA_d = nc.dram_tensor("idxA", [128, c.NBLK, kA * 8], I16, kind="ExternalInput").ap()
    idxB_d = nc.dram_tensor("idxB", [128, c.NBLK, kB * 8], I16, kind="ExternalInput").ap()
    dstrel_d = nc.dram_tensor("dstrel", [128, c.NBLK, K], BF, kind="ExternalInput").ap()
    dstrelT_d = nc.dram_tensor("dstrelT", [c.NBLK, K * 128], BF, kind="ExternalInput").ap()
    iota_d = nc.dram_tensor("iota", [128, 128], BF, kind="ExternalInput").ap()
    iotap_d = nc.dram_tensor("iota_p", [128, 1], BF, kind="ExternalInput").ap()
    ones_d = nc.dram_tensor("ones", [128, 1], F32, kind="ExternalInput").ap()
    pool_d = nc.dram_tensor("pool64", [c.OUT_C, 1], F32, kind="ExternalOutput").ap()
    if debug:
        h1dbg_d = nc.dram_tensor("h1dbg", [c.NDP, c.C1], F32, kind="ExternalOutput").ap()
        h2dbg_d = nc.dram_tensor("h2dbg", [c.NDP, 72], F32, kind="ExternalOutput").ap()

    # ---- internal DRAM ----
    nsplit_t = c.SPLIT // 128
    t1A = nc.dram_tensor("t1A", [c.SPLIT, c.ROW1], FP8, kind="Internal").ap()
    t1B = nc.dram_tensor("t1B", [c.RTOT - c.SPLIT, c.ROW1], FP8, kind="Internal").ap()
    h1d = nc.dram_tensor("h1d", [c.NDP, c.C1], BF, kind="Internal").ap()
    cc3in = nc.dram_tensor("cc3in", [c.NDP, c.ROW2], BF, kind="Internal").ap()
    cc3 = nc.dram_tensor("cc3", [c.RTOT, c.ROW2], BF, kind="Internal",
                         addr_space="Shared").ap()

    with tile.TileContext(nc) as tc:
        with tc.tile_pool(name="const", bufs=1) as cpool, \
             tc.tile_pool(name="pa", bufs=3) as pa, \
             tc.tile_pool(name="pp", bufs=2, space="PSUM") as pp, \
             tc.tile_pool(name="pg", bufs=3) as pg, \
             tc.tile_pool(name="pe2", bufs=3) as pe2, \
             tc.tile_pool(name="sm", bufs=3) as sm:

            # constants resident
            w1s = cpool.tile_from(w1_d)                     # [128, C1+8]
            w2s = cpool.tile_from(w2_d)                     # [128, KH, 72]
            iota_s = cpool.tile_from(iota_d)
            iotap_s = cpool.tile_from(iotap_d)
            ones_s = cpool.tile_from(ones_d)
            idxA_s = cpool.tile_from(idxA_d)
            idxB_s = cpool.tile_from(idxB_d)
            dstrel_s = cpool.tile_from(dstrel_d)

            # ================= phase A: h table (replicated) =================
            CH = 4
            for t0 in range(0, c.NT1, CH):
                xt = pa.tile([128, CH, c.IN_C], BF, tag="xt")
                nc.sync.dma_start(
                    out=xt[:], in_=x_t3_d[t0:t0 + CH, :, :].rearrange("a k n -> k a n"))
                ob = pa.tile([128, CH, c.AUX1], FP8, tag="ob")
                for i in range(CH):
                    ps = pp.tile([128, c.C1 + 8], F32, tag="A")
                    nc.tensor.matmul(out=ps[:], lhsT=xt[:, i, :], rhs=w1s[:],
                                     start=True, stop=True)
                    obv = ob[:, i, 0:4 * c.HW1].rearrange("p (h q) -> p h q",
                                                          q=c.HW1)
                    psv = ps[:, 0:c.C1].rearrange("p (h q) -> p h q", q=c.HID)
                    if i % 2 == 0:
                        nc.scalar.activation(out=obv[:, :, 0:c.HID], in_=psv,
                                             func=AF.Copy)
                        nc.vector.tensor_copy(
                            out=ob[:, i, 4 * c.HW1:4 * c.HW1 + 16].bitcast(BF),
                            in_=ps[:, c.C1:c.C1 + 8])
                        nc.vector.tensor_scalar(
                            out=obv[:, :, c.HID:c.HW1], in0=psv[:, :, 0:1],
                            scalar1=0.0, scalar2=1.0, op0=AL.mult, op1=AL.add)
                    else:
                        nc.vector.tensor_copy(out=obv[:, :, 0:c.HID], in_=psv)
                        nc.scalar.activation(
                            out=ob[:, i, 4 * c.HW1:4 * c.HW1 + 16].bitcast(BF),
                            in_=ps[:, c.C1:c.C1 + 8], func=AF.Copy)
                        nc.vector.tensor_scalar(
                            out=obv[:, :, c.HID:c.HW1], in0=psv[:, :, 0:1],
                            scalar1=0.0, scalar2=1.0, op0=AL.mult, op1=AL.add)
                r0 = t0 * 128
                if t0 < nsplit_t:
                    dstv = t1A[r0:r0 + CH * 128, 0:c.AUX1]
                else:
                    dstv = t1B[r0 - c.SPLIT:r0 - c.SPLIT + CH * 128, 0:c.AUX1]
                nc.sync.dma_start(
                    out=dstv.rearrange("(a p) q -> p a q", p=128), in_=ob[:])

            # ================= phase B: layer-1 edge aggregation =============
            for b in (range(c.NBLK) if stage >= "B" else []):
                hg = pg.tile([128, K, c.ROW1], FP8, tag="hg")
                nc.gpsimd.dma_gather(
                    out_ap=hg[:, 0:kA, :], in_ap=t1A[:, :],
                    idxs_ap=idxA_s[:, b, :], num_idxs=kA * 128,
                    num_idxs_reg=kA * 128, elem_size=c.ROW1, single_packet=False)
                nc.gpsimd.dma_gather(
                    out_ap=hg[:, kA:K, :], in_ap=t1B[:, :],
                    idxs_ap=idxB_s[:, b, :], num_idxs=kB * 128,
                    num_idxs_reg=kB * 128, elem_size=c.ROW1, single_packet=False)

                # transposed one-hot mask (dst-major) for alpha_dst broadcast
                dT = pg.tile([128, K * 128], BF, tag="dT")
                nc.sync.dma_start(
                    out=dT[:], in_=dstrelT_d[b:b + 1, :].to_broadcast([128, K * 128]))
                maskT = pg.tile([128, K, 128], BF, tag="mT")
                nc.vector.tensor_tensor(
                    out=maskT[:],
                    in0=iotap_s[:, :, None].to_broadcast([128, K, 128]),
                    in1=dT[:].rearrange("p (k s) -> p k s", k=K),
                    op=AL.is_equal)
                # block's own alpha_dst rows (on-the-fly from own x tile)
                xo = pa.tile([128, c.IN_C], BF, tag="xo")
                nc.sync.dma_start(out=xo[:], in_=x_own_d[b, :, :])
                pso = pp.tile([128, K * 4], F32, tag="L")
                nc.tensor.matmul(out=pso[:, 0:8], lhsT=xo[:],
                                 rhs=w1s[:, c.C1:c.C1 + 8],
                                 start=True, stop=True)
                adb = sm.tile([128, 4], BF, tag="adb")
                nc.scalar.activation(out=adb[:], in_=pso[:, 4:8], func=AF.Copy)
                psL = pp.tile([128, K * 4], F32, tag="L")
                for j in range(K):
                    nc.tensor.matmul(out=psL[:, j * 4:(j + 1) * 4],
                                     lhsT=maskT[:, j, :],
                                     rhs=adb[:],
                                     start=True, stop=True)

                z = sm.tile([128, K, c.HEADS], F32, tag="z")
                nc.vector.tensor_tensor(
                    out=z[:],
                    in0=hg[:, :, 4 * c.HW1:4 * c.HW1 + 8].bitcast(BF),
                    in1=psL[:].rearrange("p (k h) -> p k h", k=K), op=AL.add)
                lr = sm.tile([128, K, c.HEADS], F32, tag="lr")
                nc.vector.scalar_tensor_tensor(
                    out=lr[:], in0=z[:], scalar=c.NEG, in1=z[:],
                    op0=AL.mult, op1=AL.max)
                eeb = sm.tile([128, K, c.HEADS], BF, tag="eeb")
                nc.scalar.activation(out=eeb[:], in_=lr[:], func=AF.Exp)

                mask = pg.tile([128, K, 128], BF, tag="mask")
                nc.vector.tensor_tensor(
                    out=mask[:],
                    in0=iota_s[:, None, :].to_broadcast([128, K, 128]),
                    in1=dstrel_s[:, b, :, None].to_broadcast([128, K, 128]),
                    op=AL.is_equal)

                v = pg.tile([128, K, 4 * c.HW1], BF, tag="v")
                nc.vector.tensor_tensor(
                    out=v[:].rearrange("p k (h q) -> p k h q", h=c.HEADS),
                    in0=hg[:, :, 0:4 * c.HW1].rearrange("p k (h q) -> p k h q",
                                                        h=c.HEADS),
                    in1=eeb[:, :, :, None].to_broadcast([128, K, c.HEADS, c.HW1]),
                    op=AL.mult)

                ps = pp.tile([128, 4 * c.HW1], F32, tag="B")
                for j in range(K):
                    nc.tensor.matmul(out=ps[:], lhsT=mask[:, j, :],
                                     rhs=v[:, j, :],
                                     start=(j == 0), stop=(j == K - 1))

                den = sm.tile([128, c.HEADS], F32, tag="den")
                nc.vector.tensor_scalar(
                    out=den[:, :, None],
                    in0=ps[:].rearrange("p (h q) -> p h q",
                                        q=c.HW1)[:, :, c.HID:c.HW1],
                    scalar1=1e-16, scalar2=None, op0=AL.add)
                rec = sm.tile([128, c.HEADS], F32, tag="rec")
                nc.vector.reciprocal(out=rec[:], in_=den[:])
                h1b = sm.tile([128, c.C1], BF, tag="h1b")
                for hh in range(c.HEADS):
                    nc.scalar.activation(
                        out=h1b[:, hh * c.HID:(hh + 1) * c.HID],
                        in_=ps[:, hh * c.HW1:hh * c.HW1 + c.HID],
                        func=AF.Relu, scale=rec[:, hh:hh + 1])
                nc.sync.dma_start(out=h1d[b * 128:(b + 1) * 128, :], in_=h1b[:])
                if debug:
                    h1dbgf = sm.tile([128, c.C1], F32, tag="h1dbgf")
                    nc.vector.tensor_copy(out=h1dbgf[:], in_=h1b[:])
                    nc.sync.dma_start(out=h1dbg_d[b * 128:(b + 1) * 128, :],
                                      in_=h1dbgf[:])

                # ---- fused phase C: h2_pre = relu(h1) @ W2aug ----
                if stage >= "C":
                    psc = pp.tile([128, c.C1 + 8], F32, tag="A")
                    for kh in range(KH):
                        ht = pa.tile([128, 128], BF, tag="ht")
                        nc.sync.dma_start(
                            out=ht[:], in_=h1d[b * 128:(b + 1) * 128,
                                               kh * 128:(kh + 1) * 128],
                            transpose=True)
                        nc.tensor.matmul(out=psc[:, 0:72], lhsT=ht[:],
                                         rhs=w2s[:, kh, :],
                                         start=(kh == 0), stop=(kh == KH - 1))
                    hc = pa.tile([128, c.ROW2], BF, tag="hc")
                    nc.vector.memset(hc[:, 67:c.ROW2], 0.0)
                    nc.vector.memset(hc[:, c.OUT_C:c.OUT_C + 1], 1.0)
                    nc.vector.tensor_copy(out=hc[:, 0:c.OUT_C],
                                          in_=psc[:, 0:c.OUT_C])
                    nc.vector.tensor_copy(out=hc[:, 65:67],
                                          in_=psc[:, c.OUT_C:c.OUT_C + 2])
                    nc.sync.dma_start(out=cc3in[b * 128:(b + 1) * 128, :],
                                      in_=hc[:])
                    if debug:
                        h2f = pa.tile([128, 72], F32, tag="h2f")
                        nc.vector.tensor_copy(out=h2f[:], in_=psc[:, 0:72])
                        nc.sync.dma_start(out=h2dbg_d[b * 128:(b + 1) * 128, :],
                                          in_=h2f[:])

            # ================= phase D: allgather + repack ===================
            if stage >= "D":
                nc.gpsimd.collective_compute(
                    kind="AllGather", op=AL.bypass,
                    replica_groups=[list(range(c.NCORES))],
                    ins=[cc3in[:, :]], outs=[cc3[:, :]])

            # block-own alpha_dst2 column, resident for phase E
            a2dS = cpool.tile([128, c.NBLK], BF)
            if stage >= "D":
                nc.sync.dma_start(
                    out=a2dS[:],
                    in_=cc3in[0:c.NDP, 66:67].rearrange(
                        "(b p) q -> p (b q)", p=128))
            else:
                nc.vector.memset(a2dS[:], 0.0)

            # ================= phase E: layer-2 edge aggregation =============
            pacc = cpool.tile([128, c.OUT_C], F32)
            nc.vector.memset(pacc[:], 0.0)
            for b in (range(c.NBLK) if stage >= "E" else []):
                hg2 = pe2.tile([128, K, c.ROW2], BF, tag="hg2")
                nc.gpsimd.dma_gather(
                    out_ap=hg2[:, 0:kA, :], in_ap=cc3[:, :],
                    idxs_ap=idxA_s[:, b, :], num_idxs=kA * 128,
                    num_idxs_reg=kA * 128, elem_size=c.ROW2, single_packet=False)
                nc.gpsimd.dma_gather(
                    out_ap=hg2[:, kA:K, :], in_ap=cc3[c.SPLIT:c.RTOT, :],
                    idxs_ap=idxB_s[:, b, :], num_idxs=kB * 128,
                    num_idxs_reg=kB * 128, elem_size=c.ROW2, single_packet=False)

                dT2 = pe2.tile([128, K * 128], BF, tag="dT2")
                nc.sync.dma_start(
                    out=dT2[:], in_=dstrelT_d[b:b + 1, :].to_broadcast([128, K * 128]))
                maskT2 = pe2.tile([128, K, 128], BF, tag="mT2")
                nc.vector.tensor_tensor(
                    out=maskT2[:],
                    in0=iotap_s[:, :, None].to_broadcast([128, K, 128]),
                    in1=dT2[:].rearrange("p (k s) -> p k s", k=K),
                    op=AL.is_equal)
                psL2 = pp.tile([128, K * 4], F32, tag="L")
                for j in range(K):
                    nc.tensor.matmul(out=psL2[:, j:j + 1],
                                     lhsT=maskT2[:, j, :],
                                     rhs=a2dS[:, b:b + 1],
                                     start=True, stop=True)

                z2 = sm.tile([128, K], F32, tag="z2")
                nc.vector.tensor_tensor(
                    out=z2[:],
                    in0=hg2[:, :, 65:66].rearrange("p k q -> p (k q)"),
                    in1=psL2[:, 0:K], op=AL.add)
                lr2 = sm.tile([128, K], F32, tag="lr2")
                nc.vector.scalar_tensor_tensor(
                    out=lr2[:], in0=z2[:], scalar=c.NEG, in1=z2[:],
                    op0=AL.mult, op1=AL.max)
                ee2 = sm.tile([128, K], BF, tag="ee2")
                nc.scalar.activation(out=ee2[:], in_=lr2[:], func=AF.Exp)

                mask2 = pe2.tile([128, K, 128], BF, tag="mask2")
                nc.vector.tensor_tensor(
                    out=mask2[:],
                    in0=iota_s[:, None, :].to_broadcast([128, K, 128]),
                    in1=dstrel_s[:, b, :, None].to_broadcast([128, K, 128]),
                    op=AL.is_equal)

                v2 = pe2.tile([128, K, c.OUT_C + 1], BF, tag="v2")
                nc.vector.tensor_tensor(
                    out=v2[:], in0=hg2[:, :, 0:c.OUT_C + 1],
                    in1=ee2[:, :, None].to_broadcast([128, K, c.OUT_C + 1]),
                    op=AL.mult)

                ps2 = pp.tile([128, c.OUT_C + 1], F32, tag="E")
                for j in range(K):
                    nc.tensor.matmul(out=ps2[:], lhsT=mask2[:, j, :],
                                     rhs=v2[:, j, :],
                                     start=(j == 0), stop=(j == K - 1))

                den2 = sm.tile([128, 1], F32, tag="den2")
                nc.vector.tensor_scalar(
                    out=den2[:], in0=ps2[:, c.OUT_C:c.OUT_C + 1], scalar1=1e-16,
                    scalar2=None, op0=AL.add)
                rec2 = sm.tile([128, 1], F32, tag="rec2")
                nc.vector.reciprocal(out=rec2[:], in_=den2[:])
                o2r = sm.tile([128, c.OUT_C], F32, tag="o2r")
                nc.scalar.activation(out=o2r[:], in_=ps2[:, 0:c.OUT_C],
                                     func=AF.Relu, scale=rec2[:])
                nc.vector.tensor_tensor(out=pacc[:], in0=pacc[:], in1=o2r[:],
                                        op=AL.add)

            # ================= phase F: pool partial =========================
            psf = pp.tile([c.OUT_C + 1, 1], F32, tag="E")
            nc.tensor.matmul(out=psf[0:c.OUT_C, :], lhsT=pacc[:], rhs=ones_s[:],
                             start=True, stop=True)
            pf = sm.tile([c.OUT_C, 1], F32, tag="pf")
            nc.vector.tensor_copy(out=pf[:], in_=psf[0:c.OUT_C, :])
            nc.sync.dma_start(out=pool_d[:, :], in_=pf[:])

    nc.compile()
    legalize_waits(nc)
    return nc


def legalize_waits(nc):
    """Walrus encodes at most ONE sync wait per instruction on this toolchain.
    Hoist excess waits onto same-engine NoOps inserted before the instruction."""
    for fn in nc.m.functions:
        for bb in fn.blocks:
            insts = list(bb.instructions)
            out = []
            changed = False
            for inst in insts:
                si = inst.sync_info
                if si is not None and si.on_wait and len(si.on_wait) > 1:
                    waits = list(si.on_wait)
                    for w in waits[:-1]:
                        nop = mybir.InstNoOp(
                            name=nc.get_next_instruction_name(), ins=[], outs=[])
                        nop.engine = inst.engine
                        nop.sync_info = mybir.SyncInfo(on_wait=[w], on_update=[])
                        nc.register_instruction(nop)
                        out.append(nop)
                    inst.sync_info = mybir.SyncInfo(
                        on_wait=waits[-1:], on_update=list(si.on_update))
                    changed = True
                out.append(inst)
            if changed:
                bb.instructions.clear()
                bb.instructions.extend(out)


def host_finish(cfg, pools, fc_w, fc_b):
    c = cfg
    tot = np.zeros(c.OUT_C, np.float64)
    for p in pools:
        tot += p[:, 0].astype(np.float64)
    pooled = (tot / c.N).astype(np.float32)
    logits = pooled @ np.asarray(fc_w, np.float32) + np.asarray(fc_b, np.float32)
    m = logits.max()
    ls = logits - (m + np.log(np.exp(logits - m).sum()))
    return ls.reshape(1, c.NCLS).astype(np.float32)


_BUILD_CACHE = {}


def run(cfg, inputs, debug=False, trace=False, **run_kwargs):
    in_maps, meta = host_prep(
        cfg, inputs["x"], inputs["edge_index"], inputs["W1"], inputs["att_src1"],
        inputs["att_dst1"], inputs["b1"], inputs["W2"], inputs["att_src2"],
        inputs["att_dst2"], inputs["b2"])
    stage = os.environ.get("KSTAGE", "F")
    key = (cfg.N, cfg.E, meta["kA"], meta["kB"], debug, stage)
    if key not in _BUILD_CACHE:
        _BUILD_CACHE[key] = build(cfg, meta["kA"], meta["kB"], debug=debug,
                                  stage=stage)
    nc = _BUILD_CACHE[key]
    res = bass_utils.run_bass_kernel_spmd(
        nc, in_maps, core_ids=list(range(cfg.NCORES)), trace=trace, **run_kwargs)
    out = host_finish(cfg, [r["pool64"] for r in res.results],
                      inputs["fc_w"], inputs["fc_b"])
    return out, res


def kernel(**inputs):
    cfg = Cfg()
    out, _ = run(cfg, inputs)
    return out


# revision 17
# speedup vs baseline: 1.2484x; 1.0594x over previous
"""GAT (2-layer graph attention network) Bass kernel for 8 Trainium2 NeuronCores.

Strategy (per spec sharding hint): edges are partitioned by destination-node
block so segment-softmax/sum stay core-local; each core owns N/8 destination
nodes. Layer-1 node features (h = x @ W1 plus attention alphas via an
augmented weight matrix) are computed replicated on every core into a DRAM
table with 512-byte rows: 256 fp8(e4m3) h channels + 4 bf16 alpha_src + 4
bf16 alpha_dst. Per-edge source rows arrive via one SWDGE gather per region
(A/B split for the int16 index range). The per-edge alpha_dst values are NOT
gathered: they are broadcast from the block's own alpha rows through a
transposed one-hot mask on the tensor engine (K tiny matmuls), which removes
one 256-B gather per edge versus the previous build. The segment-softmax +
weighted aggregation are fused into PE matmuls with a one-hot {edge x dst}
mask; PSUM accumulates numerator and denominator together. Layer-2
pre-features are exchanged with one AllGather; layer-2 aggregation repeats
the same structure with 256-B bf16 rows. Global mean-pool partials are
reduced on host along with the tiny fc + log_softmax head.
"""
import os
import sys
import types
import math

import numpy as np
import ml_dtypes


def _setup_paths():
    for p in ("/opt/trn_rl_repo", "/root/.axon_site/_ro/trn_rl_repo"):
        if os.path.isdir(p) and p not in sys.path:
            sys.path.insert(0, p)
    try:
        import concourse.bass  # noqa: F401
    except ImportError as e:
        raise RuntimeError(f"concourse not importable: {e}")


_setup_paths()

import concourse.bass as bass  # noqa: E402
import concourse.mybir as mybir  # noqa: E402
import concourse.tile as tile  # noqa: E402
from concourse import bacc, bass_utils  # noqa: E402

bf16 = ml_dtypes.bfloat16
BF = mybir.dt.bfloat16
F32 = mybir.dt.float32
I16 = mybir.dt.int16
FP8 = mybir.dt.float8e4
AL = mybir.AluOpType
AF = mybir.ActivationFunctionType


class Cfg:
    def __init__(self, N=50000, E=800000, IN_C=128, HID=64, OUT_C=64, HEADS=4,
                 NCLS=40, NEG=0.2, NCORES=8):
        self.N, self.E = N, E
        self.IN_C, self.HID, self.OUT_C, self.HEADS = IN_C, HID, OUT_C, HEADS
        self.NCLS, self.NEG, self.NCORES = NCLS, NEG, NCORES
        assert N % NCORES == 0
        self.NB = N // NCORES                      # owned real nodes per core
        self.NBLK = math.ceil(self.NB / 128)       # dst blocks per core
        self.NDP = self.NBLK * 128                 # padded owned rows per core
        self.RTOT = self.NDP * NCORES              # global padded row space
        assert self.RTOT % 128 == 0
        self.NT1 = self.RTOT // 128                # phase-A node tiles
        # A/B gather split (int16 row-index limit), multiple of 128
        self.SPLIT = min(32768, (self.RTOT // 2 + 127) // 128 * 128)
        assert self.SPLIT % 128 == 0 and self.SPLIT < 32768 + 1
        self.C1 = HEADS * HID                      # 256 layer-1 channels
        self.ROW1 = 512                            # table1 row bytes (fp8 units)
        self.AUX1 = 276                            # used bytes per table1 row
        self.HW1 = self.HID + 1                    # 65: head block + ones col
        self.ROW2 = 128                            # table3 cols (256B rows)
        assert self.OUT_C + 2 <= self.ROW2

    def row_of(self, v):
        return self.NDP * (v // self.NB) + (v % self.NB)


def _pack_idx(vals_2d):
    """vals_2d [G, n] -> dma_gather index layout [G, 128, n//16] int16.

    Index i lives at [i % 16, i // 16]; the 16-row group is replicated 8x
    across the 128 partitions.
    """
    G, n = vals_2d.shape
    assert n % 16 == 0
    a = vals_2d.reshape(G, n // 16, 16).transpose(0, 2, 1)   # [G, 16, n/16]
    return np.tile(a, (1, 8, 1)).astype(np.int16)            # [G, 128, n/16]


def host_prep(cfg, x, edge_index, W1, att_src1, att_dst1, b1, W2, att_src2,
              att_dst2, b2):
    """Build per-core in_maps (everything except the graph-independent consts)."""
    c = cfg
    # self-loops are handled densely per block on-device (not slotted)
    src = np.asarray(edge_index[0], dtype=np.int64)
    dst = np.asarray(edge_index[1], dtype=np.int64)
    EE = src.shape[0]

    core = dst // c.NB
    # load-balanced permutation of each core's dst nodes into blocks so that
    # per-block A/B-region edge counts are even (minimizes gather slot count K)
    t0core = src // c.NB                           # provisional (pre-perm) side
    rowmap = np.zeros(c.N, np.int64)               # node -> global padded row
    for ci in range(c.NCORES):
        sel = core == ci
        dloc = dst[sel] - ci * c.NB
        srcA = (c.row_of(src[sel]) < c.SPLIT)      # approx region (pre-perm)
        degA = np.bincount(dloc[srcA], minlength=c.NB).astype(np.int64)
        degT = np.bincount(dloc, minlength=c.NB).astype(np.int64)
        degB = degT - degA
        order_d = np.argsort(-degT, kind="stable")
        cnt = np.zeros(c.NBLK, np.int64)
        lA = np.zeros(c.NBLK, np.float64)
        lB = np.zeros(c.NBLK, np.float64)
        perm = np.zeros(c.NB, np.int64)
        for v in order_d:
            score = np.maximum(lA + degA[v], lB + degB[v]) + 1e9 * (cnt >= 128)
            j = int(np.argmin(score))
            perm[v] = j * 128 + cnt[j]
            cnt[j] += 1
            lA[j] += degA[v]
            lB[j] += degB[v]
        rowmap[ci * c.NB:(ci + 1) * c.NB] = ci * c.NDP + perm
    drow = rowmap[dst]
    blk = (drow - core * c.NDP) // 128
    din = (drow - core * c.NDP) % 128              # dst index within block
    srow = rowmap[src]
    isB = (srow >= c.SPLIT).astype(np.int64)

    gid = (core * c.NBLK + blk) * 2 + isB          # group id (A/B separate)
    order = np.argsort(gid, kind="stable")
    gid_s = gid[order]
    counts = np.bincount(gid_s, minlength=c.NCORES * c.NBLK * 2)
    nA = counts[0::2].reshape(c.NCORES, c.NBLK)
    nB = counts[1::2].reshape(c.NCORES, c.NBLK)
    kA = max(1, int(math.ceil(nA.max() / 128)))
    kB = max(1, int(math.ceil(nB.max() / 128)))
    K = kA + kB

    # rank within group
    starts = np.zeros_like(counts)
    starts[1:] = np.cumsum(counts)[:-1]
    rank = np.arange(EE) - starts[gid_s]

    # destination slot within the (core, blk) slot array of length K*128
    slot = np.where(isB[order] == 0, rank, kA * 128 + rank)
    cg = core[order] * c.NBLK + blk[order]          # [EE] group (core, blk)

    srow_slot = np.zeros((c.NCORES * c.NBLK, K * 128), np.int64)
    srow_slot[:, kA * 128:] = c.SPLIT               # B-region pad -> idx 0
    din_slot = np.full((c.NCORES * c.NBLK, K * 128), 128.0, np.float32)
    srow_slot[cg, slot] = srow[order]
    din_slot[cg, slot] = din[order]

    srow_slot = srow_slot.reshape(c.NCORES, c.NBLK, K * 128)
    din_slot = din_slot.reshape(c.NCORES, c.NBLK, K * 128)

    # augmented weights
    W1 = np.asarray(W1, np.float32)
    a_s1 = np.asarray(att_src1, np.float32).reshape(c.HEADS, c.HID)
    a_d1 = np.asarray(att_dst1, np.float32).reshape(c.HEADS, c.HID)
    W1r = W1.reshape(c.IN_C, c.HEADS, c.HID)
    Wa_s = np.einsum("khc,hc->kh", W1r, a_s1)       # [IN_C, HEADS]
    Wa_d = np.einsum("khc,hc->kh", W1r, a_d1)
    w1aug = np.zeros((c.IN_C, c.C1 + 8), np.float32)
    w1aug[:, :c.C1] = W1
    w1aug[:, c.C1:c.C1 + c.HEADS] = Wa_s
    w1aug[:, c.C1 + 4:c.C1 + 4 + c.HEADS] = Wa_d

    W2 = np.asarray(W2, np.float32)
    a_s2 = np.asarray(att_src2, np.float32).reshape(c.OUT_C)
    a_d2 = np.asarray(att_dst2, np.float32).reshape(c.OUT_C)
    w2aug = np.zeros((c.C1, 72), np.float32)
    w2aug[:, :c.OUT_C] = W2
    w2aug[:, c.OUT_C] = W2 @ a_s2
    w2aug[:, c.OUT_C + 1] = W2 @ a_d2

    assert np.allclose(np.asarray(b1), 0) and np.allclose(np.asarray(b2), 0), \
        "nonzero biases not folded in this build"

    # padded, row-mapped, transposed x tiles
    x = np.asarray(x, np.float32)
    x_pad = np.zeros((c.RTOT, c.IN_C), np.float32)
    x_pad[rowmap] = x
    xT = x_pad.reshape(c.NT1, 128, c.IN_C).transpose(0, 2, 1)  # [t, k, n]
    xT = np.ascontiguousarray(xT).astype(bf16)

    iota = np.broadcast_to(np.arange(128, dtype=np.float32),
                           (128, 128)).astype(bf16).copy()
    iota_p = np.arange(128, dtype=np.float32).reshape(128, 1).astype(bf16)
    ones = np.ones((128, 1), np.float32)

    in_maps = []
    meta = dict(kA=kA, kB=kB, K=K)
    for ci in range(c.NCORES):
        idxA = _pack_idx(srow_slot[ci, :, :kA * 128].copy())         # [NBLK,128,kA*8]
        idxB = _pack_idx(srow_slot[ci, :, kA * 128:] - c.SPLIT)
        dr = din_slot[ci].reshape(c.NBLK, K, 128).transpose(2, 0, 1)  # [128,NBLK,K]
        drT = din_slot[ci].reshape(c.NBLK, K * 128)                   # [NBLK,K*128]
        xo = xT[ci * c.NBLK:(ci + 1) * c.NBLK]                       # own tiles
        in_maps.append({
            "x_t3": xT,
            "x_own": np.ascontiguousarray(xo),
            "w1aug": w1aug.astype(bf16),
            "w2aug": np.ascontiguousarray(w2aug.astype(bf16).reshape(c.C1 // 128, 128, 72).transpose(1, 0, 2)),
            "idxA": np.ascontiguousarray(idxA.transpose(1, 0, 2)),   # [128,NBLK,kA*8]
            "idxB": np.ascontiguousarray(idxB.transpose(1, 0, 2)),
            "dstrel": np.ascontiguousarray(dr).astype(bf16),
            "dstrelT": np.ascontiguousarray(drT).astype(bf16),
            "iota": iota,
            "iota_p": iota_p,
            "ones": ones,
        })
    return in_maps, meta


def build(cfg, kA, kB, core_id_split=None, debug=False, stage="F"):
    """stage: truncate program after phase A/B/C/D/E/F (for HW bisection)."""
    c = cfg
    K = kA + kB
    KH = c.C1 // 128                      # k-halves for layer-2 contraction
    nc = bacc.Bacc("TRN2", target_bir_lowering=False, debug=False,
                   num_devices=c.NCORES)

    # ---- IO ----
    x_t3_d = nc.dram_tensor("x_t3", [c.NT1, 128, c.IN_C], BF, kind="ExternalInput").ap()
    x_own_d = nc.dram_tensor("x_own", [c.NBLK, 128, c.IN_C], BF, kind="ExternalInput").ap()
    w1_d = nc.dram_tensor("w1aug", [c.IN_C, c.C1 + 8], BF, kind="ExternalInput").ap()
    w2_d = nc.dram_tensor("w2aug", [128, KH, 72], BF, kind="ExternalInput").ap()
    idxA_d = nc.dram_tensor("idxA", [128, c.NBLK, kA * 8], I16, kind="ExternalInput").ap()
    idxB_d = nc.dram_tensor("idxB", [128, c.NBLK, kB * 8], I16, kind="ExternalInput").ap()
    dstrel_d = nc.dram_tensor("dstrel", [128, c.NBLK, K], BF, kind="ExternalInput").ap()
    dstrelT_d = nc.dram_tensor("dstrelT", [c.NBLK, K * 128], BF, kind="ExternalInput").ap()
    iota_d = nc.dram_tensor("iota", [128, 128], BF, kind="ExternalInput").ap()
    iotap_d = nc.dram_tensor("iota_p", [128, 1], BF, kind="ExternalInput").ap()
    ones_d = nc.dram_tensor("ones", [128, 1], F32, kind="ExternalInput").ap()
    pool_d = nc.dram_tensor("pool64", [c.OUT_C, 1], F32, kind="ExternalOutput").ap()
    if debug:
        h1dbg_d = nc.dram_tensor("h1dbg", [c.NDP, c.C1], F32, kind="ExternalOutput").ap()
        h2dbg_d = nc.dram_tensor("h2dbg", [c.NDP, 72], F32, kind="ExternalOutput").ap()

    # ---- internal DRAM ----
    nsplit_t = c.SPLIT // 128
    t1A = nc.dram_tensor("t1A", [c.SPLIT, c.ROW1], FP8, kind="Internal").ap()
    t1B = nc.dram_tensor("t1B", [c.RTOT - c.SPLIT, c.ROW1], FP8, kind="Internal").ap()
    h1d = nc.dram_tensor("h1d", [c.NDP, c.C1], BF, kind="Internal").ap()
    cc3in = nc.dram_tensor("cc3in", [c.NDP, c.ROW2], BF, kind="Internal").ap()
    cc3 = nc.dram_tensor("cc3", [c.RTOT, c.ROW2], BF, kind="Internal",
                         addr_space="Shared").ap()

    with tile.TileContext(nc) as tc:
        with tc.tile_pool(name="const", bufs=1) as cpool, \
             tc.tile_pool(name="pa", bufs=4) as pa, \
             tc.tile_pool(name="pp", bufs=2, space="PSUM") as pp, \
             tc.tile_pool(name="pg", bufs=3) as pg, \
             tc.tile_pool(name="pe2", bufs=3) as pe2, \
             tc.tile_pool(name="sm", bufs=3) as sm:

            # constants resident
            w1s = cpool.tile_from(w1_d)                     # [128, C1+8]
            w2s = cpool.tile_from(w2_d)                     # [128, KH, 72]
            iota_s = cpool.tile_from(iota_d)
            iotap_s = cpool.tile_from(iotap_d)
            ones_s = cpool.tile_from(ones_d)
            idxA_s = cpool.tile_from(idxA_d)
            idxB_s = cpool.tile_from(idxB_d)
            dstrel_s = cpool.tile_from(dstrel_d)

            # ================= phase A: h table (replicated) =================
            CH = 4
            for t0 in range(0, c.NT1, CH):
                xt = pa.tile([128, CH, c.IN_C], BF, tag="xt")
                nc.sync.dma_start(
                    out=xt[:], in_=x_t3_d[t0:t0 + CH, :, :].rearrange("a k n -> k a n"))
                ob = pa.tile([128, CH, c.AUX1], FP8, tag="ob")
                for i in range(CH):
                    ps = pp.tile([128, c.C1 + 8], F32, tag="A")
                    nc.tensor.matmul(out=ps[:], lhsT=xt[:, i, :], rhs=w1s[:],
                                     start=True, stop=True)
                    obv = ob[:, i, 0:4 * c.HW1].rearrange("p (h q) -> p h q",
                                                          q=c.HW1)
                    psv = ps[:, 0:c.C1].rearrange("p (h q) -> p h q", q=c.HID)
                    if i % 2 == 0:
                        nc.scalar.activation(out=obv[:, :, 0:c.HID], in_=psv,
                                             func=AF.Copy)
                        nc.vector.tensor_copy(
                            out=ob[:, i, 4 * c.HW1:4 * c.HW1 + 16].bitcast(BF),
                            in_=ps[:, c.C1:c.C1 + 8])
                        nc.vector.tensor_scalar(
                            out=obv[:, :, c.HID:c.HW1], in0=psv[:, :, 0:1],
                            scalar1=0.0, scalar2=1.0, op0=AL.mult, op1=AL.add)
                    else:
                        nc.vector.tensor_copy(out=obv[:, :, 0:c.HID], in_=psv)
                        nc.scalar.activation(
                            out=ob[:, i, 4 * c.HW1:4 * c.HW1 + 16].bitcast(BF),
                            in_=ps[:, c.C1:c.C1 + 8], func=AF.Copy)
                        nc.vector.tensor_scalar(
                            out=obv[:, :, c.HID:c.HW1], in0=psv[:, :, 0:1],
                            scalar1=0.0, scalar2=1.0, op0=AL.mult, op1=AL.add)
                r0 = t0 * 128
                if t0 < nsplit_t:
                    dstv = t1A[r0:r0 + CH * 128, 0:c.AUX1]
                else:
                    dstv = t1B[r0 - c.SPLIT:r0 - c.SPLIT + CH * 128, 0:c.AUX1]
                nc.sync.dma_start(
                    out=dstv.rearrange("(a p) q -> p a q", p=128), in_=ob[:])

            # ================= phase B: layer-1 edge aggregation =============
            for b in (range(c.NBLK) if stage >= "B" else []):
                hg = pg.tile([128, K, c.ROW1], FP8, tag="hg")
                nc.gpsimd.dma_gather(
                    out_ap=hg[:, 0:kA, :], in_ap=t1A[:, :],
                    idxs_ap=idxA_s[:, b, :], num_idxs=kA * 128,
                    num_idxs_reg=kA * 128, elem_size=c.ROW1, single_packet=False)
                nc.gpsimd.dma_gather(
                    out_ap=hg[:, kA:K, :], in_ap=t1B[:, :],
                    idxs_ap=idxB_s[:, b, :], num_idxs=kB * 128,
                    num_idxs_reg=kB * 128, elem_size=c.ROW1, single_packet=False)

                # transposed one-hot mask (dst-major) for alpha_dst broadcast
                dT = pg.tile([128, K * 128], BF, tag="dT")
                nc.sync.dma_start(
                    out=dT[:], in_=dstrelT_d[b:b + 1, :].to_broadcast([128, K * 128]))
                maskT = pg.tile([128, K, 128], BF, tag="mT")
                nc.vector.tensor_tensor(
                    out=maskT[:],
                    in0=iotap_s[:, :, None].to_broadcast([128, K, 128]),
                    in1=dT[:].rearrange("p (k s) -> p k s", k=K),
                    op=AL.is_equal)
                # block's own alpha_dst rows (on-the-fly from own x tile)
                xo = pa.tile([128, c.IN_C], BF, tag="xo")
                nc.sync.dma_start(out=xo[:], in_=x_own_d[b, :, :])
                pso = pp.tile([128, c.C1 + 8], F32, tag="L")
                nc.tensor.matmul(out=pso[:], lhsT=xo[:], rhs=w1s[:],
                                 start=True, stop=True)
                adb = sm.tile([128, 4], BF, tag="adb")
                nc.scalar.activation(out=adb[:], in_=pso[:, c.C1 + 4:c.C1 + 8],
                                     func=AF.Copy)
                als = sm.tile([128, 8], F32, tag="als")
                nc.scalar.activation(out=als[:], in_=pso[:, c.C1:c.C1 + 8],
                                     func=AF.Copy)
                psL = pp.tile([128, c.C1 + 8], F32, tag="L")
                for j in range(K):
                    nc.tensor.matmul(out=psL[:, j * 4:(j + 1) * 4],
                                     lhsT=maskT[:, j, :],
                                     rhs=adb[:],
                                     start=True, stop=True)

                z = sm.tile([128, K, c.HEADS], F32, tag="z")
                nc.vector.tensor_tensor(
                    out=z[:],
                    in0=hg[:, :, 4 * c.HW1:4 * c.HW1 + 8].bitcast(BF),
                    in1=psL[:, 0:K * 4].rearrange("p (k h) -> p k h", k=K),
                    op=AL.add)
                lr = sm.tile([128, K, c.HEADS], F32, tag="lr")
                nc.vector.scalar_tensor_tensor(
                    out=lr[:], in0=z[:], scalar=c.NEG, in1=z[:],
                    op0=AL.mult, op1=AL.max)
                eeb = sm.tile([128, K, c.HEADS], BF, tag="eeb")
                nc.scalar.activation(out=eeb[:], in_=lr[:], func=AF.Exp)

                mask = pg.tile([128, K, 128], BF, tag="mask")
                nc.vector.tensor_tensor(
                    out=mask[:],
                    in0=iota_s[:, None, :].to_broadcast([128, K, 128]),
                    in1=dstrel_s[:, b, :, None].to_broadcast([128, K, 128]),
                    op=AL.is_equal)

                v = pg.tile([128, K, 4 * c.HW1], BF, tag="v")
                nc.vector.tensor_tensor(
                    out=v[:].rearrange("p k (h q) -> p k h q", h=c.HEADS),
                    in0=hg[:, :, 0:4 * c.HW1].rearrange("p k (h q) -> p k h q",
                                                        h=c.HEADS),
                    in1=eeb[:, :, :, None].to_broadcast([128, K, c.HEADS, c.HW1]),
                    op=AL.mult)

                ps = pp.tile([128, 4 * c.HW1], F32, tag="B")
                for j in range(K):
                    nc.tensor.matmul(out=ps[:], lhsT=mask[:, j, :],
                                     rhs=v[:, j, :],
                                     start=(j == 0), stop=(j == K - 1))

                # dense self-loop contribution (own rows, partition = dst)
                zs = sm.tile([128, c.HEADS], F32, tag="zs")
                nc.vector.tensor_tensor(
                    out=zs[:], in0=als[:, 0:4], in1=als[:, 4:8], op=AL.add)
                lrs = sm.tile([128, c.HEADS], F32, tag="lrs")
                nc.vector.scalar_tensor_tensor(
                    out=lrs[:], in0=zs[:], scalar=c.NEG, in1=zs[:],
                    op0=AL.mult, op1=AL.max)
                ees = sm.tile([128, c.HEADS], BF, tag="ees")
                nc.scalar.activation(out=ees[:], in_=lrs[:], func=AF.Exp)
                vself = sm.tile([128, 4 * c.HW1], BF, tag="vself")
                vsv = vself[:].rearrange("p (h q) -> p h q", q=c.HW1)
                nc.vector.tensor_tensor(
                    out=vsv[:, :, 0:c.HID],
                    in0=pso[:, 0:c.C1].rearrange("p (h q) -> p h q", q=c.HID),
                    in1=ees[:, :, None].to_broadcast([128, c.HEADS, c.HID]),
                    op=AL.mult)
                nc.vector.tensor_copy(out=vsv[:, :, c.HID:c.HW1],
                                      in_=ees[:, :, None])
                nc.vector.tensor_tensor(out=ps[:], in0=ps[:], in1=vself[:],
                                        op=AL.add)

                den = sm.tile([128, c.HEADS], F32, tag="den")
                nc.vector.tensor_scalar(
                    out=den[:, :, None],
                    in0=ps[:].rearrange("p (h q) -> p h q",
                                        q=c.HW1)[:, :, c.HID:c.HW1],
                    scalar1=1e-16, scalar2=None, op0=AL.add)
                rec = sm.tile([128, c.HEADS], F32, tag="rec")
                nc.vector.reciprocal(out=rec[:], in_=den[:])
                h1b = sm.tile([128, c.C1], BF, tag="h1b")
                for hh in range(c.HEADS):
                    nc.scalar.activation(
                        out=h1b[:, hh * c.HID:(hh + 1) * c.HID],
                        in_=ps[:, hh * c.HW1:hh * c.HW1 + c.HID],
                        func=AF.Relu, scale=rec[:, hh:hh + 1])
                nc.sync.dma_start(out=h1d[b * 128:(b + 1) * 128, :], in_=h1b[:])
                if debug:
                    h1dbgf = sm.tile([128, c.C1], F32, tag="h1dbgf")
                    nc.vector.tensor_copy(out=h1dbgf[:], in_=h1b[:])
                    nc.sync.dma_start(out=h1dbg_d[b * 128:(b + 1) * 128, :],
                                      in_=h1dbgf[:])

                # ---- fused phase C: h2_pre = relu(h1) @ W2aug ----
                if stage >= "C":
                    psc = pp.tile([128, c.C1 + 8], F32, tag="A")
                    for kh in range(KH):
                        ht = pa.tile([128, 128], BF, tag="ht")
                        nc.sync.dma_start(
                            out=ht[:], in_=h1d[b * 128:(b + 1) * 128,
                                               kh * 128:(kh + 1) * 128],
                            transpose=True)
                        nc.tensor.matmul(out=psc[:, 0:72], lhsT=ht[:],
                                         rhs=w2s[:, kh, :],
                                         start=(kh == 0), stop=(kh == KH - 1))
                    hc = pa.tile([128, c.ROW2], BF, tag="hc")
                    nc.vector.memset(hc[:, 67:c.ROW2], 0.0)
                    nc.vector.memset(hc[:, c.OUT_C:c.OUT_C + 1], 1.0)
                    nc.vector.tensor_copy(out=hc[:, 0:c.OUT_C],
                                          in_=psc[:, 0:c.OUT_C])
                    nc.vector.tensor_copy(out=hc[:, 65:67],
                                          in_=psc[:, c.OUT_C:c.OUT_C + 2])
                    nc.sync.dma_start(out=cc3in[b * 128:(b + 1) * 128, :],
                                      in_=hc[:])
                    if debug:
                        h2f = pa.tile([128, 72], F32, tag="h2f")
                        nc.vector.tensor_copy(out=h2f[:], in_=psc[:, 0:72])
                        nc.sync.dma_start(out=h2dbg_d[b * 128:(b + 1) * 128, :],
                                          in_=h2f[:])

            # ================= phase D: allgather + repack ===================
            if stage >= "D":
                nc.gpsimd.collective_compute(
                    kind="AllGather", op=AL.bypass,
                    replica_groups=[list(range(c.NCORES))],
                    ins=[cc3in[:, :]], outs=[cc3[:, :]])

            # block-own alpha_dst2 column, resident for phase E
            a2dS = cpool.tile([128, c.NBLK], BF)
            if stage >= "D":
                nc.sync.dma_start(
                    out=a2dS[:],
                    in_=cc3in[0:c.NDP, 66:67].rearrange(
                        "(b p) q -> p (b q)", p=128))
            else:
                nc.vector.memset(a2dS[:], 0.0)

            # ================= phase E: layer-2 edge aggregation =============
            pacc = cpool.tile([128, c.OUT_C], F32)
            nc.vector.memset(pacc[:], 0.0)
            for b in (range(c.NBLK) if stage >= "E" else []):
                hg2 = pe2.tile([128, K, c.ROW2], BF, tag="hg2")
                nc.gpsimd.dma_gather(
                    out_ap=hg2[:, 0:kA, :], in_ap=cc3[:, :],
                    idxs_ap=idxA_s[:, b, :], num_idxs=kA * 128,
                    num_idxs_reg=kA * 128, elem_size=c.ROW2, single_packet=False)
                nc.gpsimd.dma_gather(
                    out_ap=hg2[:, kA:K, :], in_ap=cc3[c.SPLIT:c.RTOT, :],
                    idxs_ap=idxB_s[:, b, :], num_idxs=kB * 128,
                    num_idxs_reg=kB * 128, elem_size=c.ROW2, single_packet=False)

                dT2 = pe2.tile([128, K * 128], BF, tag="dT2")
                nc.sync.dma_start(
                    out=dT2[:], in_=dstrelT_d[b:b + 1, :].to_broadcast([128, K * 128]))
                maskT2 = pe2.tile([128, K, 128], BF, tag="mT2")
                nc.vector.tensor_tensor(
                    out=maskT2[:],
                    in0=iotap_s[:, :, None].to_broadcast([128, K, 128]),
                    in1=dT2[:].rearrange("p (k s) -> p k s", k=K),
                    op=AL.is_equal)
                psL2 = pp.tile([128, K * 4], F32, tag="L")
                for j in range(K):
                    nc.tensor.matmul(out=psL2[:, j:j + 1],
                                     lhsT=maskT2[:, j, :],
                                     rhs=a2dS[:, b:b + 1],
                                     start=True, stop=True)

                z2 = sm.tile([128, K], F32, tag="z2")
                nc.vector.tensor_tensor(
                    out=z2[:],
                    in0=hg2[:, :, 65:66].rearrange("p k q -> p (k q)"),
                    in1=psL2[:, 0:K], op=AL.add)
                lr2 = sm.tile([128, K], F32, tag="lr2")
                nc.vector.scalar_tensor_tensor(
                    out=lr2[:], in0=z2[:], scalar=c.NEG, in1=z2[:],
                    op0=AL.mult, op1=AL.max)
                ee2 = sm.tile([128, K], BF, tag="ee2")
                nc.scalar.activation(out=ee2[:], in_=lr2[:], func=AF.Exp)

                mask2 = pe2.tile([128, K, 128], BF, tag="mask2")
                nc.vector.tensor_tensor(
                    out=mask2[:],
                    in0=iota_s[:, None, :].to_broadcast([128, K, 128]),
                    in1=dstrel_s[:, b, :, None].to_broadcast([128, K, 128]),
                    op=AL.is_equal)

                v2 = pe2.tile([128, K, c.OUT_C + 1], BF, tag="v2")
                nc.vector.tensor_tensor(
                    out=v2[:], in0=hg2[:, :, 0:c.OUT_C + 1],
                    in1=ee2[:, :, None].to_broadcast([128, K, c.OUT_C + 1]),
                    op=AL.mult)

                ps2 = pp.tile([128, c.OUT_C + 1], F32, tag="E")
                for j in range(K):
                    nc.tensor.matmul(out=ps2[:], lhsT=mask2[:, j, :],
                                     rhs=v2[:, j, :],
                                     start=(j == 0), stop=(j == K - 1))

                own2 = pe2.tile([128, 67], BF, tag="own2")
                nc.sync.dma_start(out=own2[:],
                                  in_=cc3in[b * 128:(b + 1) * 128, 0:67])
                z2s = sm.tile([128, 1], F32, tag="z2s")
                nc.vector.tensor_tensor(out=z2s[:], in0=own2[:, 65:66],
                                        in1=own2[:, 66:67], op=AL.add)
                lr2s = sm.tile([128, 1], F32, tag="lr2s")
                nc.vector.scalar_tensor_tensor(
                    out=lr2s[:], in0=z2s[:], scalar=c.NEG, in1=z2s[:],
                    op0=AL.mult, op1=AL.max)
                ee2s = sm.tile([128, 1], BF, tag="ee2s")
                nc.scalar.activation(out=ee2s[:], in_=lr2s[:], func=AF.Exp)
                v2s = sm.tile([128, c.OUT_C + 1], BF, tag="v2s")
                nc.vector.tensor_tensor(
                    out=v2s[:], in0=own2[:, 0:c.OUT_C + 1],
                    in1=ee2s[:, 0:1].to_broadcast([128, c.OUT_C + 1]),
                    op=AL.mult)
                nc.vector.tensor_tensor(out=ps2[:], in0=ps2[:], in1=v2s[:],
                                        op=AL.add)

                den2 = sm.tile([128, 1], F32, tag="den2")
                nc.vector.tensor_scalar(
                    out=den2[:], in0=ps2[:, c.OUT_C:c.OUT_C + 1], scalar1=1e-16,
                    scalar2=None, op0=AL.add)
                rec2 = sm.tile([128, 1], F32, tag="rec2")
                nc.vector.reciprocal(out=rec2[:], in_=den2[:])
                o2r = sm.tile([128, c.OUT_C], F32, tag="o2r")
                nc.scalar.activation(out=o2r[:], in_=ps2[:, 0:c.OUT_C],
                                     func=AF.Relu, scale=rec2[:])
                nc.vector.tensor_tensor(out=pacc[:], in0=pacc[:], in1=o2r[:],
                                        op=AL.add)

            # ================= phase F: pool partial =========================
            psf = pp.tile([c.OUT_C + 1, 1], F32, tag="E")
            nc.tensor.matmul(out=psf[0:c.OUT_C, :], lhsT=pacc[:], rhs=ones_s[:],
                             start=True, stop=True)
            pf = sm.tile([c.OUT_C, 1], F32, tag="pf")
            nc.vector.tensor_copy(out=pf[:], in_=psf[0:c.OUT_C, :])
            nc.sync.dma_start(out=pool_d[:, :], in_=pf[:])

    nc.compile()
    legalize_waits(nc)
    return nc


def legalize_waits(nc):
    """Walrus encodes at most ONE sync wait per instruction on this toolchain.
    Hoist excess waits onto same-engine NoOps inserted before the instruction."""
    for fn in nc.m.functions:
        for bb in fn.blocks:
            insts = list(bb.instructions)
            out = []
            changed = False
            for inst in insts:
                si = inst.sync_info
                if si is not None and si.on_wait and len(si.on_wait) > 1:
                    waits = list(si.on_wait)
                    for w in waits[:-1]:
                        nop = mybir.InstNoOp(
                            name=nc.get_next_instruction_name(), ins=[], outs=[])
                        nop.engine = inst.engine
                        nop.sync_info = mybir.SyncInfo(on_wait=[w], on_update=[])
                        nc.register_instruction(nop)
                        out.append(nop)
                    inst.sync_info = mybir.SyncInfo(
                        on_wait=waits[-1:], on_update=list(si.on_update))
                    changed = True
                out.append(inst)
            if changed:
                bb.instructions.clear()
                bb.instructions.extend(out)


def host_finish(cfg, pools, fc_w, fc_b):
    c = cfg
    tot = np.zeros(c.OUT_C, np.float64)
    for p in pools:
        tot += p[:, 0].astype(np.float64)
    pooled = (tot / c.N).astype(np.float32)
    logits = pooled @ np.asarray(fc_w, np.float32) + np.asarray(fc_b, np.float32)
    m = logits.max()
    ls = logits - (m + np.log(np.exp(logits - m).sum()))
    return ls.reshape(1, c.NCLS).astype(np.float32)


_BUILD_CACHE = {}


def run(cfg, inputs, debug=False, trace=False, **run_kwargs):
    in_maps, meta = host_prep(
        cfg, inputs["x"], inputs["edge_index"], inputs["W1"], inputs["att_src1"],
        inputs["att_dst1"], inputs["b1"], inputs["W2"], inputs["att_src2"],
        inputs["att_dst2"], inputs["b2"])
    stage = os.environ.get("KSTAGE", "F")
    key = (cfg.N, cfg.E, meta["kA"], meta["kB"], debug, stage)
    if key not in _BUILD_CACHE:
        _BUILD_CACHE[key] = build(cfg, meta["kA"], meta["kB"], debug=debug,
                                  stage=stage)
    nc = _BUILD_CACHE[key]
    res = bass_utils.run_bass_kernel_spmd(
        nc, in_maps, core_ids=list(range(cfg.NCORES)), trace=trace, **run_kwargs)
    out = host_finish(cfg, [r["pool64"] for r in res.results],
                      inputs["fc_w"], inputs["fc_b"])
    return out, res


def kernel(**inputs):
    cfg = Cfg()
    out, _ = run(cfg, inputs)
    return out


# revision 18
# speedup vs baseline: 1.2631x; 1.0118x over previous
"""GAT (2-layer graph attention network) Bass kernel for 8 Trainium2 NeuronCores.

Strategy (per spec sharding hint): edges are partitioned by destination-node
block so segment-softmax/sum stay core-local; each core owns N/8 destination
nodes. Layer-1 node features (h = x @ W1 plus attention alphas via an
augmented weight matrix) are computed replicated on every core into a DRAM
table with 512-byte rows: 256 fp8(e4m3) h channels + 4 bf16 alpha_src + 4
bf16 alpha_dst. Per-edge source rows arrive via one SWDGE gather per region
(A/B split for the int16 index range). The per-edge alpha_dst values are NOT
gathered: they are broadcast from the block's own alpha rows through a
transposed one-hot mask on the tensor engine (K tiny matmuls), which removes
one 256-B gather per edge versus the previous build. The segment-softmax +
weighted aggregation are fused into PE matmuls with a one-hot {edge x dst}
mask; PSUM accumulates numerator and denominator together. Layer-2
pre-features are exchanged with one AllGather; layer-2 aggregation repeats
the same structure with 256-B bf16 rows. Global mean-pool partials are
reduced on host along with the tiny fc + log_softmax head.
"""
import os
import sys
import types
import math

import numpy as np
import ml_dtypes


def _setup_paths():
    for p in ("/opt/trn_rl_repo", "/root/.axon_site/_ro/trn_rl_repo"):
        if os.path.isdir(p) and p not in sys.path:
            sys.path.insert(0, p)
    try:
        import concourse.bass  # noqa: F401
    except ImportError as e:
        raise RuntimeError(f"concourse not importable: {e}")


_setup_paths()

import concourse.bass as bass  # noqa: E402
import concourse.mybir as mybir  # noqa: E402
import concourse.tile as tile  # noqa: E402
from concourse import bacc, bass_utils  # noqa: E402

bf16 = ml_dtypes.bfloat16
BF = mybir.dt.bfloat16
F32 = mybir.dt.float32
I16 = mybir.dt.int16
FP8 = mybir.dt.float8e4
AL = mybir.AluOpType
AF = mybir.ActivationFunctionType


class Cfg:
    def __init__(self, N=50000, E=800000, IN_C=128, HID=64, OUT_C=64, HEADS=4,
                 NCLS=40, NEG=0.2, NCORES=8):
        self.N, self.E = N, E
        self.IN_C, self.HID, self.OUT_C, self.HEADS = IN_C, HID, OUT_C, HEADS
        self.NCLS, self.NEG, self.NCORES = NCLS, NEG, NCORES
        assert N % NCORES == 0
        self.NB = N // NCORES                      # owned real nodes per core
        self.NBLK = math.ceil(self.NB / 128)       # dst blocks per core
        self.NDP = self.NBLK * 128                 # padded owned rows per core
        self.RTOT = self.NDP * NCORES              # global padded row space
        assert self.RTOT % 128 == 0
        self.NT1 = self.RTOT // 128                # phase-A node tiles
        # A/B gather split (int16 row-index limit), multiple of 128
        self.SPLIT = min(32768, (self.RTOT // 2 + 127) // 128 * 128)
        assert self.SPLIT % 128 == 0 and self.SPLIT < 32768 + 1
        self.C1 = HEADS * HID                      # 256 layer-1 channels
        self.ROW1 = 512                            # table1 row bytes (fp8 units)
        self.AUX1 = 276                            # used bytes per table1 row
        self.HW1 = self.HID + 1                    # 65: head block + ones col
        self.ROW2 = 128                            # table3 cols (256B rows)
        assert self.OUT_C + 2 <= self.ROW2

    def row_of(self, v):
        return self.NDP * (v // self.NB) + (v % self.NB)


def _pack_idx(vals_2d):
    """vals_2d [G, n] -> dma_gather index layout [G, 128, n//16] int16.

    Index i lives at [i % 16, i // 16]; the 16-row group is replicated 8x
    across the 128 partitions.
    """
    G, n = vals_2d.shape
    assert n % 16 == 0
    a = vals_2d.reshape(G, n // 16, 16).transpose(0, 2, 1)   # [G, 16, n/16]
    return np.tile(a, (1, 8, 1)).astype(np.int16)            # [G, 128, n/16]


def host_prep(cfg, x, edge_index, W1, att_src1, att_dst1, b1, W2, att_src2,
              att_dst2, b2):
    """Build per-core in_maps (everything except the graph-independent consts)."""
    c = cfg
    # self-loops are handled densely per block on-device (not slotted)
    src = np.asarray(edge_index[0], dtype=np.int64)
    dst = np.asarray(edge_index[1], dtype=np.int64)
    EE = src.shape[0]

    core = dst // c.NB
    # load-balanced permutation of each core's dst nodes into blocks so that
    # per-block A/B-region edge counts are even (minimizes gather slot count K)
    t0core = src // c.NB                           # provisional (pre-perm) side
    rowmap = np.zeros(c.N, np.int64)               # node -> global padded row
    for ci in range(c.NCORES):
        sel = core == ci
        dloc = dst[sel] - ci * c.NB
        srcA = (c.row_of(src[sel]) < c.SPLIT)      # approx region (pre-perm)
        degA = np.bincount(dloc[srcA], minlength=c.NB).astype(np.int64)
        degT = np.bincount(dloc, minlength=c.NB).astype(np.int64)
        degB = degT - degA
        order_d = np.argsort(-degT, kind="stable")
        cnt = np.zeros(c.NBLK, np.int64)
        lA = np.zeros(c.NBLK, np.float64)
        lB = np.zeros(c.NBLK, np.float64)
        perm = np.zeros(c.NB, np.int64)
        for v in order_d:
            score = np.maximum(lA + degA[v], lB + degB[v]) + 1e9 * (cnt >= 128)
            j = int(np.argmin(score))
            perm[v] = j * 128 + cnt[j]
            cnt[j] += 1
            lA[j] += degA[v]
            lB[j] += degB[v]
        rowmap[ci * c.NB:(ci + 1) * c.NB] = ci * c.NDP + perm
    drow = rowmap[dst]
    blk = (drow - core * c.NDP) // 128
    din = (drow - core * c.NDP) % 128              # dst index within block
    srow = rowmap[src]
    isB = (srow >= c.SPLIT).astype(np.int64)

    gid = (core * c.NBLK + blk) * 2 + isB          # group id (A/B separate)
    order = np.argsort(gid, kind="stable")
    gid_s = gid[order]
    counts = np.bincount(gid_s, minlength=c.NCORES * c.NBLK * 2)
    nA = counts[0::2].reshape(c.NCORES, c.NBLK)
    nB = counts[1::2].reshape(c.NCORES, c.NBLK)
    kA = max(1, int(math.ceil(nA.max() / 128)))
    kB = max(1, int(math.ceil(nB.max() / 128)))
    K = kA + kB

    # rank within group
    starts = np.zeros_like(counts)
    starts[1:] = np.cumsum(counts)[:-1]
    rank = np.arange(EE) - starts[gid_s]

    # destination slot within the (core, blk) slot array of length K*128
    slot = np.where(isB[order] == 0, rank, kA * 128 + rank)
    cg = core[order] * c.NBLK + blk[order]          # [EE] group (core, blk)

    srow_slot = np.zeros((c.NCORES * c.NBLK, K * 128), np.int64)
    srow_slot[:, kA * 128:] = c.SPLIT               # B-region pad -> idx 0
    din_slot = np.full((c.NCORES * c.NBLK, K * 128), 128.0, np.float32)
    srow_slot[cg, slot] = srow[order]
    din_slot[cg, slot] = din[order]

    srow_slot = srow_slot.reshape(c.NCORES, c.NBLK, K * 128)
    din_slot = din_slot.reshape(c.NCORES, c.NBLK, K * 128)

    # augmented weights
    W1 = np.asarray(W1, np.float32)
    a_s1 = np.asarray(att_src1, np.float32).reshape(c.HEADS, c.HID)
    a_d1 = np.asarray(att_dst1, np.float32).reshape(c.HEADS, c.HID)
    W1r = W1.reshape(c.IN_C, c.HEADS, c.HID)
    Wa_s = np.einsum("khc,hc->kh", W1r, a_s1)       # [IN_C, HEADS]
    Wa_d = np.einsum("khc,hc->kh", W1r, a_d1)
    w1aug = np.zeros((c.IN_C, c.C1 + 8), np.float32)
    w1aug[:, :c.C1] = W1
    w1aug[:, c.C1:c.C1 + c.HEADS] = Wa_s
    w1aug[:, c.C1 + 4:c.C1 + 4 + c.HEADS] = Wa_d

    W2 = np.asarray(W2, np.float32)
    a_s2 = np.asarray(att_src2, np.float32).reshape(c.OUT_C)
    a_d2 = np.asarray(att_dst2, np.float32).reshape(c.OUT_C)
    w2aug = np.zeros((c.C1, 72), np.float32)
    w2aug[:, :c.OUT_C] = W2
    w2aug[:, c.OUT_C] = W2 @ a_s2
    w2aug[:, c.OUT_C + 1] = W2 @ a_d2

    assert np.allclose(np.asarray(b1), 0) and np.allclose(np.asarray(b2), 0), \
        "nonzero biases not folded in this build"

    # padded, row-mapped, transposed x tiles
    x = np.asarray(x, np.float32)
    x_pad = np.zeros((c.RTOT, c.IN_C), np.float32)
    x_pad[rowmap] = x
    xT = x_pad.reshape(c.NT1, 128, c.IN_C).transpose(0, 2, 1)  # [t, k, n]
    xT = np.ascontiguousarray(xT).astype(ml_dtypes.float8_e4m3fn)

    iota = np.broadcast_to(np.arange(128, dtype=np.float32),
                           (128, 128)).astype(bf16).copy()
    iota_p = np.arange(128, dtype=np.float32).reshape(128, 1).astype(bf16)
    ones = np.ones((128, 1), np.float32)

    in_maps = []
    meta = dict(kA=kA, kB=kB, K=K)
    for ci in range(c.NCORES):
        idxA = _pack_idx(srow_slot[ci, :, :kA * 128].copy())         # [NBLK,128,kA*8]
        idxB = _pack_idx(srow_slot[ci, :, kA * 128:] - c.SPLIT)
        dr = din_slot[ci].reshape(c.NBLK, K, 128).transpose(2, 0, 1)  # [128,NBLK,K]
        drT = din_slot[ci].reshape(c.NBLK, K * 128)                   # [NBLK,K*128]
        xo = xT[ci * c.NBLK:(ci + 1) * c.NBLK]                       # own tiles
        in_maps.append({
            "x_t3": xT,
            "x_own": np.ascontiguousarray(xo),
            "w1aug": w1aug.astype(bf16),
            "w2aug": np.ascontiguousarray(w2aug.astype(bf16).reshape(c.C1 // 128, 128, 72).transpose(1, 0, 2)),
            "idxA": np.ascontiguousarray(idxA.transpose(1, 0, 2)),   # [128,NBLK,kA*8]
            "idxB": np.ascontiguousarray(idxB.transpose(1, 0, 2)),
            "dstrel": np.ascontiguousarray(dr).astype(bf16),
            "dstrelT": np.ascontiguousarray(drT).astype(bf16),
            "iota": iota,
            "iota_p": iota_p,
            "ones": ones,
        })
    return in_maps, meta


def build(cfg, kA, kB, core_id_split=None, debug=False, stage="F"):
    """stage: truncate program after phase A/B/C/D/E/F (for HW bisection)."""
    c = cfg
    K = kA + kB
    KH = c.C1 // 128                      # k-halves for layer-2 contraction
    nc = bacc.Bacc("TRN2", target_bir_lowering=False, debug=False,
                   num_devices=c.NCORES)

    # ---- IO ----
    x_t3_d = nc.dram_tensor("x_t3", [c.NT1, 128, c.IN_C], FP8, kind="ExternalInput").ap()
    x_own_d = nc.dram_tensor("x_own", [c.NBLK, 128, c.IN_C], FP8, kind="ExternalInput").ap()
    w1_d = nc.dram_tensor("w1aug", [c.IN_C, c.C1 + 8], BF, kind="ExternalInput").ap()
    w2_d = nc.dram_tensor("w2aug", [128, KH, 72], BF, kind="ExternalInput").ap()
    idxA_d = nc.dram_tensor("idxA", [128, c.NBLK, kA * 8], I16, kind="ExternalInput").ap()
    idxB_d = nc.dram_tensor("idxB", [128, c.NBLK, kB * 8], I16, kind="ExternalInput").ap()
    dstrel_d = nc.dram_tensor("dstrel", [128, c.NBLK, K], BF, kind="ExternalInput").ap()
    dstrelT_d = nc.dram_tensor("dstrelT", [c.NBLK, K * 128], BF, kind="ExternalInput").ap()
    iota_d = nc.dram_tensor("iota", [128, 128], BF, kind="ExternalInput").ap()
    iotap_d = nc.dram_tensor("iota_p", [128, 1], BF, kind="ExternalInput").ap()
    ones_d = nc.dram_tensor("ones", [128, 1], F32, kind="ExternalInput").ap()
    pool_d = nc.dram_tensor("pool64", [c.OUT_C, 1], F32, kind="ExternalOutput").ap()
    if debug:
        h1dbg_d = nc.dram_tensor("h1dbg", [c.NDP, c.C1], F32, kind="ExternalOutput").ap()
        h2dbg_d = nc.dram_tensor("h2dbg", [c.NDP, 72], F32, kind="ExternalOutput").ap()

    # ---- internal DRAM ----
    nsplit_t = c.SPLIT // 128
    t1A = nc.dram_tensor("t1A", [c.SPLIT, c.ROW1], FP8, kind="Internal").ap()
    t1B = nc.dram_tensor("t1B", [c.RTOT - c.SPLIT, c.ROW1], FP8, kind="Internal").ap()
    h1d = nc.dram_tensor("h1d", [c.NDP, c.C1], BF, kind="Internal").ap()
    cc3in = nc.dram_tensor("cc3in", [c.NDP, c.ROW2], BF, kind="Internal").ap()
    cc3 = nc.dram_tensor("cc3", [c.RTOT, c.ROW2], BF, kind="Internal",
                         addr_space="Shared").ap()

    with tile.TileContext(nc) as tc:
        with tc.tile_pool(name="const", bufs=1) as cpool, \
             tc.tile_pool(name="pa", bufs=4) as pa, \
             tc.tile_pool(name="pp", bufs=2, space="PSUM") as pp, \
             tc.tile_pool(name="pg", bufs=3) as pg, \
             tc.tile_pool(name="pe2", bufs=3) as pe2, \
             tc.tile_pool(name="sm", bufs=3) as sm:

            # constants resident
            w1s = cpool.tile_from(w1_d)                     # [128, C1+8]
            w2s = cpool.tile_from(w2_d)                     # [128, KH, 72]
            iota_s = cpool.tile_from(iota_d)
            iotap_s = cpool.tile_from(iotap_d)
            ones_s = cpool.tile_from(ones_d)
            idxA_s = cpool.tile_from(idxA_d)
            idxB_s = cpool.tile_from(idxB_d)
            dstrel_s = cpool.tile_from(dstrel_d)

            # ================= phase A: h table (replicated) =================
            CH = 4
            for t0 in range(0, c.NT1, CH):
                xt = pa.tile([128, CH, c.IN_C], FP8, tag="xt")
                nc.sync.dma_start(
                    out=xt[:], in_=x_t3_d[t0:t0 + CH, :, :].rearrange("a k n -> k a n"))
                ob = pa.tile([128, CH, c.AUX1], FP8, tag="ob")
                for i in range(CH):
                    ps = pp.tile([128, c.C1 + 8], F32, tag="A")
                    nc.tensor.matmul(out=ps[:], lhsT=xt[:, i, :], rhs=w1s[:],
                                     start=True, stop=True)
                    obv = ob[:, i, 0:4 * c.HW1].rearrange("p (h q) -> p h q",
                                                          q=c.HW1)
                    psv = ps[:, 0:c.C1].rearrange("p (h q) -> p h q", q=c.HID)
                    if i % 2 == 0:
                        nc.scalar.activation(out=obv[:, :, 0:c.HID], in_=psv,
                                             func=AF.Copy)
                        nc.vector.tensor_copy(
                            out=ob[:, i, 4 * c.HW1:4 * c.HW1 + 16].bitcast(BF),
                            in_=ps[:, c.C1:c.C1 + 8])
                        nc.vector.tensor_scalar(
                            out=obv[:, :, c.HID:c.HW1], in0=psv[:, :, 0:1],
                            scalar1=0.0, scalar2=1.0, op0=AL.mult, op1=AL.add)
                    else:
                        nc.vector.tensor_copy(out=obv[:, :, 0:c.HID], in_=psv)
                        nc.scalar.activation(
                            out=ob[:, i, 4 * c.HW1:4 * c.HW1 + 16].bitcast(BF),
                            in_=ps[:, c.C1:c.C1 + 8], func=AF.Copy)
                        nc.vector.tensor_scalar(
                            out=obv[:, :, c.HID:c.HW1], in0=psv[:, :, 0:1],
                            scalar1=0.0, scalar2=1.0, op0=AL.mult, op1=AL.add)
                r0 = t0 * 128
                if t0 < nsplit_t:
                    dstv = t1A[r0:r0 + CH * 128, 0:c.AUX1]
                else:
                    dstv = t1B[r0 - c.SPLIT:r0 - c.SPLIT + CH * 128, 0:c.AUX1]
                nc.sync.dma_start(
                    out=dstv.rearrange("(a p) q -> p a q", p=128), in_=ob[:])

            # ================= phase B: layer-1 edge aggregation =============
            for b in (range(c.NBLK) if stage >= "B" else []):
                hg = pg.tile([128, K, c.ROW1], FP8, tag="hg", bufs=4)
                nc.gpsimd.dma_gather(
                    out_ap=hg[:, 0:kA, :], in_ap=t1A[:, :],
                    idxs_ap=idxA_s[:, b, :], num_idxs=kA * 128,
                    num_idxs_reg=kA * 128, elem_size=c.ROW1, single_packet=False)
                nc.gpsimd.dma_gather(
                    out_ap=hg[:, kA:K, :], in_ap=t1B[:, :],
                    idxs_ap=idxB_s[:, b, :], num_idxs=kB * 128,
                    num_idxs_reg=kB * 128, elem_size=c.ROW1, single_packet=False)

                # transposed one-hot mask (dst-major) for alpha_dst broadcast
                dT = pg.tile([128, K * 128], BF, tag="dT")
                nc.sync.dma_start(
                    out=dT[:], in_=dstrelT_d[b:b + 1, :].to_broadcast([128, K * 128]))
                maskT = pg.tile([128, K, 128], BF, tag="mT")
                nc.vector.tensor_tensor(
                    out=maskT[:],
                    in0=iotap_s[:, :, None].to_broadcast([128, K, 128]),
                    in1=dT[:].rearrange("p (k s) -> p k s", k=K),
                    op=AL.is_equal)
                # block's own alpha_dst rows (on-the-fly from own x tile)
                xo = pa.tile([128, c.IN_C], FP8, tag="xo")
                nc.sync.dma_start(out=xo[:], in_=x_own_d[b, :, :])
                pso = pp.tile([128, c.C1 + 8], F32, tag="L")
                nc.tensor.matmul(out=pso[:], lhsT=xo[:], rhs=w1s[:],
                                 start=True, stop=True)
                adb = sm.tile([128, 4], BF, tag="adb")
                nc.scalar.activation(out=adb[:], in_=pso[:, c.C1 + 4:c.C1 + 8],
                                     func=AF.Copy)
                als = sm.tile([128, 8], F32, tag="als")
                nc.scalar.activation(out=als[:], in_=pso[:, c.C1:c.C1 + 8],
                                     func=AF.Copy)
                psL = pp.tile([128, c.C1 + 8], F32, tag="L")
                for j in range(K):
                    nc.tensor.matmul(out=psL[:, j * 4:(j + 1) * 4],
                                     lhsT=maskT[:, j, :],
                                     rhs=adb[:],
                                     start=True, stop=True)

                z = sm.tile([128, K, c.HEADS], F32, tag="z")
                nc.vector.tensor_tensor(
                    out=z[:],
                    in0=hg[:, :, 4 * c.HW1:4 * c.HW1 + 8].bitcast(BF),
                    in1=psL[:, 0:K * 4].rearrange("p (k h) -> p k h", k=K),
                    op=AL.add)
                lr = sm.tile([128, K, c.HEADS], F32, tag="lr")
                nc.vector.scalar_tensor_tensor(
                    out=lr[:], in0=z[:], scalar=c.NEG, in1=z[:],
                    op0=AL.mult, op1=AL.max)
                eeb = sm.tile([128, K, c.HEADS], BF, tag="eeb")
                nc.scalar.activation(out=eeb[:], in_=lr[:], func=AF.Exp)

                mask = pg.tile([128, K, 128], BF, tag="mask")
                nc.vector.tensor_tensor(
                    out=mask[:],
                    in0=iota_s[:, None, :].to_broadcast([128, K, 128]),
                    in1=dstrel_s[:, b, :, None].to_broadcast([128, K, 128]),
                    op=AL.is_equal)

                v = pg.tile([128, K, 4 * c.HW1], BF, tag="v", bufs=2)
                nc.vector.tensor_tensor(
                    out=v[:].rearrange("p k (h q) -> p k h q", h=c.HEADS),
                    in0=hg[:, :, 0:4 * c.HW1].rearrange("p k (h q) -> p k h q",
                                                        h=c.HEADS),
                    in1=eeb[:, :, :, None].to_broadcast([128, K, c.HEADS, c.HW1]),
                    op=AL.mult)

                ps = pp.tile([128, 4 * c.HW1], F32, tag="B")
                for j in range(K):
                    nc.tensor.matmul(out=ps[:], lhsT=mask[:, j, :],
                                     rhs=v[:, j, :],
                                     start=(j == 0), stop=(j == K - 1))

                # dense self-loop contribution (own rows, partition = dst)
                zs = sm.tile([128, c.HEADS], F32, tag="zs")
                nc.vector.tensor_tensor(
                    out=zs[:], in0=als[:, 0:4], in1=als[:, 4:8], op=AL.add)
                lrs = sm.tile([128, c.HEADS], F32, tag="lrs")
                nc.vector.scalar_tensor_tensor(
                    out=lrs[:], in0=zs[:], scalar=c.NEG, in1=zs[:],
                    op0=AL.mult, op1=AL.max)
                ees = sm.tile([128, c.HEADS], BF, tag="ees")
                nc.scalar.activation(out=ees[:], in_=lrs[:], func=AF.Exp)
                vself = sm.tile([128, 4 * c.HW1], BF, tag="vself")
                vsv = vself[:].rearrange("p (h q) -> p h q", q=c.HW1)
                nc.vector.tensor_tensor(
                    out=vsv[:, :, 0:c.HID],
                    in0=pso[:, 0:c.C1].rearrange("p (h q) -> p h q", q=c.HID),
                    in1=ees[:, :, None].to_broadcast([128, c.HEADS, c.HID]),
                    op=AL.mult)
                nc.vector.tensor_copy(out=vsv[:, :, c.HID:c.HW1],
                                      in_=ees[:, :, None])
                nc.vector.tensor_tensor(out=ps[:], in0=ps[:], in1=vself[:],
                                        op=AL.add)

                den = sm.tile([128, c.HEADS], F32, tag="den")
                nc.vector.tensor_scalar(
                    out=den[:, :, None],
                    in0=ps[:].rearrange("p (h q) -> p h q",
                                        q=c.HW1)[:, :, c.HID:c.HW1],
                    scalar1=1e-16, scalar2=None, op0=AL.add)
                rec = sm.tile([128, c.HEADS], F32, tag="rec")
                nc.vector.reciprocal(out=rec[:], in_=den[:])
                h1b = sm.tile([128, c.C1], BF, tag="h1b")
                for hh in range(c.HEADS):
                    nc.scalar.activation(
                        out=h1b[:, hh * c.HID:(hh + 1) * c.HID],
                        in_=ps[:, hh * c.HW1:hh * c.HW1 + c.HID],
                        func=AF.Relu, scale=rec[:, hh:hh + 1])
                nc.sync.dma_start(out=h1d[b * 128:(b + 1) * 128, :], in_=h1b[:])
                if debug:
                    h1dbgf = sm.tile([128, c.C1], F32, tag="h1dbgf")
                    nc.vector.tensor_copy(out=h1dbgf[:], in_=h1b[:])
                    nc.sync.dma_start(out=h1dbg_d[b * 128:(b + 1) * 128, :],
                                      in_=h1dbgf[:])

                # ---- fused phase C: h2_pre = relu(h1) @ W2aug ----
                if stage >= "C":
                    psc = pp.tile([128, c.C1 + 8], F32, tag="A")
                    for kh in range(KH):
                        ht = pa.tile([128, 128], BF, tag="ht")
                        nc.sync.dma_start(
                            out=ht[:], in_=h1d[b * 128:(b + 1) * 128,
                                               kh * 128:(kh + 1) * 128],
                            transpose=True)
                        nc.tensor.matmul(out=psc[:, 0:72], lhsT=ht[:],
                                         rhs=w2s[:, kh, :],
                                         start=(kh == 0), stop=(kh == KH - 1))
                    hc = pa.tile([128, c.ROW2], BF, tag="hc")
                    nc.vector.memset(hc[:, 67:c.ROW2], 0.0)
                    nc.vector.memset(hc[:, c.OUT_C:c.OUT_C + 1], 1.0)
                    nc.vector.tensor_copy(out=hc[:, 0:c.OUT_C],
                                          in_=psc[:, 0:c.OUT_C])
                    nc.vector.tensor_copy(out=hc[:, 65:67],
                                          in_=psc[:, c.OUT_C:c.OUT_C + 2])
                    nc.sync.dma_start(out=cc3in[b * 128:(b + 1) * 128, :],
                                      in_=hc[:])
                    if debug:
                        h2f = pa.tile([128, 72], F32, tag="h2f")
                        nc.vector.tensor_copy(out=h2f[:], in_=psc[:, 0:72])
                        nc.sync.dma_start(out=h2dbg_d[b * 128:(b + 1) * 128, :],
                                          in_=h2f[:])

            # ================= phase D: allgather + repack ===================
            if stage >= "D":
                nc.gpsimd.collective_compute(
                    kind="AllGather", op=AL.bypass,
                    replica_groups=[list(range(c.NCORES))],
                    ins=[cc3in[:, :]], outs=[cc3[:, :]])

            # block-own alpha_dst2 column, resident for phase E
            a2dS = cpool.tile([128, c.NBLK], BF)
            if stage >= "D":
                nc.sync.dma_start(
                    out=a2dS[:],
                    in_=cc3in[0:c.NDP, 66:67].rearrange(
                        "(b p) q -> p (b q)", p=128))
            else:
                nc.vector.memset(a2dS[:], 0.0)

            # ================= phase E: layer-2 edge aggregation =============
            pacc = cpool.tile([128, c.OUT_C], F32)
            nc.vector.memset(pacc[:], 0.0)
            for b in (range(c.NBLK) if stage >= "E" else []):
                hg2 = pe2.tile([128, K, c.ROW2], BF, tag="hg2", bufs=4)
                nc.gpsimd.dma_gather(
                    out_ap=hg2[:, 0:kA, :], in_ap=cc3[:, :],
                    idxs_ap=idxA_s[:, b, :], num_idxs=kA * 128,
                    num_idxs_reg=kA * 128, elem_size=c.ROW2, single_packet=False)
                nc.gpsimd.dma_gather(
                    out_ap=hg2[:, kA:K, :], in_ap=cc3[c.SPLIT:c.RTOT, :],
                    idxs_ap=idxB_s[:, b, :], num_idxs=kB * 128,
                    num_idxs_reg=kB * 128, elem_size=c.ROW2, single_packet=False)

                dT2 = pe2.tile([128, K * 128], BF, tag="dT2")
                nc.sync.dma_start(
                    out=dT2[:], in_=dstrelT_d[b:b + 1, :].to_broadcast([128, K * 128]))
                maskT2 = pe2.tile([128, K, 128], BF, tag="mT2")
                nc.vector.tensor_tensor(
                    out=maskT2[:],
                    in0=iotap_s[:, :, None].to_broadcast([128, K, 128]),
                    in1=dT2[:].rearrange("p (k s) -> p k s", k=K),
                    op=AL.is_equal)
                psL2 = pp.tile([128, K * 4], F32, tag="L")
                for j in range(K):
                    nc.tensor.matmul(out=psL2[:, j:j + 1],
                                     lhsT=maskT2[:, j, :],
                                     rhs=a2dS[:, b:b + 1],
                                     start=True, stop=True)

                z2 = sm.tile([128, K], F32, tag="z2")
                nc.vector.tensor_tensor(
                    out=z2[:],
                    in0=hg2[:, :, 65:66].rearrange("p k q -> p (k q)"),
                    in1=psL2[:, 0:K], op=AL.add)
                lr2 = sm.tile([128, K], F32, tag="lr2")
                nc.vector.scalar_tensor_tensor(
                    out=lr2[:], in0=z2[:], scalar=c.NEG, in1=z2[:],
                    op0=AL.mult, op1=AL.max)
                ee2 = sm.tile([128, K], BF, tag="ee2")
                nc.scalar.activation(out=ee2[:], in_=lr2[:], func=AF.Exp)

                mask2 = pe2.tile([128, K, 128], BF, tag="mask2")
                nc.vector.tensor_tensor(
                    out=mask2[:],
                    in0=iota_s[:, None, :].to_broadcast([128, K, 128]),
                    in1=dstrel_s[:, b, :, None].to_broadcast([128, K, 128]),
                    op=AL.is_equal)

                v2 = pe2.tile([128, K, c.OUT_C + 1], BF, tag="v2", bufs=2)
                nc.vector.tensor_tensor(
                    out=v2[:], in0=hg2[:, :, 0:c.OUT_C + 1],
                    in1=ee2[:, :, None].to_broadcast([128, K, c.OUT_C + 1]),
                    op=AL.mult)

                ps2 = pp.tile([128, c.OUT_C + 1], F32, tag="E")
                for j in range(K):
                    nc.tensor.matmul(out=ps2[:], lhsT=mask2[:, j, :],
                                     rhs=v2[:, j, :],
                                     start=(j == 0), stop=(j == K - 1))

                own2 = pe2.tile([128, 67], BF, tag="own2")
                nc.sync.dma_start(out=own2[:],
                                  in_=cc3in[b * 128:(b + 1) * 128, 0:67])
                z2s = sm.tile([128, 1], F32, tag="z2s")
                nc.vector.tensor_tensor(out=z2s[:], in0=own2[:, 65:66],
                                        in1=own2[:, 66:67], op=AL.add)
                lr2s = sm.tile([128, 1], F32, tag="lr2s")
                nc.vector.scalar_tensor_tensor(
                    out=lr2s[:], in0=z2s[:], scalar=c.NEG, in1=z2s[:],
                    op0=AL.mult, op1=AL.max)
                ee2s = sm.tile([128, 1], BF, tag="ee2s")
                nc.scalar.activation(out=ee2s[:], in_=lr2s[:], func=AF.Exp)
                v2s = sm.tile([128, c.OUT_C + 1], BF, tag="v2s")
                nc.vector.tensor_tensor(
                    out=v2s[:], in0=own2[:, 0:c.OUT_C + 1],
                    in1=ee2s[:, 0:1].to_broadcast([128, c.OUT_C + 1]),
                    op=AL.mult)
                nc.vector.tensor_tensor(out=ps2[:], in0=ps2[:], in1=v2s[:],
                                        op=AL.add)

                den2 = sm.tile([128, 1], F32, tag="den2")
                nc.vector.tensor_scalar(
                    out=den2[:], in0=ps2[:, c.OUT_C:c.OUT_C + 1], scalar1=1e-16,
                    scalar2=None, op0=AL.add)
                rec2 = sm.tile([128, 1], F32, tag="rec2")
                nc.vector.reciprocal(out=rec2[:], in_=den2[:])
                o2r = sm.tile([128, c.OUT_C], F32, tag="o2r")
                nc.scalar.activation(out=o2r[:], in_=ps2[:, 0:c.OUT_C],
                                     func=AF.Relu, scale=rec2[:])
                nc.vector.tensor_tensor(out=pacc[:], in0=pacc[:], in1=o2r[:],
                                        op=AL.add)

            # ================= phase F: pool partial =========================
            psf = pp.tile([c.OUT_C + 1, 1], F32, tag="E")
            nc.tensor.matmul(out=psf[0:c.OUT_C, :], lhsT=pacc[:], rhs=ones_s[:],
                             start=True, stop=True)
            pf = sm.tile([c.OUT_C, 1], F32, tag="pf")
            nc.vector.tensor_copy(out=pf[:], in_=psf[0:c.OUT_C, :])
            nc.sync.dma_start(out=pool_d[:, :], in_=pf[:])

    nc.compile()
    legalize_waits(nc)
    return nc


def legalize_waits(nc):
    """Walrus encodes at most ONE sync wait per instruction on this toolchain.
    Hoist excess waits onto same-engine NoOps inserted before the instruction."""
    for fn in nc.m.functions:
        for bb in fn.blocks:
            insts = list(bb.instructions)
            out = []
            changed = False
            for inst in insts:
                si = inst.sync_info
                if si is not None and si.on_wait and len(si.on_wait) > 1:
                    waits = list(si.on_wait)
                    for w in waits[:-1]:
                        nop = mybir.InstNoOp(
                            name=nc.get_next_instruction_name(), ins=[], outs=[])
                        nop.engine = inst.engine
                        nop.sync_info = mybir.SyncInfo(on_wait=[w], on_update=[])
                        nc.register_instruction(nop)
                        out.append(nop)
                    inst.sync_info = mybir.SyncInfo(
                        on_wait=waits[-1:], on_update=list(si.on_update))
                    changed = True
                out.append(inst)
            if changed:
                bb.instructions.clear()
                bb.instructions.extend(out)


def host_finish(cfg, pools, fc_w, fc_b):
    c = cfg
    tot = np.zeros(c.OUT_C, np.float64)
    for p in pools:
        tot += p[:, 0].astype(np.float64)
    pooled = (tot / c.N).astype(np.float32)
    logits = pooled @ np.asarray(fc_w, np.float32) + np.asarray(fc_b, np.float32)
    m = logits.max()
    ls = logits - (m + np.log(np.exp(logits - m).sum()))
    return ls.reshape(1, c.NCLS).astype(np.float32)


_BUILD_CACHE = {}


def run(cfg, inputs, debug=False, trace=False, **run_kwargs):
    in_maps, meta = host_prep(
        cfg, inputs["x"], inputs["edge_index"], inputs["W1"], inputs["att_src1"],
        inputs["att_dst1"], inputs["b1"], inputs["W2"], inputs["att_src2"],
        inputs["att_dst2"], inputs["b2"])
    stage = os.environ.get("KSTAGE", "F")
    key = (cfg.N, cfg.E, meta["kA"], meta["kB"], debug, stage)
    if key not in _BUILD_CACHE:
        _BUILD_CACHE[key] = build(cfg, meta["kA"], meta["kB"], debug=debug,
                                  stage=stage)
    nc = _BUILD_CACHE[key]
    res = bass_utils.run_bass_kernel_spmd(
        nc, in_maps, core_ids=list(range(cfg.NCORES)), trace=trace, **run_kwargs)
    out = host_finish(cfg, [r["pool64"] for r in res.results],
                      inputs["fc_w"], inputs["fc_b"])
    return out, res


def kernel(**inputs):
    cfg = Cfg()
    out, _ = run(cfg, inputs)
    return out


# revision 19
# speedup vs baseline: 1.2748x; 1.0093x over previous
"""GAT (2-layer graph attention network) Bass kernel for 8 Trainium2 NeuronCores.

Strategy (per spec sharding hint): edges are partitioned by destination-node
block so segment-softmax/sum stay core-local; each core owns N/8 destination
nodes. Layer-1 node features (h = x @ W1 plus attention alphas via an
augmented weight matrix) are computed replicated on every core into a DRAM
table with 512-byte rows: 256 fp8(e4m3) h channels + 4 bf16 alpha_src + 4
bf16 alpha_dst. Per-edge source rows arrive via one SWDGE gather per region
(A/B split for the int16 index range). The per-edge alpha_dst values are NOT
gathered: they are broadcast from the block's own alpha rows through a
transposed one-hot mask on the tensor engine (K tiny matmuls), which removes
one 256-B gather per edge versus the previous build. The segment-softmax +
weighted aggregation are fused into PE matmuls with a one-hot {edge x dst}
mask; PSUM accumulates numerator and denominator together. Layer-2
pre-features are exchanged with one AllGather; layer-2 aggregation repeats
the same structure with 256-B bf16 rows. Global mean-pool partials are
reduced on host along with the tiny fc + log_softmax head.
"""
import os
import sys
import types
import math

import numpy as np
import ml_dtypes


def _setup_paths():
    for p in ("/opt/trn_rl_repo", "/root/.axon_site/_ro/trn_rl_repo"):
        if os.path.isdir(p) and p not in sys.path:
            sys.path.insert(0, p)
    try:
        import concourse.bass  # noqa: F401
    except ImportError as e:
        raise RuntimeError(f"concourse not importable: {e}")


_setup_paths()

import concourse.bass as bass  # noqa: E402
import concourse.mybir as mybir  # noqa: E402
import concourse.tile as tile  # noqa: E402
from concourse import bacc, bass_utils  # noqa: E402

bf16 = ml_dtypes.bfloat16
BF = mybir.dt.bfloat16
F32 = mybir.dt.float32
I16 = mybir.dt.int16
FP8 = mybir.dt.float8e4
AL = mybir.AluOpType
AF = mybir.ActivationFunctionType


class Cfg:
    def __init__(self, N=50000, E=800000, IN_C=128, HID=64, OUT_C=64, HEADS=4,
                 NCLS=40, NEG=0.2, NCORES=8):
        self.N, self.E = N, E
        self.IN_C, self.HID, self.OUT_C, self.HEADS = IN_C, HID, OUT_C, HEADS
        self.NCLS, self.NEG, self.NCORES = NCLS, NEG, NCORES
        assert N % NCORES == 0
        self.NB = N // NCORES                      # owned real nodes per core
        self.NBLK = math.ceil(self.NB / 128)       # dst blocks per core
        self.NDP = self.NBLK * 128                 # padded owned rows per core
        self.RTOT = self.NDP * NCORES              # global padded row space
        assert self.RTOT % 128 == 0
        self.NT1 = self.RTOT // 128                # phase-A node tiles
        # A/B gather split (int16 row-index limit), multiple of 128
        self.SPLIT = min(32768, (self.RTOT // 2 + 127) // 128 * 128)
        assert self.SPLIT % 128 == 0 and self.SPLIT < 32768 + 1
        self.C1 = HEADS * HID                      # 256 layer-1 channels
        self.ROW1 = 512                            # table1 row bytes (fp8 units)
        self.AUX1 = 276                            # used bytes per table1 row
        self.HW1 = self.HID + 1                    # 65: head block + ones col
        self.ROW2 = 128                            # table3 cols (256B rows)
        assert self.OUT_C + 2 <= self.ROW2

    def row_of(self, v):
        return self.NDP * (v // self.NB) + (v % self.NB)


def _pack_idx(vals_2d):
    """vals_2d [G, n] -> dma_gather index layout [G, 128, n//16] int16.

    Index i lives at [i % 16, i // 16]; the 16-row group is replicated 8x
    across the 128 partitions.
    """
    G, n = vals_2d.shape
    assert n % 16 == 0
    a = vals_2d.reshape(G, n // 16, 16).transpose(0, 2, 1)   # [G, 16, n/16]
    return np.tile(a, (1, 8, 1)).astype(np.int16)            # [G, 128, n/16]


def host_prep(cfg, x, edge_index, W1, att_src1, att_dst1, b1, W2, att_src2,
              att_dst2, b2):
    """Build per-core in_maps (everything except the graph-independent consts)."""
    c = cfg
    # self-loops are handled densely per block on-device (not slotted)
    src = np.asarray(edge_index[0], dtype=np.int64)
    dst = np.asarray(edge_index[1], dtype=np.int64)
    EE = src.shape[0]

    core = dst // c.NB
    # load-balanced permutation of each core's dst nodes into blocks so that
    # per-block A/B-region edge counts are even (minimizes gather slot count K)
    t0core = src // c.NB                           # provisional (pre-perm) side
    rowmap = np.zeros(c.N, np.int64)               # node -> global padded row
    for ci in range(c.NCORES):
        sel = core == ci
        dloc = dst[sel] - ci * c.NB
        srcA = (c.row_of(src[sel]) < c.SPLIT)      # approx region (pre-perm)
        degA = np.bincount(dloc[srcA], minlength=c.NB).astype(np.int64)
        degT = np.bincount(dloc, minlength=c.NB).astype(np.int64)
        degB = degT - degA
        order_d = np.argsort(-degT, kind="stable")
        cnt = np.zeros(c.NBLK, np.int64)
        lA = np.zeros(c.NBLK, np.float64)
        lB = np.zeros(c.NBLK, np.float64)
        perm = np.zeros(c.NB, np.int64)
        for v in order_d:
            score = np.maximum(lA + degA[v], lB + degB[v]) + 1e9 * (cnt >= 128)
            j = int(np.argmin(score))
            perm[v] = j * 128 + cnt[j]
            cnt[j] += 1
            lA[j] += degA[v]
            lB[j] += degB[v]
        rowmap[ci * c.NB:(ci + 1) * c.NB] = ci * c.NDP + perm
    drow = rowmap[dst]
    blk = (drow - core * c.NDP) // 128
    din = (drow - core * c.NDP) % 128              # dst index within block
    srow = rowmap[src]
    isB = (srow >= c.SPLIT).astype(np.int64)

    gid = (core * c.NBLK + blk) * 2 + isB          # group id (A/B separate)
    order = np.argsort(gid, kind="stable")
    gid_s = gid[order]
    counts = np.bincount(gid_s, minlength=c.NCORES * c.NBLK * 2)
    nA = counts[0::2].reshape(c.NCORES, c.NBLK)
    nB = counts[1::2].reshape(c.NCORES, c.NBLK)
    kA = max(1, int(math.ceil(nA.max() / 128)))
    kB = max(1, int(math.ceil(nB.max() / 128)))
    K = kA + kB

    # rank within group
    starts = np.zeros_like(counts)
    starts[1:] = np.cumsum(counts)[:-1]
    rank = np.arange(EE) - starts[gid_s]

    # destination slot within the (core, blk) slot array of length K*128
    slot = np.where(isB[order] == 0, rank, kA * 128 + rank)
    cg = core[order] * c.NBLK + blk[order]          # [EE] group (core, blk)

    srow_slot = np.zeros((c.NCORES * c.NBLK, K * 128), np.int64)
    srow_slot[:, kA * 128:] = c.SPLIT               # B-region pad -> idx 0
    din_slot = np.full((c.NCORES * c.NBLK, K * 128), 128.0, np.float32)
    srow_slot[cg, slot] = srow[order]
    din_slot[cg, slot] = din[order]

    srow_slot = srow_slot.reshape(c.NCORES, c.NBLK, K * 128)
    din_slot = din_slot.reshape(c.NCORES, c.NBLK, K * 128)

    # augmented weights
    W1 = np.asarray(W1, np.float32)
    a_s1 = np.asarray(att_src1, np.float32).reshape(c.HEADS, c.HID)
    a_d1 = np.asarray(att_dst1, np.float32).reshape(c.HEADS, c.HID)
    W1r = W1.reshape(c.IN_C, c.HEADS, c.HID)
    Wa_s = np.einsum("khc,hc->kh", W1r, a_s1)       # [IN_C, HEADS]
    Wa_d = np.einsum("khc,hc->kh", W1r, a_d1)
    w1aug = np.zeros((c.IN_C, c.C1 + 8), np.float32)
    w1aug[:, :c.C1] = W1
    w1aug[:, c.C1:c.C1 + c.HEADS] = Wa_s
    w1aug[:, c.C1 + 4:c.C1 + 4 + c.HEADS] = Wa_d

    W2 = np.asarray(W2, np.float32)
    a_s2 = np.asarray(att_src2, np.float32).reshape(c.OUT_C)
    a_d2 = np.asarray(att_dst2, np.float32).reshape(c.OUT_C)
    w2aug = np.zeros((c.C1, 72), np.float32)
    w2aug[:, :c.OUT_C] = W2
    w2aug[:, c.OUT_C] = W2 @ a_s2
    w2aug[:, c.OUT_C + 1] = W2 @ a_d2

    assert np.allclose(np.asarray(b1), 0) and np.allclose(np.asarray(b2), 0), \
        "nonzero biases not folded in this build"

    # padded, row-mapped, transposed x tiles
    x = np.asarray(x, np.float32)
    x_pad = np.zeros((c.RTOT, c.IN_C), np.float32)
    x_pad[rowmap] = x
    xT = x_pad.reshape(c.NT1, 128, c.IN_C).transpose(0, 2, 1)  # [t, k, n]
    xT = np.ascontiguousarray(xT).astype(ml_dtypes.float8_e4m3fn)

    iota = np.broadcast_to(np.arange(128, dtype=np.float32),
                           (128, 128)).astype(bf16).copy()
    iota_p = np.arange(128, dtype=np.float32).reshape(128, 1).astype(bf16)
    ones = np.ones((128, 1), np.float32)

    in_maps = []
    meta = dict(kA=kA, kB=kB, K=K)
    for ci in range(c.NCORES):
        idxA = _pack_idx(srow_slot[ci, :, :kA * 128].copy())         # [NBLK,128,kA*8]
        idxB = _pack_idx(srow_slot[ci, :, kA * 128:] - c.SPLIT)
        dr = din_slot[ci].reshape(c.NBLK, K, 128).transpose(2, 0, 1)  # [128,NBLK,K]
        drT = din_slot[ci].reshape(c.NBLK, K * 128)                   # [NBLK,K*128]
        xo = xT[ci * c.NBLK:(ci + 1) * c.NBLK]                       # own tiles
        in_maps.append({
            "x_t3": xT,
            "x_own": np.ascontiguousarray(xo),
            "w1aug": w1aug.astype(bf16),
            "w2aug": np.ascontiguousarray(w2aug.astype(bf16).reshape(c.C1 // 128, 128, 72).transpose(1, 0, 2)),
            "idxA": np.ascontiguousarray(idxA.transpose(1, 0, 2)),   # [128,NBLK,kA*8]
            "idxB": np.ascontiguousarray(idxB.transpose(1, 0, 2)),
            "dstrel": np.ascontiguousarray(dr).astype(bf16),
            "dstrelT": np.ascontiguousarray(drT).astype(bf16),
            "iota": iota,
            "iota_p": iota_p,
            "ones": ones,
        })
    return in_maps, meta


def build(cfg, kA, kB, core_id_split=None, debug=False, stage="F"):
    """stage: truncate program after phase A/B/C/D/E/F (for HW bisection)."""
    c = cfg
    K = kA + kB
    KH = c.C1 // 128                      # k-halves for layer-2 contraction
    nc = bacc.Bacc("TRN2", target_bir_lowering=False, debug=False,
                   num_devices=c.NCORES)

    # ---- IO ----
    x_t3_d = nc.dram_tensor("x_t3", [c.NT1, 128, c.IN_C], FP8, kind="ExternalInput").ap()
    x_own_d = nc.dram_tensor("x_own", [c.NBLK, 128, c.IN_C], FP8, kind="ExternalInput").ap()
    w1_d = nc.dram_tensor("w1aug", [c.IN_C, c.C1 + 8], BF, kind="ExternalInput").ap()
    w2_d = nc.dram_tensor("w2aug", [128, KH, 72], BF, kind="ExternalInput").ap()
    idxA_d = nc.dram_tensor("idxA", [128, c.NBLK, kA * 8], I16, kind="ExternalInput").ap()
    idxB_d = nc.dram_tensor("idxB", [128, c.NBLK, kB * 8], I16, kind="ExternalInput").ap()
    dstrel_d = nc.dram_tensor("dstrel", [128, c.NBLK, K], BF, kind="ExternalInput").ap()
    dstrelT_d = nc.dram_tensor("dstrelT", [c.NBLK, K * 128], BF, kind="ExternalInput").ap()
    iota_d = nc.dram_tensor("iota", [128, 128], BF, kind="ExternalInput").ap()
    iotap_d = nc.dram_tensor("iota_p", [128, 1], BF, kind="ExternalInput").ap()
    ones_d = nc.dram_tensor("ones", [128, 1], F32, kind="ExternalInput").ap()
    pool_d = nc.dram_tensor("pool64", [c.OUT_C, 1], F32, kind="ExternalOutput").ap()
    if debug:
        h1dbg_d = nc.dram_tensor("h1dbg", [c.NDP, c.C1], F32, kind="ExternalOutput").ap()
        h2dbg_d = nc.dram_tensor("h2dbg", [c.NDP, 72], F32, kind="ExternalOutput").ap()

    # ---- internal DRAM ----
    nsplit_t = c.SPLIT // 128
    t1A = nc.dram_tensor("t1A", [c.SPLIT, c.ROW1], FP8, kind="Internal").ap()
    t1B = nc.dram_tensor("t1B", [c.RTOT - c.SPLIT, c.ROW1], FP8, kind="Internal").ap()
    h1d = nc.dram_tensor("h1d", [c.NDP, c.C1], BF, kind="Internal").ap()
    cc3in = nc.dram_tensor("cc3in", [c.NDP, c.ROW2], BF, kind="Internal").ap()
    cc3 = nc.dram_tensor("cc3", [c.RTOT, c.ROW2], BF, kind="Internal",
                         addr_space="Shared").ap()

    with tile.TileContext(nc) as tc:
        with tc.tile_pool(name="const", bufs=1) as cpool, \
             tc.tile_pool(name="pa", bufs=4) as pa, \
             tc.tile_pool(name="pp", bufs=2, space="PSUM") as pp, \
             tc.tile_pool(name="pg", bufs=3) as pg, \
             tc.tile_pool(name="pe2", bufs=3) as pe2, \
             tc.tile_pool(name="sm", bufs=3) as sm:

            # constants resident
            w1s = cpool.tile_from(w1_d)                     # [128, C1+8]
            w2s = cpool.tile_from(w2_d)                     # [128, KH, 72]
            iota_s = cpool.tile_from(iota_d)
            iotap_s = cpool.tile_from(iotap_d)
            ones_s = cpool.tile_from(ones_d)
            idxA_s = cpool.tile_from(idxA_d)
            idxB_s = cpool.tile_from(idxB_d)
            dstrel_s = cpool.tile_from(dstrel_d)

            # ================= phase A: h table (replicated) =================
            CH = 4
            for t0 in range(0, c.NT1, CH):
                xt = pa.tile([128, CH, c.IN_C], FP8, tag="xt")
                nc.sync.dma_start(
                    out=xt[:], in_=x_t3_d[t0:t0 + CH, :, :].rearrange("a k n -> k a n"))
                ob = pa.tile([128, CH, c.AUX1], FP8, tag="ob")
                for i in range(CH):
                    ps = pp.tile([128, c.C1 + 8], F32,
                                 tag="A" if i % 2 == 0 else "L")
                    nc.tensor.matmul(out=ps[:], lhsT=xt[:, i, :], rhs=w1s[:],
                                     start=True, stop=True)
                    obv = ob[:, i, 0:4 * c.HW1].rearrange("p (h q) -> p h q",
                                                          q=c.HW1)
                    psv = ps[:, 0:c.C1].rearrange("p (h q) -> p h q", q=c.HID)
                    if i % 2 == 0:
                        nc.scalar.activation(out=obv[:, :, 0:c.HID], in_=psv,
                                             func=AF.Copy)
                        nc.vector.tensor_copy(
                            out=ob[:, i, 4 * c.HW1:4 * c.HW1 + 16].bitcast(BF),
                            in_=ps[:, c.C1:c.C1 + 8])
                        nc.vector.tensor_scalar(
                            out=obv[:, :, c.HID:c.HW1], in0=psv[:, :, 0:1],
                            scalar1=0.0, scalar2=1.0, op0=AL.mult, op1=AL.add)
                    else:
                        nc.vector.tensor_copy(out=obv[:, :, 0:c.HID], in_=psv)
                        nc.scalar.activation(
                            out=ob[:, i, 4 * c.HW1:4 * c.HW1 + 16].bitcast(BF),
                            in_=ps[:, c.C1:c.C1 + 8], func=AF.Copy)
                        nc.vector.tensor_scalar(
                            out=obv[:, :, c.HID:c.HW1], in0=psv[:, :, 0:1],
                            scalar1=0.0, scalar2=1.0, op0=AL.mult, op1=AL.add)
                r0 = t0 * 128
                if t0 < nsplit_t:
                    dstv = t1A[r0:r0 + CH * 128, 0:c.AUX1]
                else:
                    dstv = t1B[r0 - c.SPLIT:r0 - c.SPLIT + CH * 128, 0:c.AUX1]
                nc.sync.dma_start(
                    out=dstv.rearrange("(a p) q -> p a q", p=128), in_=ob[:])

            # ================= phase B: layer-1 edge aggregation =============
            for b in (range(c.NBLK) if stage >= "B" else []):
                hg = pg.tile([128, K, c.ROW1], FP8, tag="hg", bufs=4)
                nc.gpsimd.dma_gather(
                    out_ap=hg[:, 0:kA, :], in_ap=t1A[:, :],
                    idxs_ap=idxA_s[:, b, :], num_idxs=kA * 128,
                    num_idxs_reg=kA * 128, elem_size=c.ROW1, single_packet=False)
                nc.gpsimd.dma_gather(
                    out_ap=hg[:, kA:K, :], in_ap=t1B[:, :],
                    idxs_ap=idxB_s[:, b, :], num_idxs=kB * 128,
                    num_idxs_reg=kB * 128, elem_size=c.ROW1, single_packet=False)

                # transposed one-hot mask (dst-major) for alpha_dst broadcast
                dT = pg.tile([128, K * 128], BF, tag="dT")
                nc.sync.dma_start(
                    out=dT[:], in_=dstrelT_d[b:b + 1, :].to_broadcast([128, K * 128]))
                maskT = pg.tile([128, K, 128], BF, tag="mT")
                nc.vector.tensor_tensor(
                    out=maskT[:],
                    in0=iotap_s[:, :, None].to_broadcast([128, K, 128]),
                    in1=dT[:].rearrange("p (k s) -> p k s", k=K),
                    op=AL.is_equal)
                # block's own alpha_dst rows (on-the-fly from own x tile)
                xo = pa.tile([128, c.IN_C], FP8, tag="xo")
                nc.sync.dma_start(out=xo[:], in_=x_own_d[b, :, :])
                pso = pp.tile([128, c.C1 + 8], F32, tag="L")
                nc.tensor.matmul(out=pso[:], lhsT=xo[:], rhs=w1s[:],
                                 start=True, stop=True)
                adb = sm.tile([128, 4], BF, tag="adb")
                nc.scalar.activation(out=adb[:], in_=pso[:, c.C1 + 4:c.C1 + 8],
                                     func=AF.Copy)
                als = sm.tile([128, 8], F32, tag="als")
                nc.scalar.activation(out=als[:], in_=pso[:, c.C1:c.C1 + 8],
                                     func=AF.Copy)
                psL = pp.tile([128, c.C1 + 8], F32, tag="L")
                for j in range(K):
                    nc.tensor.matmul(out=psL[:, j * 4:(j + 1) * 4],
                                     lhsT=maskT[:, j, :],
                                     rhs=adb[:],
                                     start=True, stop=True)

                z = sm.tile([128, K, c.HEADS], F32, tag="z")
                nc.vector.tensor_tensor(
                    out=z[:],
                    in0=hg[:, :, 4 * c.HW1:4 * c.HW1 + 8].bitcast(BF),
                    in1=psL[:, 0:K * 4].rearrange("p (k h) -> p k h", k=K),
                    op=AL.add)
                lr = sm.tile([128, K, c.HEADS], F32, tag="lr")
                nc.vector.scalar_tensor_tensor(
                    out=lr[:], in0=z[:], scalar=c.NEG, in1=z[:],
                    op0=AL.mult, op1=AL.max)
                eeb = sm.tile([128, K, c.HEADS], BF, tag="eeb")
                nc.scalar.activation(out=eeb[:], in_=lr[:], func=AF.Exp)

                mask = pg.tile([128, K, 128], BF, tag="mask")
                nc.vector.tensor_tensor(
                    out=mask[:],
                    in0=iota_s[:, None, :].to_broadcast([128, K, 128]),
                    in1=dstrel_s[:, b, :, None].to_broadcast([128, K, 128]),
                    op=AL.is_equal)

                v = pg.tile([128, K, 4 * c.HW1], BF, tag="v", bufs=2)
                nc.vector.tensor_tensor(
                    out=v[:].rearrange("p k (h q) -> p k h q", h=c.HEADS),
                    in0=hg[:, :, 0:4 * c.HW1].rearrange("p k (h q) -> p k h q",
                                                        h=c.HEADS),
                    in1=eeb[:, :, :, None].to_broadcast([128, K, c.HEADS, c.HW1]),
                    op=AL.mult)

                ps = pp.tile([128, 4 * c.HW1], F32, tag="B")
                for j in range(K):
                    nc.tensor.matmul(out=ps[:], lhsT=mask[:, j, :],
                                     rhs=v[:, j, :],
                                     start=(j == 0), stop=(j == K - 1))

                # dense self-loop contribution (own rows, partition = dst)
                zs = sm.tile([128, c.HEADS], F32, tag="zs")
                nc.vector.tensor_tensor(
                    out=zs[:], in0=als[:, 0:4], in1=als[:, 4:8], op=AL.add)
                lrs = sm.tile([128, c.HEADS], F32, tag="lrs")
                nc.vector.scalar_tensor_tensor(
                    out=lrs[:], in0=zs[:], scalar=c.NEG, in1=zs[:],
                    op0=AL.mult, op1=AL.max)
                ees = sm.tile([128, c.HEADS], BF, tag="ees")
                nc.scalar.activation(out=ees[:], in_=lrs[:], func=AF.Exp)
                vself = sm.tile([128, 4 * c.HW1], BF, tag="vself")
                vsv = vself[:].rearrange("p (h q) -> p h q", q=c.HW1)
                nc.vector.tensor_tensor(
                    out=vsv[:, :, 0:c.HID],
                    in0=pso[:, 0:c.C1].rearrange("p (h q) -> p h q", q=c.HID),
                    in1=ees[:, :, None].to_broadcast([128, c.HEADS, c.HID]),
                    op=AL.mult)
                nc.vector.tensor_copy(out=vsv[:, :, c.HID:c.HW1],
                                      in_=ees[:, :, None])
                nc.vector.tensor_tensor(out=ps[:], in0=ps[:], in1=vself[:],
                                        op=AL.add)

                den = sm.tile([128, c.HEADS], F32, tag="den")
                nc.vector.tensor_scalar(
                    out=den[:, :, None],
                    in0=ps[:].rearrange("p (h q) -> p h q",
                                        q=c.HW1)[:, :, c.HID:c.HW1],
                    scalar1=1e-16, scalar2=None, op0=AL.add)
                rec = sm.tile([128, c.HEADS], F32, tag="rec")
                nc.vector.reciprocal(out=rec[:], in_=den[:])
                h1b = sm.tile([128, c.C1], BF, tag="h1b")
                for hh in range(c.HEADS):
                    nc.scalar.activation(
                        out=h1b[:, hh * c.HID:(hh + 1) * c.HID],
                        in_=ps[:, hh * c.HW1:hh * c.HW1 + c.HID],
                        func=AF.Relu, scale=rec[:, hh:hh + 1])
                nc.sync.dma_start(out=h1d[b * 128:(b + 1) * 128, :], in_=h1b[:])
                if debug:
                    h1dbgf = sm.tile([128, c.C1], F32, tag="h1dbgf")
                    nc.vector.tensor_copy(out=h1dbgf[:], in_=h1b[:])
                    nc.sync.dma_start(out=h1dbg_d[b * 128:(b + 1) * 128, :],
                                      in_=h1dbgf[:])

                # ---- fused phase C: h2_pre = relu(h1) @ W2aug ----
                if stage >= "C":
                    psc = pp.tile([128, c.C1 + 8], F32, tag="A")
                    for kh in range(KH):
                        ht = pa.tile([128, 128], BF, tag="ht")
                        nc.sync.dma_start(
                            out=ht[:], in_=h1d[b * 128:(b + 1) * 128,
                                               kh * 128:(kh + 1) * 128],
                            transpose=True)
                        nc.tensor.matmul(out=psc[:, 0:72], lhsT=ht[:],
                                         rhs=w2s[:, kh, :],
                                         start=(kh == 0), stop=(kh == KH - 1))
                    hc = pa.tile([128, c.ROW2], BF, tag="hc")
                    nc.vector.memset(hc[:, 67:c.ROW2], 0.0)
                    nc.vector.memset(hc[:, c.OUT_C:c.OUT_C + 1], 1.0)
                    nc.vector.tensor_copy(out=hc[:, 0:c.OUT_C],
                                          in_=psc[:, 0:c.OUT_C])
                    nc.vector.tensor_copy(out=hc[:, 65:67],
                                          in_=psc[:, c.OUT_C:c.OUT_C + 2])
                    nc.sync.dma_start(out=cc3in[b * 128:(b + 1) * 128, :],
                                      in_=hc[:])
                    if debug:
                        h2f = pa.tile([128, 72], F32, tag="h2f")
                        nc.vector.tensor_copy(out=h2f[:], in_=psc[:, 0:72])
                        nc.sync.dma_start(out=h2dbg_d[b * 128:(b + 1) * 128, :],
                                          in_=h2f[:])

            # ================= phase D: allgather + repack ===================
            if stage >= "D":
                nc.gpsimd.collective_compute(
                    kind="AllGather", op=AL.bypass,
                    replica_groups=[list(range(c.NCORES))],
                    ins=[cc3in[:, :]], outs=[cc3[:, :]])

            # block-own alpha_dst2 column, resident for phase E
            a2dS = cpool.tile([128, c.NBLK], BF)
            if stage >= "D":
                nc.sync.dma_start(
                    out=a2dS[:],
                    in_=cc3in[0:c.NDP, 66:67].rearrange(
                        "(b p) q -> p (b q)", p=128))
            else:
                nc.vector.memset(a2dS[:], 0.0)

            # ================= phase E: layer-2 edge aggregation =============
            pacc = cpool.tile([128, c.OUT_C], F32)
            nc.vector.memset(pacc[:], 0.0)
            for b in (range(c.NBLK) if stage >= "E" else []):
                hg2 = pe2.tile([128, K, c.ROW2], BF, tag="hg2", bufs=4)
                nc.gpsimd.dma_gather(
                    out_ap=hg2[:, 0:kA, :], in_ap=cc3[:, :],
                    idxs_ap=idxA_s[:, b, :], num_idxs=kA * 128,
                    num_idxs_reg=kA * 128, elem_size=c.ROW2, single_packet=False)
                nc.gpsimd.dma_gather(
                    out_ap=hg2[:, kA:K, :], in_ap=cc3[c.SPLIT:c.RTOT, :],
                    idxs_ap=idxB_s[:, b, :], num_idxs=kB * 128,
                    num_idxs_reg=kB * 128, elem_size=c.ROW2, single_packet=False)

                dT2 = pe2.tile([128, K * 128], BF, tag="dT2")
                nc.sync.dma_start(
                    out=dT2[:], in_=dstrelT_d[b:b + 1, :].to_broadcast([128, K * 128]))
                maskT2 = pe2.tile([128, K, 128], BF, tag="mT2")
                nc.vector.tensor_tensor(
                    out=maskT2[:],
                    in0=iotap_s[:, :, None].to_broadcast([128, K, 128]),
                    in1=dT2[:].rearrange("p (k s) -> p k s", k=K),
                    op=AL.is_equal)
                psL2 = pp.tile([128, K * 4], F32, tag="L")
                for j in range(K):
                    nc.tensor.matmul(out=psL2[:, j:j + 1],
                                     lhsT=maskT2[:, j, :],
                                     rhs=a2dS[:, b:b + 1],
                                     start=True, stop=True)

                z2 = sm.tile([128, K], F32, tag="z2")
                nc.vector.tensor_tensor(
                    out=z2[:],
                    in0=hg2[:, :, 65:66].rearrange("p k q -> p (k q)"),
                    in1=psL2[:, 0:K], op=AL.add)
                lr2 = sm.tile([128, K], F32, tag="lr2")
                nc.vector.scalar_tensor_tensor(
                    out=lr2[:], in0=z2[:], scalar=c.NEG, in1=z2[:],
                    op0=AL.mult, op1=AL.max)
                ee2 = sm.tile([128, K], BF, tag="ee2")
                nc.scalar.activation(out=ee2[:], in_=lr2[:], func=AF.Exp)

                mask2 = pe2.tile([128, K, 128], BF, tag="mask2")
                nc.vector.tensor_tensor(
                    out=mask2[:],
                    in0=iota_s[:, None, :].to_broadcast([128, K, 128]),
                    in1=dstrel_s[:, b, :, None].to_broadcast([128, K, 128]),
                    op=AL.is_equal)

                v2 = pe2.tile([128, K, c.OUT_C + 1], BF, tag="v2", bufs=2)
                nc.vector.tensor_tensor(
                    out=v2[:], in0=hg2[:, :, 0:c.OUT_C + 1],
                    in1=ee2[:, :, None].to_broadcast([128, K, c.OUT_C + 1]),
                    op=AL.mult)

                ps2 = pp.tile([128, c.OUT_C + 1], F32, tag="E")
                for j in range(K):
                    nc.tensor.matmul(out=ps2[:], lhsT=mask2[:, j, :],
                                     rhs=v2[:, j, :],
                                     start=(j == 0), stop=(j == K - 1))

                own2 = pe2.tile([128, 67], BF, tag="own2")
                nc.sync.dma_start(out=own2[:],
                                  in_=cc3in[b * 128:(b + 1) * 128, 0:67])
                z2s = sm.tile([128, 1], F32, tag="z2s")
                nc.vector.tensor_tensor(out=z2s[:], in0=own2[:, 65:66],
                                        in1=own2[:, 66:67], op=AL.add)
                lr2s = sm.tile([128, 1], F32, tag="lr2s")
                nc.vector.scalar_tensor_tensor(
                    out=lr2s[:], in0=z2s[:], scalar=c.NEG, in1=z2s[:],
                    op0=AL.mult, op1=AL.max)
                ee2s = sm.tile([128, 1], BF, tag="ee2s")
                nc.scalar.activation(out=ee2s[:], in_=lr2s[:], func=AF.Exp)
                v2s = sm.tile([128, c.OUT_C + 1], BF, tag="v2s")
                nc.vector.tensor_tensor(
                    out=v2s[:], in0=own2[:, 0:c.OUT_C + 1],
                    in1=ee2s[:, 0:1].to_broadcast([128, c.OUT_C + 1]),
                    op=AL.mult)
                nc.vector.tensor_tensor(out=ps2[:], in0=ps2[:], in1=v2s[:],
                                        op=AL.add)

                den2 = sm.tile([128, 1], F32, tag="den2")
                nc.vector.tensor_scalar(
                    out=den2[:], in0=ps2[:, c.OUT_C:c.OUT_C + 1], scalar1=1e-16,
                    scalar2=None, op0=AL.add)
                rec2 = sm.tile([128, 1], F32, tag="rec2")
                nc.vector.reciprocal(out=rec2[:], in_=den2[:])
                o2r = sm.tile([128, c.OUT_C], F32, tag="o2r")
                nc.scalar.activation(out=o2r[:], in_=ps2[:, 0:c.OUT_C],
                                     func=AF.Relu, scale=rec2[:])
                nc.vector.tensor_tensor(out=pacc[:], in0=pacc[:], in1=o2r[:],
                                        op=AL.add)

            # ================= phase F: pool partial =========================
            psf = pp.tile([c.OUT_C + 1, 1], F32, tag="E")
            nc.tensor.matmul(out=psf[0:c.OUT_C, :], lhsT=pacc[:], rhs=ones_s[:],
                             start=True, stop=True)
            pf = sm.tile([c.OUT_C, 1], F32, tag="pf")
            nc.vector.tensor_copy(out=pf[:], in_=psf[0:c.OUT_C, :])
            nc.sync.dma_start(out=pool_d[:, :], in_=pf[:])

    nc.compile()
    legalize_waits(nc)
    return nc


def legalize_waits(nc):
    """Walrus encodes at most ONE sync wait per instruction on this toolchain.
    Hoist excess waits onto same-engine NoOps inserted before the instruction."""
    for fn in nc.m.functions:
        for bb in fn.blocks:
            insts = list(bb.instructions)
            out = []
            changed = False
            for inst in insts:
                si = inst.sync_info
                if si is not None and si.on_wait and len(si.on_wait) > 1:
                    waits = list(si.on_wait)
                    for w in waits[:-1]:
                        nop = mybir.InstNoOp(
                            name=nc.get_next_instruction_name(), ins=[], outs=[])
                        nop.engine = inst.engine
                        nop.sync_info = mybir.SyncInfo(on_wait=[w], on_update=[])
                        nc.register_instruction(nop)
                        out.append(nop)
                    inst.sync_info = mybir.SyncInfo(
                        on_wait=waits[-1:], on_update=list(si.on_update))
                    changed = True
                out.append(inst)
            if changed:
                bb.instructions.clear()
                bb.instructions.extend(out)


def host_finish(cfg, pools, fc_w, fc_b):
    c = cfg
    tot = np.zeros(c.OUT_C, np.float64)
    for p in pools:
        tot += p[:, 0].astype(np.float64)
    pooled = (tot / c.N).astype(np.float32)
    logits = pooled @ np.asarray(fc_w, np.float32) + np.asarray(fc_b, np.float32)
    m = logits.max()
    ls = logits - (m + np.log(np.exp(logits - m).sum()))
    return ls.reshape(1, c.NCLS).astype(np.float32)


_BUILD_CACHE = {}


def run(cfg, inputs, debug=False, trace=False, **run_kwargs):
    in_maps, meta = host_prep(
        cfg, inputs["x"], inputs["edge_index"], inputs["W1"], inputs["att_src1"],
        inputs["att_dst1"], inputs["b1"], inputs["W2"], inputs["att_src2"],
        inputs["att_dst2"], inputs["b2"])
    stage = os.environ.get("KSTAGE", "F")
    key = (cfg.N, cfg.E, meta["kA"], meta["kB"], debug, stage)
    if key not in _BUILD_CACHE:
        _BUILD_CACHE[key] = build(cfg, meta["kA"], meta["kB"], debug=debug,
                                  stage=stage)
    nc = _BUILD_CACHE[key]
    res = bass_utils.run_bass_kernel_spmd(
        nc, in_maps, core_ids=list(range(cfg.NCORES)), trace=trace, **run_kwargs)
    out = host_finish(cfg, [r["pool64"] for r in res.results],
                      inputs["fc_w"], inputs["fc_b"])
    return out, res


def kernel(**inputs):
    cfg = Cfg()
    out, _ = run(cfg, inputs)
    return out


# revision 20
# speedup vs baseline: 1.3573x; 1.0648x over previous
"""GAT (2-layer graph attention network) Bass kernel for 8 Trainium2 NeuronCores.

Strategy (per spec sharding hint): edges are partitioned by destination-node
block so segment-softmax/sum stay core-local; each core owns N/8 destination
nodes. Layer-1 node features (h = x @ W1 plus attention alphas via an
augmented weight matrix) are computed replicated on every core into a DRAM
table with 512-byte rows: 256 fp8(e4m3) h channels + 4 bf16 alpha_src + 4
bf16 alpha_dst. Per-edge source rows arrive via one SWDGE gather per region
(A/B split for the int16 index range). The per-edge alpha_dst values are NOT
gathered: they are broadcast from the block's own alpha rows through a
transposed one-hot mask on the tensor engine (K tiny matmuls), which removes
one 256-B gather per edge versus the previous build. The segment-softmax +
weighted aggregation are fused into PE matmuls with a one-hot {edge x dst}
mask; PSUM accumulates numerator and denominator together. Layer-2
pre-features are exchanged with one AllGather; layer-2 aggregation repeats
the same structure with 256-B bf16 rows. Global mean-pool partials are
reduced on host along with the tiny fc + log_softmax head.
"""
import os
import sys
import types
import math

import numpy as np
import ml_dtypes


def _setup_paths():
    for p in ("/opt/trn_rl_repo", "/root/.axon_site/_ro/trn_rl_repo"):
        if os.path.isdir(p) and p not in sys.path:
            sys.path.insert(0, p)
    try:
        import concourse.bass  # noqa: F401
    except ImportError as e:
        raise RuntimeError(f"concourse not importable: {e}")


_setup_paths()

import concourse.bass as bass  # noqa: E402
import concourse.mybir as mybir  # noqa: E402
import concourse.tile as tile  # noqa: E402
from concourse import bacc, bass_utils  # noqa: E402

bf16 = ml_dtypes.bfloat16
BF = mybir.dt.bfloat16
F32 = mybir.dt.float32
I16 = mybir.dt.int16
FP8 = mybir.dt.float8e4
AL = mybir.AluOpType
AF = mybir.ActivationFunctionType


class Cfg:
    def __init__(self, N=50000, E=800000, IN_C=128, HID=64, OUT_C=64, HEADS=4,
                 NCLS=40, NEG=0.2, NCORES=8):
        self.N, self.E = N, E
        self.IN_C, self.HID, self.OUT_C, self.HEADS = IN_C, HID, OUT_C, HEADS
        self.NCLS, self.NEG, self.NCORES = NCLS, NEG, NCORES
        assert N % NCORES == 0
        self.NB = N // NCORES                      # owned real nodes per core
        self.NBLK = math.ceil(self.NB / 128)       # dst blocks per core
        self.NDP = self.NBLK * 128                 # padded owned rows per core
        self.RTOT = self.NDP * NCORES              # global padded row space
        assert self.RTOT % 128 == 0
        self.NT1 = self.RTOT // 128                # phase-A node tiles
        # A/B gather split (int16 row-index limit), multiple of 128
        self.SPLIT = min(32768, (self.RTOT // 2 + 127) // 128 * 128)
        assert self.SPLIT % 128 == 0 and self.SPLIT < 32768 + 1
        self.C1 = HEADS * HID                      # 256 layer-1 channels
        self.ROW1 = 512                            # table1 row bytes (fp8 units)
        self.AUX1 = 276                            # used bytes per table1 row
        self.HW1 = self.HID + 1                    # 65: head block + ones col
        self.ROW2 = 128                            # table3 cols (256B rows)
        assert self.OUT_C + 2 <= self.ROW2

    def row_of(self, v):
        return self.NDP * (v // self.NB) + (v % self.NB)


def _pack_idx(vals_2d):
    """vals_2d [G, n] -> dma_gather index layout [G, 128, n//16] int16.

    Index i lives at [i % 16, i // 16]; the 16-row group is replicated 8x
    across the 128 partitions.
    """
    G, n = vals_2d.shape
    assert n % 16 == 0
    a = vals_2d.reshape(G, n // 16, 16).transpose(0, 2, 1)   # [G, 16, n/16]
    return np.tile(a, (1, 8, 1)).astype(np.int16)            # [G, 128, n/16]


def host_prep(cfg, x, edge_index, W1, att_src1, att_dst1, b1, W2, att_src2,
              att_dst2, b2):
    """Build per-core in_maps (everything except the graph-independent consts)."""
    c = cfg
    # self-loops are handled densely per block on-device (not slotted)
    src = np.asarray(edge_index[0], dtype=np.int64)
    dst = np.asarray(edge_index[1], dtype=np.int64)
    EE = src.shape[0]

    core = dst // c.NB
    # load-balanced permutation of each core's dst nodes into blocks so that
    # per-block A/B-region edge counts are even (minimizes gather slot count K)
    t0core = src // c.NB                           # provisional (pre-perm) side
    rowmap = np.zeros(c.N, np.int64)               # node -> global padded row
    for ci in range(c.NCORES):
        sel = core == ci
        dloc = dst[sel] - ci * c.NB
        srcA = (c.row_of(src[sel]) < c.SPLIT)      # approx region (pre-perm)
        degA = np.bincount(dloc[srcA], minlength=c.NB).astype(np.int64)
        degT = np.bincount(dloc, minlength=c.NB).astype(np.int64)
        degB = degT - degA
        order_d = np.argsort(-degT, kind="stable")
        cnt = np.zeros(c.NBLK, np.int64)
        lA = np.zeros(c.NBLK, np.float64)
        lB = np.zeros(c.NBLK, np.float64)
        perm = np.zeros(c.NB, np.int64)
        for v in order_d:
            score = np.maximum(lA + degA[v], lB + degB[v]) + 1e9 * (cnt >= 128)
            j = int(np.argmin(score))
            perm[v] = j * 128 + cnt[j]
            cnt[j] += 1
            lA[j] += degA[v]
            lB[j] += degB[v]
        rowmap[ci * c.NB:(ci + 1) * c.NB] = ci * c.NDP + perm
    drow = rowmap[dst]
    blk = (drow - core * c.NDP) // 128
    din = (drow - core * c.NDP) % 128              # dst index within block
    srow = rowmap[src]
    isB = (srow >= c.SPLIT).astype(np.int64)

    gid = (core * c.NBLK + blk) * 2 + isB          # group id (A/B separate)
    order = np.argsort(gid, kind="stable")
    gid_s = gid[order]
    counts = np.bincount(gid_s, minlength=c.NCORES * c.NBLK * 2)
    nA = counts[0::2].reshape(c.NCORES, c.NBLK)
    nB = counts[1::2].reshape(c.NCORES, c.NBLK)
    kA = max(1, int(math.ceil(nA.max() / 128)))
    kB = max(1, int(math.ceil(nB.max() / 128)))
    K = kA + kB

    # rank within group
    starts = np.zeros_like(counts)
    starts[1:] = np.cumsum(counts)[:-1]
    rank = np.arange(EE) - starts[gid_s]

    # destination slot within the (core, blk) slot array of length K*128
    slot = np.where(isB[order] == 0, rank, kA * 128 + rank)
    cg = core[order] * c.NBLK + blk[order]          # [EE] group (core, blk)

    srow_slot = np.zeros((c.NCORES * c.NBLK, K * 128), np.int64)
    srow_slot[:, kA * 128:] = c.SPLIT               # B-region pad -> idx 0
    din_slot = np.full((c.NCORES * c.NBLK, K * 128), 128.0, np.float32)
    srow_slot[cg, slot] = srow[order]
    din_slot[cg, slot] = din[order]

    srow_slot = srow_slot.reshape(c.NCORES, c.NBLK, K * 128)
    din_slot = din_slot.reshape(c.NCORES, c.NBLK, K * 128)

    # augmented weights
    W1 = np.asarray(W1, np.float32)
    a_s1 = np.asarray(att_src1, np.float32).reshape(c.HEADS, c.HID)
    a_d1 = np.asarray(att_dst1, np.float32).reshape(c.HEADS, c.HID)
    W1r = W1.reshape(c.IN_C, c.HEADS, c.HID)
    Wa_s = np.einsum("khc,hc->kh", W1r, a_s1)       # [IN_C, HEADS]
    Wa_d = np.einsum("khc,hc->kh", W1r, a_d1)
    w1aug = np.zeros((c.IN_C, c.C1 + 8), np.float32)
    w1aug[:, :c.C1] = W1
    w1aug[:, c.C1:c.C1 + c.HEADS] = Wa_s
    w1aug[:, c.C1 + 4:c.C1 + 4 + c.HEADS] = Wa_d

    W2 = np.asarray(W2, np.float32)
    a_s2 = np.asarray(att_src2, np.float32).reshape(c.OUT_C)
    a_d2 = np.asarray(att_dst2, np.float32).reshape(c.OUT_C)
    w2aug = np.zeros((c.C1, 72), np.float32)
    w2aug[:, :c.OUT_C] = W2
    w2aug[:, c.OUT_C] = W2 @ a_s2
    w2aug[:, c.OUT_C + 1] = W2 @ a_d2

    assert np.allclose(np.asarray(b1), 0) and np.allclose(np.asarray(b2), 0), \
        "nonzero biases not folded in this build"

    # padded, row-mapped, transposed x tiles
    x = np.asarray(x, np.float32)
    x_pad = np.zeros((c.RTOT, c.IN_C), np.float32)
    x_pad[rowmap] = x
    xT = x_pad.reshape(c.NT1, 128, c.IN_C).transpose(0, 2, 1)  # [t, k, n]
    xT = np.ascontiguousarray(xT).astype(ml_dtypes.float8_e4m3fn)

    iota = np.broadcast_to(np.arange(128, dtype=np.float32),
                           (128, 128)).astype(bf16).copy()
    iota_p = np.arange(128, dtype=np.float32).reshape(128, 1).astype(bf16)
    ones = np.ones((128, 1), np.float32)

    in_maps = []
    meta = dict(kA=kA, kB=kB, K=K)
    for ci in range(c.NCORES):
        idxA = _pack_idx(srow_slot[ci, :, :kA * 128].copy())         # [NBLK,128,kA*8]
        idxB = _pack_idx(srow_slot[ci, :, kA * 128:] - c.SPLIT)
        dr = din_slot[ci].reshape(c.NBLK, K, 128).transpose(2, 0, 1)  # [128,NBLK,K]
        drT = din_slot[ci].reshape(c.NBLK, K * 128)                   # [NBLK,K*128]
        xo = xT[ci * c.NBLK:(ci + 1) * c.NBLK]                       # own tiles
        in_maps.append({
            "x_own": np.ascontiguousarray(xo),
            "w1aug": w1aug.astype(bf16),
            "w2aug": np.ascontiguousarray(w2aug.astype(bf16).reshape(c.C1 // 128, 128, 72).transpose(1, 0, 2)),
            "idxA": np.ascontiguousarray(idxA.transpose(1, 0, 2)),   # [128,NBLK,kA*8]
            "idxB": np.ascontiguousarray(idxB.transpose(1, 0, 2)),
            "dstrel": np.ascontiguousarray(dr).astype(bf16),
            "dstrelT": np.ascontiguousarray(drT).astype(bf16),
            "iota": iota,
            "iota_p": iota_p,
            "ones": ones,
        })
    return in_maps, meta


def build(cfg, kA, kB, core_id_split=None, debug=False, stage="F"):
    """stage: truncate program after phase A/B/C/D/E/F (for HW bisection)."""
    c = cfg
    K = kA + kB
    KH = c.C1 // 128                      # k-halves for layer-2 contraction
    nc = bacc.Bacc("TRN2", target_bir_lowering=False, debug=False,
                   num_devices=c.NCORES)

    # ---- IO ----
    x_own_d = nc.dram_tensor("x_own", [c.NBLK, 128, c.IN_C], FP8, kind="ExternalInput").ap()
    w1_d = nc.dram_tensor("w1aug", [c.IN_C, c.C1 + 8], BF, kind="ExternalInput").ap()
    w2_d = nc.dram_tensor("w2aug", [128, KH, 72], BF, kind="ExternalInput").ap()
    idxA_d = nc.dram_tensor("idxA", [128, c.NBLK, kA * 8], I16, kind="ExternalInput").ap()
    idxB_d = nc.dram_tensor("idxB", [128, c.NBLK, kB * 8], I16, kind="ExternalInput").ap()
    dstrel_d = nc.dram_tensor("dstrel", [128, c.NBLK, K], BF, kind="ExternalInput").ap()
    dstrelT_d = nc.dram_tensor("dstrelT", [c.NBLK, K * 128], BF, kind="ExternalInput").ap()
    iota_d = nc.dram_tensor("iota", [128, 128], BF, kind="ExternalInput").ap()
    iotap_d = nc.dram_tensor("iota_p", [128, 1], BF, kind="ExternalInput").ap()
    ones_d = nc.dram_tensor("ones", [128, 1], F32, kind="ExternalInput").ap()
    pool_d = nc.dram_tensor("pool64", [c.OUT_C, 1], F32, kind="ExternalOutput").ap()
    if debug:
        h1dbg_d = nc.dram_tensor("h1dbg", [c.NDP, c.C1], F32, kind="ExternalOutput").ap()
        h2dbg_d = nc.dram_tensor("h2dbg", [c.NDP, 72], F32, kind="ExternalOutput").ap()

    # ---- internal DRAM ----
    nsplit_t = c.SPLIT // 128
    t1loc = nc.dram_tensor("t1loc", [c.NDP, c.ROW1], FP8, kind="Internal").ap()
    t1 = nc.dram_tensor("t1", [c.RTOT, c.ROW1], FP8, kind="Internal",
                        addr_space="Shared").ap()
    h1d = nc.dram_tensor("h1d", [c.NDP, c.C1], BF, kind="Internal").ap()
    cc3in = nc.dram_tensor("cc3in", [c.NDP, c.ROW2], BF, kind="Internal").ap()
    cc3 = nc.dram_tensor("cc3", [c.RTOT, c.ROW2], BF, kind="Internal",
                         addr_space="Shared").ap()

    with tile.TileContext(nc) as tc:
        with tc.tile_pool(name="const", bufs=1) as cpool, \
             tc.tile_pool(name="pa", bufs=4) as pa, \
             tc.tile_pool(name="pp", bufs=2, space="PSUM") as pp, \
             tc.tile_pool(name="pg", bufs=3) as pg, \
             tc.tile_pool(name="pe2", bufs=3) as pe2, \
             tc.tile_pool(name="sm", bufs=3) as sm:

            # constants resident
            w1s = cpool.tile_from(w1_d)                     # [128, C1+8]
            w2s = cpool.tile_from(w2_d)                     # [128, KH, 72]
            iota_s = cpool.tile_from(iota_d)
            iotap_s = cpool.tile_from(iotap_d)
            ones_s = cpool.tile_from(ones_d)
            idxA_s = cpool.tile_from(idxA_d)
            idxB_s = cpool.tile_from(idxB_d)
            dstrel_s = cpool.tile_from(dstrel_d)

            # ========== phase A: own-strip h table + AllGather ==========
            CH = 7
            for t0 in range(0, c.NBLK, CH):
                xt = pa.tile([128, CH, c.IN_C], FP8, tag="xt")
                nc.sync.dma_start(
                    out=xt[:], in_=x_own_d[t0:t0 + CH, :, :].rearrange("a k n -> k a n"))
                ob = pa.tile([128, CH, c.AUX1], FP8, tag="ob")
                for i in range(CH):
                    ps = pp.tile([128, c.C1 + 8], F32,
                                 tag="A" if i % 2 == 0 else "L")
                    nc.tensor.matmul(out=ps[:], lhsT=xt[:, i, :], rhs=w1s[:],
                                     start=True, stop=True)
                    obv = ob[:, i, 0:4 * c.HW1].rearrange("p (h q) -> p h q",
                                                          q=c.HW1)
                    psv = ps[:, 0:c.C1].rearrange("p (h q) -> p h q", q=c.HID)
                    if i % 2 == 0:
                        nc.scalar.activation(out=obv[:, :, 0:c.HID], in_=psv,
                                             func=AF.Copy)
                        nc.vector.tensor_copy(
                            out=ob[:, i, 4 * c.HW1:4 * c.HW1 + 16].bitcast(BF),
                            in_=ps[:, c.C1:c.C1 + 8])
                        nc.vector.tensor_scalar(
                            out=obv[:, :, c.HID:c.HW1], in0=psv[:, :, 0:1],
                            scalar1=0.0, scalar2=1.0, op0=AL.mult, op1=AL.add)
                    else:
                        nc.vector.tensor_copy(out=obv[:, :, 0:c.HID], in_=psv)
                        nc.scalar.activation(
                            out=ob[:, i, 4 * c.HW1:4 * c.HW1 + 16].bitcast(BF),
                            in_=ps[:, c.C1:c.C1 + 8], func=AF.Copy)
                        nc.vector.tensor_scalar(
                            out=obv[:, :, c.HID:c.HW1], in0=psv[:, :, 0:1],
                            scalar1=0.0, scalar2=1.0, op0=AL.mult, op1=AL.add)
                r0 = t0 * 128
                nc.sync.dma_start(
                    out=t1loc[r0:r0 + CH * 128, 0:c.AUX1].rearrange(
                        "(a p) q -> p a q", p=128), in_=ob[:])
            nc.gpsimd.collective_compute(
                kind="AllGather", op=AL.bypass,
                replica_groups=[list(range(c.NCORES))],
                ins=[t1loc[:, :]], outs=[t1[:, :]])

            # ================= phase B: layer-1 edge aggregation =============
            for b in (range(c.NBLK) if stage >= "B" else []):
                hg = pg.tile([128, K, c.ROW1], FP8, tag="hg", bufs=4)
                nc.gpsimd.dma_gather(
                    out_ap=hg[:, 0:kA, :], in_ap=t1[0:c.SPLIT, :],
                    idxs_ap=idxA_s[:, b, :], num_idxs=kA * 128,
                    num_idxs_reg=kA * 128, elem_size=c.ROW1, single_packet=False)
                nc.gpsimd.dma_gather(
                    out_ap=hg[:, kA:K, :], in_ap=t1[c.SPLIT:c.RTOT, :],
                    idxs_ap=idxB_s[:, b, :], num_idxs=kB * 128,
                    num_idxs_reg=kB * 128, elem_size=c.ROW1, single_packet=False)

                # transposed one-hot mask (dst-major) for alpha_dst broadcast
                dT = pg.tile([128, K * 128], BF, tag="dT")
                nc.sync.dma_start(
                    out=dT[:], in_=dstrelT_d[b:b + 1, :].to_broadcast([128, K * 128]))
                maskT = pg.tile([128, K, 128], BF, tag="mT")
                nc.vector.tensor_tensor(
                    out=maskT[:],
                    in0=iotap_s[:, :, None].to_broadcast([128, K, 128]),
                    in1=dT[:].rearrange("p (k s) -> p k s", k=K),
                    op=AL.is_equal)
                # block's own alpha_dst rows (on-the-fly from own x tile)
                xo = pa.tile([128, c.IN_C], FP8, tag="xo")
                nc.sync.dma_start(out=xo[:], in_=x_own_d[b, :, :])
                pso = pp.tile([128, c.C1 + 8], F32, tag="L")
                nc.tensor.matmul(out=pso[:], lhsT=xo[:], rhs=w1s[:],
                                 start=True, stop=True)
                adb = sm.tile([128, 4], BF, tag="adb")
                nc.scalar.activation(out=adb[:], in_=pso[:, c.C1 + 4:c.C1 + 8],
                                     func=AF.Copy)
                als = sm.tile([128, 8], F32, tag="als")
                nc.scalar.activation(out=als[:], in_=pso[:, c.C1:c.C1 + 8],
                                     func=AF.Copy)
                psL = pp.tile([128, c.C1 + 8], F32, tag="L")
                for j in range(K):
                    nc.tensor.matmul(out=psL[:, j * 4:(j + 1) * 4],
                                     lhsT=maskT[:, j, :],
                                     rhs=adb[:],
                                     start=True, stop=True)

                z = sm.tile([128, K, c.HEADS], F32, tag="z")
                nc.vector.tensor_tensor(
                    out=z[:],
                    in0=hg[:, :, 4 * c.HW1:4 * c.HW1 + 8].bitcast(BF),
                    in1=psL[:, 0:K * 4].rearrange("p (k h) -> p k h", k=K),
                    op=AL.add)
                lr = sm.tile([128, K, c.HEADS], F32, tag="lr")
                nc.vector.scalar_tensor_tensor(
                    out=lr[:], in0=z[:], scalar=c.NEG, in1=z[:],
                    op0=AL.mult, op1=AL.max)
                eeb = sm.tile([128, K, c.HEADS], BF, tag="eeb")
                nc.scalar.activation(out=eeb[:], in_=lr[:], func=AF.Exp)

                mask = pg.tile([128, K, 128], BF, tag="mask")
                nc.vector.tensor_tensor(
                    out=mask[:],
                    in0=iota_s[:, None, :].to_broadcast([128, K, 128]),
                    in1=dstrel_s[:, b, :, None].to_broadcast([128, K, 128]),
                    op=AL.is_equal)

                v = pg.tile([128, K, 4 * c.HW1], BF, tag="v", bufs=2)
                nc.vector.tensor_tensor(
                    out=v[:].rearrange("p k (h q) -> p k h q", h=c.HEADS),
                    in0=hg[:, :, 0:4 * c.HW1].rearrange("p k (h q) -> p k h q",
                                                        h=c.HEADS),
                    in1=eeb[:, :, :, None].to_broadcast([128, K, c.HEADS, c.HW1]),
                    op=AL.mult)

                ps = pp.tile([128, 4 * c.HW1], F32, tag="B")
                for j in range(K):
                    nc.tensor.matmul(out=ps[:], lhsT=mask[:, j, :],
                                     rhs=v[:, j, :],
                                     start=(j == 0), stop=(j == K - 1))

                # dense self-loop contribution (own rows, partition = dst)
                zs = sm.tile([128, c.HEADS], F32, tag="zs")
                nc.vector.tensor_tensor(
                    out=zs[:], in0=als[:, 0:4], in1=als[:, 4:8], op=AL.add)
                lrs = sm.tile([128, c.HEADS], F32, tag="lrs")
                nc.vector.scalar_tensor_tensor(
                    out=lrs[:], in0=zs[:], scalar=c.NEG, in1=zs[:],
                    op0=AL.mult, op1=AL.max)
                ees = sm.tile([128, c.HEADS], BF, tag="ees")
                nc.scalar.activation(out=ees[:], in_=lrs[:], func=AF.Exp)
                vself = sm.tile([128, 4 * c.HW1], BF, tag="vself")
                vsv = vself[:].rearrange("p (h q) -> p h q", q=c.HW1)
                nc.vector.tensor_tensor(
                    out=vsv[:, :, 0:c.HID],
                    in0=pso[:, 0:c.C1].rearrange("p (h q) -> p h q", q=c.HID),
                    in1=ees[:, :, None].to_broadcast([128, c.HEADS, c.HID]),
                    op=AL.mult)
                nc.vector.tensor_copy(out=vsv[:, :, c.HID:c.HW1],
                                      in_=ees[:, :, None])
                nc.vector.tensor_tensor(out=ps[:], in0=ps[:], in1=vself[:],
                                        op=AL.add)

                den = sm.tile([128, c.HEADS], F32, tag="den")
                nc.vector.tensor_scalar(
                    out=den[:, :, None],
                    in0=ps[:].rearrange("p (h q) -> p h q",
                                        q=c.HW1)[:, :, c.HID:c.HW1],
                    scalar1=1e-16, scalar2=None, op0=AL.add)
                rec = sm.tile([128, c.HEADS], F32, tag="rec")
                nc.vector.reciprocal(out=rec[:], in_=den[:])
                h1b = sm.tile([128, c.C1], BF, tag="h1b")
                for hh in range(c.HEADS):
                    nc.scalar.activation(
                        out=h1b[:, hh * c.HID:(hh + 1) * c.HID],
                        in_=ps[:, hh * c.HW1:hh * c.HW1 + c.HID],
                        func=AF.Relu, scale=rec[:, hh:hh + 1])
                nc.sync.dma_start(out=h1d[b * 128:(b + 1) * 128, :], in_=h1b[:])
                if debug:
                    h1dbgf = sm.tile([128, c.C1], F32, tag="h1dbgf")
                    nc.vector.tensor_copy(out=h1dbgf[:], in_=h1b[:])
                    nc.sync.dma_start(out=h1dbg_d[b * 128:(b + 1) * 128, :],
                                      in_=h1dbgf[:])

                # ---- fused phase C: h2_pre = relu(h1) @ W2aug ----
                if stage >= "C":
                    psc = pp.tile([128, c.C1 + 8], F32, tag="A")
                    for kh in range(KH):
                        ht = pa.tile([128, 128], BF, tag="ht")
                        nc.sync.dma_start(
                            out=ht[:], in_=h1d[b * 128:(b + 1) * 128,
                                               kh * 128:(kh + 1) * 128],
                            transpose=True)
                        nc.tensor.matmul(out=psc[:, 0:72], lhsT=ht[:],
                                         rhs=w2s[:, kh, :],
                                         start=(kh == 0), stop=(kh == KH - 1))
                    hc = pa.tile([128, c.ROW2], BF, tag="hc")
                    nc.vector.memset(hc[:, 67:c.ROW2], 0.0)
                    nc.vector.memset(hc[:, c.OUT_C:c.OUT_C + 1], 1.0)
                    nc.vector.tensor_copy(out=hc[:, 0:c.OUT_C],
                                          in_=psc[:, 0:c.OUT_C])
                    nc.vector.tensor_copy(out=hc[:, 65:67],
                                          in_=psc[:, c.OUT_C:c.OUT_C + 2])
                    nc.sync.dma_start(out=cc3in[b * 128:(b + 1) * 128, :],
                                      in_=hc[:])
                    if debug:
                        h2f = pa.tile([128, 72], F32, tag="h2f")
                        nc.vector.tensor_copy(out=h2f[:], in_=psc[:, 0:72])
                        nc.sync.dma_start(out=h2dbg_d[b * 128:(b + 1) * 128, :],
                                          in_=h2f[:])

            # ================= phase D: allgather + repack ===================
            if stage >= "D":
                nc.gpsimd.collective_compute(
                    kind="AllGather", op=AL.bypass,
                    replica_groups=[list(range(c.NCORES))],
                    ins=[cc3in[:, :]], outs=[cc3[:, :]])

            # block-own alpha_dst2 column, resident for phase E
            a2dS = cpool.tile([128, c.NBLK], BF)
            if stage >= "D":
                nc.sync.dma_start(
                    out=a2dS[:],
                    in_=cc3in[0:c.NDP, 66:67].rearrange(
                        "(b p) q -> p (b q)", p=128))
            else:
                nc.vector.memset(a2dS[:], 0.0)

            # ================= phase E: layer-2 edge aggregation =============
            pacc = cpool.tile([128, c.OUT_C], F32)
            nc.vector.memset(pacc[:], 0.0)
            for b in (range(c.NBLK) if stage >= "E" else []):
                hg2 = pe2.tile([128, K, c.ROW2], BF, tag="hg2", bufs=4)
                nc.gpsimd.dma_gather(
                    out_ap=hg2[:, 0:kA, :], in_ap=cc3[:, :],
                    idxs_ap=idxA_s[:, b, :], num_idxs=kA * 128,
                    num_idxs_reg=kA * 128, elem_size=c.ROW2, single_packet=False)
                nc.gpsimd.dma_gather(
                    out_ap=hg2[:, kA:K, :], in_ap=cc3[c.SPLIT:c.RTOT, :],
                    idxs_ap=idxB_s[:, b, :], num_idxs=kB * 128,
                    num_idxs_reg=kB * 128, elem_size=c.ROW2, single_packet=False)

                dT2 = pe2.tile([128, K * 128], BF, tag="dT2")
                nc.sync.dma_start(
                    out=dT2[:], in_=dstrelT_d[b:b + 1, :].to_broadcast([128, K * 128]))
                maskT2 = pe2.tile([128, K, 128], BF, tag="mT2")
                nc.vector.tensor_tensor(
                    out=maskT2[:],
                    in0=iotap_s[:, :, None].to_broadcast([128, K, 128]),
                    in1=dT2[:].rearrange("p (k s) -> p k s", k=K),
                    op=AL.is_equal)
                psL2 = pp.tile([128, K * 4], F32, tag="L")
                for j in range(K):
                    nc.tensor.matmul(out=psL2[:, j:j + 1],
                                     lhsT=maskT2[:, j, :],
                                     rhs=a2dS[:, b:b + 1],
                                     start=True, stop=True)

                z2 = sm.tile([128, K], F32, tag="z2")
                nc.vector.tensor_tensor(
                    out=z2[:],
                    in0=hg2[:, :, 65:66].rearrange("p k q -> p (k q)"),
                    in1=psL2[:, 0:K], op=AL.add)
                lr2 = sm.tile([128, K], F32, tag="lr2")
                nc.vector.scalar_tensor_tensor(
                    out=lr2[:], in0=z2[:], scalar=c.NEG, in1=z2[:],
                    op0=AL.mult, op1=AL.max)
                ee2 = sm.tile([128, K], BF, tag="ee2")
                nc.scalar.activation(out=ee2[:], in_=lr2[:], func=AF.Exp)

                mask2 = pe2.tile([128, K, 128], BF, tag="mask2")
                nc.vector.tensor_tensor(
                    out=mask2[:],
                    in0=iota_s[:, None, :].to_broadcast([128, K, 128]),
                    in1=dstrel_s[:, b, :, None].to_broadcast([128, K, 128]),
                    op=AL.is_equal)

                v2 = pe2.tile([128, K, c.OUT_C + 1], BF, tag="v2", bufs=2)
                nc.vector.tensor_tensor(
                    out=v2[:], in0=hg2[:, :, 0:c.OUT_C + 1],
                    in1=ee2[:, :, None].to_broadcast([128, K, c.OUT_C + 1]),
                    op=AL.mult)

                ps2 = pp.tile([128, c.OUT_C + 1], F32, tag="E")
                for j in range(K):
                    nc.tensor.matmul(out=ps2[:], lhsT=mask2[:, j, :],
                                     rhs=v2[:, j, :],
                                     start=(j == 0), stop=(j == K - 1))

                own2 = pe2.tile([128, 67], BF, tag="own2")
                nc.sync.dma_start(out=own2[:],
                                  in_=cc3in[b * 128:(b + 1) * 128, 0:67])
                z2s = sm.tile([128, 1], F32, tag="z2s")
                nc.vector.tensor_tensor(out=z2s[:], in0=own2[:, 65:66],
                                        in1=own2[:, 66:67], op=AL.add)
                lr2s = sm.tile([128, 1], F32, tag="lr2s")
                nc.vector.scalar_tensor_tensor(
                    out=lr2s[:], in0=z2s[:], scalar=c.NEG, in1=z2s[:],
                    op0=AL.mult, op1=AL.max)
                ee2s = sm.tile([128, 1], BF, tag="ee2s")
                nc.scalar.activation(out=ee2s[:], in_=lr2s[:], func=AF.Exp)
                v2s = sm.tile([128, c.OUT_C + 1], BF, tag="v2s")
                nc.vector.tensor_tensor(
                    out=v2s[:], in0=own2[:, 0:c.OUT_C + 1],
                    in1=ee2s[:, 0:1].to_broadcast([128, c.OUT_C + 1]),
                    op=AL.mult)
                nc.vector.tensor_tensor(out=ps2[:], in0=ps2[:], in1=v2s[:],
                                        op=AL.add)

                den2 = sm.tile([128, 1], F32, tag="den2")
                nc.vector.tensor_scalar(
                    out=den2[:], in0=ps2[:, c.OUT_C:c.OUT_C + 1], scalar1=1e-16,
                    scalar2=None, op0=AL.add)
                rec2 = sm.tile([128, 1], F32, tag="rec2")
                nc.vector.reciprocal(out=rec2[:], in_=den2[:])
                o2r = sm.tile([128, c.OUT_C], F32, tag="o2r")
                nc.scalar.activation(out=o2r[:], in_=ps2[:, 0:c.OUT_C],
                                     func=AF.Relu, scale=rec2[:])
                nc.vector.tensor_tensor(out=pacc[:], in0=pacc[:], in1=o2r[:],
                                        op=AL.add)

            # ================= phase F: pool partial =========================
            psf = pp.tile([c.OUT_C + 1, 1], F32, tag="E")
            nc.tensor.matmul(out=psf[0:c.OUT_C, :], lhsT=pacc[:], rhs=ones_s[:],
                             start=True, stop=True)
            pf = sm.tile([c.OUT_C, 1], F32, tag="pf")
            nc.vector.tensor_copy(out=pf[:], in_=psf[0:c.OUT_C, :])
            nc.sync.dma_start(out=pool_d[:, :], in_=pf[:])

    nc.compile()
    legalize_waits(nc)
    return nc


def legalize_waits(nc):
    """Walrus encodes at most ONE sync wait per instruction on this toolchain.
    Hoist excess waits onto same-engine NoOps inserted before the instruction."""
    for fn in nc.m.functions:
        for bb in fn.blocks:
            insts = list(bb.instructions)
            out = []
            changed = False
            for inst in insts:
                si = inst.sync_info
                if si is not None and si.on_wait and len(si.on_wait) > 1:
                    waits = list(si.on_wait)
                    for w in waits[:-1]:
                        nop = mybir.InstNoOp(
                            name=nc.get_next_instruction_name(), ins=[], outs=[])
                        nop.engine = inst.engine
                        nop.sync_info = mybir.SyncInfo(on_wait=[w], on_update=[])
                        nc.register_instruction(nop)
                        out.append(nop)
                    inst.sync_info = mybir.SyncInfo(
                        on_wait=waits[-1:], on_update=list(si.on_update))
                    changed = True
                out.append(inst)
            if changed:
                bb.instructions.clear()
                bb.instructions.extend(out)


def host_finish(cfg, pools, fc_w, fc_b):
    c = cfg
    tot = np.zeros(c.OUT_C, np.float64)
    for p in pools:
        tot += p[:, 0].astype(np.float64)
    pooled = (tot / c.N).astype(np.float32)
    logits = pooled @ np.asarray(fc_w, np.float32) + np.asarray(fc_b, np.float32)
    m = logits.max()
    ls = logits - (m + np.log(np.exp(logits - m).sum()))
    return ls.reshape(1, c.NCLS).astype(np.float32)


_BUILD_CACHE = {}


def run(cfg, inputs, debug=False, trace=False, **run_kwargs):
    in_maps, meta = host_prep(
        cfg, inputs["x"], inputs["edge_index"], inputs["W1"], inputs["att_src1"],
        inputs["att_dst1"], inputs["b1"], inputs["W2"], inputs["att_src2"],
        inputs["att_dst2"], inputs["b2"])
    stage = os.environ.get("KSTAGE", "F")
    key = (cfg.N, cfg.E, meta["kA"], meta["kB"], debug, stage)
    if key not in _BUILD_CACHE:
        _BUILD_CACHE[key] = build(cfg, meta["kA"], meta["kB"], debug=debug,
                                  stage=stage)
    nc = _BUILD_CACHE[key]
    res = bass_utils.run_bass_kernel_spmd(
        nc, in_maps, core_ids=list(range(cfg.NCORES)), trace=trace, **run_kwargs)
    out = host_finish(cfg, [r["pool64"] for r in res.results],
                      inputs["fc_w"], inputs["fc_b"])
    return out, res


def kernel(**inputs):
    cfg = Cfg()
    out, _ = run(cfg, inputs)
    return out


# revision 21
# speedup vs baseline: 1.3657x; 1.0062x over previous
"""GAT (2-layer graph attention network) Bass kernel for 8 Trainium2 NeuronCores.

Strategy (per spec sharding hint): edges are partitioned by destination-node
block so segment-softmax/sum stay core-local; each core owns N/8 destination
nodes. Layer-1 node features (h = x @ W1 plus attention alphas via an
augmented weight matrix) are computed replicated on every core into a DRAM
table with 512-byte rows: 256 fp8(e4m3) h channels + 4 bf16 alpha_src + 4
bf16 alpha_dst. Per-edge source rows arrive via one SWDGE gather per region
(A/B split for the int16 index range). The per-edge alpha_dst values are NOT
gathered: they are broadcast from the block's own alpha rows through a
transposed one-hot mask on the tensor engine (K tiny matmuls), which removes
one 256-B gather per edge versus the previous build. The segment-softmax +
weighted aggregation are fused into PE matmuls with a one-hot {edge x dst}
mask; PSUM accumulates numerator and denominator together. Layer-2
pre-features are exchanged with one AllGather; layer-2 aggregation repeats
the same structure with 256-B bf16 rows. Global mean-pool partials are
reduced on host along with the tiny fc + log_softmax head.
"""
import os
import sys
import types
import math

import numpy as np
import ml_dtypes


def _setup_paths():
    for p in ("/opt/trn_rl_repo", "/root/.axon_site/_ro/trn_rl_repo"):
        if os.path.isdir(p) and p not in sys.path:
            sys.path.insert(0, p)
    try:
        import concourse.bass  # noqa: F401
    except ImportError as e:
        raise RuntimeError(f"concourse not importable: {e}")


_setup_paths()

import concourse.bass as bass  # noqa: E402
import concourse.mybir as mybir  # noqa: E402
import concourse.tile as tile  # noqa: E402
from concourse import bacc, bass_utils  # noqa: E402

bf16 = ml_dtypes.bfloat16
BF = mybir.dt.bfloat16
F32 = mybir.dt.float32
I16 = mybir.dt.int16
FP8 = mybir.dt.float8e4
AL = mybir.AluOpType
AF = mybir.ActivationFunctionType


class Cfg:
    def __init__(self, N=50000, E=800000, IN_C=128, HID=64, OUT_C=64, HEADS=4,
                 NCLS=40, NEG=0.2, NCORES=8):
        self.N, self.E = N, E
        self.IN_C, self.HID, self.OUT_C, self.HEADS = IN_C, HID, OUT_C, HEADS
        self.NCLS, self.NEG, self.NCORES = NCLS, NEG, NCORES
        assert N % NCORES == 0
        self.NB = N // NCORES                      # owned real nodes per core
        self.NBLK = math.ceil(self.NB / 128)       # dst blocks per core
        self.NDP = self.NBLK * 128                 # padded owned rows per core
        self.RTOT = self.NDP * NCORES              # global padded row space
        assert self.RTOT % 128 == 0
        self.NT1 = self.RTOT // 128                # phase-A node tiles
        # A/B gather split (int16 row-index limit), multiple of 128
        self.SPLIT = min(32768, (self.RTOT // 2 + 127) // 128 * 128)
        assert self.SPLIT % 128 == 0 and self.SPLIT < 32768 + 1
        self.C1 = HEADS * HID                      # 256 layer-1 channels
        self.ROW1 = 512                            # table1 row bytes (fp8 units)
        self.AUX1 = 276                            # used bytes per table1 row
        self.HW1 = self.HID + 1                    # 65: head block + ones col
        self.ROW2 = 128                            # table3 cols (256B rows)
        assert self.OUT_C + 2 <= self.ROW2

    def row_of(self, v):
        return self.NDP * (v // self.NB) + (v % self.NB)


def _pack_idx(vals_2d):
    """vals_2d [G, n] -> dma_gather index layout [G, 128, n//16] int16.

    Index i lives at [i % 16, i // 16]; the 16-row group is replicated 8x
    across the 128 partitions.
    """
    G, n = vals_2d.shape
    assert n % 16 == 0
    a = vals_2d.reshape(G, n // 16, 16).transpose(0, 2, 1)   # [G, 16, n/16]
    return np.tile(a, (1, 8, 1)).astype(np.int16)            # [G, 128, n/16]


def host_prep(cfg, x, edge_index, W1, att_src1, att_dst1, b1, W2, att_src2,
              att_dst2, b2):
    """Build per-core in_maps (everything except the graph-independent consts)."""
    c = cfg
    # self-loops are handled densely per block on-device (not slotted)
    src = np.asarray(edge_index[0], dtype=np.int64)
    dst = np.asarray(edge_index[1], dtype=np.int64)
    EE = src.shape[0]

    core = dst // c.NB
    # load-balanced permutation of each core's dst nodes into blocks so that
    # per-block A/B-region edge counts are even (minimizes gather slot count K)
    t0core = src // c.NB                           # provisional (pre-perm) side
    rowmap = np.zeros(c.N, np.int64)               # node -> global padded row
    for ci in range(c.NCORES):
        sel = core == ci
        dloc = dst[sel] - ci * c.NB
        srcA = (c.row_of(src[sel]) < c.SPLIT)      # approx region (pre-perm)
        degA = np.bincount(dloc[srcA], minlength=c.NB).astype(np.int64)
        degT = np.bincount(dloc, minlength=c.NB).astype(np.int64)
        degB = degT - degA
        order_d = np.argsort(-degT, kind="stable")
        cnt = np.zeros(c.NBLK, np.int64)
        lA = np.zeros(c.NBLK, np.float64)
        lB = np.zeros(c.NBLK, np.float64)
        perm = np.zeros(c.NB, np.int64)
        for v in order_d:
            score = np.maximum(lA + degA[v], lB + degB[v]) + 1e9 * (cnt >= 128)
            j = int(np.argmin(score))
            perm[v] = j * 128 + cnt[j]
            cnt[j] += 1
            lA[j] += degA[v]
            lB[j] += degB[v]
        rowmap[ci * c.NB:(ci + 1) * c.NB] = ci * c.NDP + perm
    drow = rowmap[dst]
    blk = (drow - core * c.NDP) // 128
    din = (drow - core * c.NDP) % 128              # dst index within block
    srow = rowmap[src]
    isB = (srow >= c.SPLIT).astype(np.int64)

    gid = (core * c.NBLK + blk) * 2 + isB          # group id (A/B separate)
    order = np.argsort(gid, kind="stable")
    gid_s = gid[order]
    counts = np.bincount(gid_s, minlength=c.NCORES * c.NBLK * 2)
    nA = counts[0::2].reshape(c.NCORES, c.NBLK)
    nB = counts[1::2].reshape(c.NCORES, c.NBLK)
    kA = max(1, int(math.ceil(nA.max() / 128)))
    kB = max(1, int(math.ceil(nB.max() / 128)))
    K = kA + kB

    # rank within group
    starts = np.zeros_like(counts)
    starts[1:] = np.cumsum(counts)[:-1]
    rank = np.arange(EE) - starts[gid_s]

    # destination slot within the (core, blk) slot array of length K*128
    slot = np.where(isB[order] == 0, rank, kA * 128 + rank)
    cg = core[order] * c.NBLK + blk[order]          # [EE] group (core, blk)

    srow_slot = np.zeros((c.NCORES * c.NBLK, K * 128), np.int64)
    srow_slot[:, kA * 128:] = c.SPLIT               # B-region pad -> idx 0
    din_slot = np.full((c.NCORES * c.NBLK, K * 128), 128.0, np.float32)
    srow_slot[cg, slot] = srow[order]
    din_slot[cg, slot] = din[order]

    srow_slot = srow_slot.reshape(c.NCORES, c.NBLK, K * 128)
    din_slot = din_slot.reshape(c.NCORES, c.NBLK, K * 128)

    # augmented weights
    W1 = np.asarray(W1, np.float32)
    a_s1 = np.asarray(att_src1, np.float32).reshape(c.HEADS, c.HID)
    a_d1 = np.asarray(att_dst1, np.float32).reshape(c.HEADS, c.HID)
    W1r = W1.reshape(c.IN_C, c.HEADS, c.HID)
    Wa_s = np.einsum("khc,hc->kh", W1r, a_s1)       # [IN_C, HEADS]
    Wa_d = np.einsum("khc,hc->kh", W1r, a_d1)
    w1aug = np.zeros((c.IN_C, c.C1 + 8), np.float32)
    w1aug[:, :c.C1] = W1
    w1aug[:, c.C1:c.C1 + c.HEADS] = Wa_s
    w1aug[:, c.C1 + 4:c.C1 + 4 + c.HEADS] = Wa_d

    W2 = np.asarray(W2, np.float32)
    a_s2 = np.asarray(att_src2, np.float32).reshape(c.OUT_C)
    a_d2 = np.asarray(att_dst2, np.float32).reshape(c.OUT_C)
    w2aug = np.zeros((c.C1, 72), np.float32)
    w2aug[:, :c.OUT_C] = W2
    w2aug[:, c.OUT_C] = W2 @ a_s2
    w2aug[:, c.OUT_C + 1] = W2 @ a_d2

    assert np.allclose(np.asarray(b1), 0) and np.allclose(np.asarray(b2), 0), \
        "nonzero biases not folded in this build"

    # padded, row-mapped, transposed x tiles
    x = np.asarray(x, np.float32)
    x_pad = np.zeros((c.RTOT, c.IN_C), np.float32)
    x_pad[rowmap] = x
    xT = x_pad.reshape(c.NT1, 128, c.IN_C).transpose(0, 2, 1)  # [t, k, n]
    xT = np.ascontiguousarray(xT).astype(ml_dtypes.float8_e4m3fn)

    iota = np.broadcast_to(np.arange(128, dtype=np.float32),
                           (128, 128)).astype(bf16).copy()
    iota_p = np.arange(128, dtype=np.float32).reshape(128, 1).astype(bf16)
    ones = np.ones((128, 1), np.float32)

    in_maps = []
    meta = dict(kA=kA, kB=kB, K=K)
    for ci in range(c.NCORES):
        idxA = _pack_idx(srow_slot[ci, :, :kA * 128].copy())         # [NBLK,128,kA*8]
        idxB = _pack_idx(srow_slot[ci, :, kA * 128:] - c.SPLIT)
        dr = din_slot[ci].reshape(c.NBLK, K, 128).transpose(2, 0, 1)  # [128,NBLK,K]
        drT = din_slot[ci].reshape(c.NBLK, K * 128)                   # [NBLK,K*128]
        xo = xT[ci * c.NBLK:(ci + 1) * c.NBLK]                       # own tiles
        in_maps.append({
            "x_own": np.ascontiguousarray(xo),
            "w1aug": w1aug.astype(bf16),
            "w2aug": np.ascontiguousarray(w2aug.astype(bf16).reshape(c.C1 // 128, 128, 72).transpose(1, 0, 2)),
            "idxA": np.ascontiguousarray(idxA.transpose(1, 0, 2)),   # [128,NBLK,kA*8]
            "idxB": np.ascontiguousarray(idxB.transpose(1, 0, 2)),
            "dstrel": np.ascontiguousarray(dr).astype(bf16),
            "dstrelT": np.ascontiguousarray(drT).astype(bf16),
            "iota": iota,
            "iota_p": iota_p,
            "ones": ones,
        })
    return in_maps, meta


def build(cfg, kA, kB, core_id_split=None, debug=False, stage="F"):
    """stage: truncate program after phase A/B/C/D/E/F (for HW bisection)."""
    c = cfg
    K = kA + kB
    KH = c.C1 // 128                      # k-halves for layer-2 contraction
    nc = bacc.Bacc("TRN2", target_bir_lowering=False, debug=False,
                   num_devices=c.NCORES)

    # ---- IO ----
    x_own_d = nc.dram_tensor("x_own", [c.NBLK, 128, c.IN_C], FP8, kind="ExternalInput").ap()
    w1_d = nc.dram_tensor("w1aug", [c.IN_C, c.C1 + 8], BF, kind="ExternalInput").ap()
    w2_d = nc.dram_tensor("w2aug", [128, KH, 72], BF, kind="ExternalInput").ap()
    idxA_d = nc.dram_tensor("idxA", [128, c.NBLK, kA * 8], I16, kind="ExternalInput").ap()
    idxB_d = nc.dram_tensor("idxB", [128, c.NBLK, kB * 8], I16, kind="ExternalInput").ap()
    dstrel_d = nc.dram_tensor("dstrel", [128, c.NBLK, K], BF, kind="ExternalInput").ap()
    dstrelT_d = nc.dram_tensor("dstrelT", [c.NBLK, K * 128], BF, kind="ExternalInput").ap()
    iota_d = nc.dram_tensor("iota", [128, 128], BF, kind="ExternalInput").ap()
    iotap_d = nc.dram_tensor("iota_p", [128, 1], BF, kind="ExternalInput").ap()
    ones_d = nc.dram_tensor("ones", [128, 1], F32, kind="ExternalInput").ap()
    pool_d = nc.dram_tensor("pool64", [c.OUT_C, 1], F32, kind="ExternalOutput").ap()
    if debug:
        h1dbg_d = nc.dram_tensor("h1dbg", [c.NDP, c.C1], F32, kind="ExternalOutput").ap()
        h2dbg_d = nc.dram_tensor("h2dbg", [c.NDP, 72], F32, kind="ExternalOutput").ap()

    # ---- internal DRAM ----
    nsplit_t = c.SPLIT // 128
    t1loc = nc.dram_tensor("t1loc", [c.NDP, c.ROW1], FP8, kind="Internal").ap()
    t1 = nc.dram_tensor("t1", [c.RTOT, c.ROW1], FP8, kind="Internal",
                        addr_space="Shared").ap()
    h1d = nc.dram_tensor("h1d", [c.NDP, c.C1], BF, kind="Internal").ap()
    cc3in = nc.dram_tensor("cc3in", [c.NDP, c.ROW2], BF, kind="Internal").ap()
    cc3 = nc.dram_tensor("cc3", [c.RTOT, c.ROW2], BF, kind="Internal",
                         addr_space="Shared").ap()

    with tile.TileContext(nc) as tc:
        with tc.tile_pool(name="const", bufs=1) as cpool, \
             tc.tile_pool(name="pa", bufs=4) as pa, \
             tc.tile_pool(name="pp", bufs=2, space="PSUM") as pp, \
             tc.tile_pool(name="pg", bufs=3) as pg, \
             tc.tile_pool(name="pe2", bufs=3) as pe2, \
             tc.tile_pool(name="sm", bufs=3) as sm:

            # constants resident
            w1s = cpool.tile_from(w1_d)                     # [128, C1+8]
            w2s = cpool.tile_from(w2_d)                     # [128, KH, 72]
            iota_s = cpool.tile_from(iota_d)
            iotap_s = cpool.tile_from(iotap_d)
            ones_s = cpool.tile_from(ones_d)
            idxA_s = cpool.tile_from(idxA_d)
            idxB_s = cpool.tile_from(idxB_d)
            dstrel_s = cpool.tile_from(dstrel_d)

            # ========== phase A: own-strip h table + AllGather ==========
            CH = 7
            for t0 in range(0, c.NBLK, CH):
                xt = pa.tile([128, CH, c.IN_C], FP8, tag="xt")
                nc.sync.dma_start(
                    out=xt[:], in_=x_own_d[t0:t0 + CH, :, :].rearrange("a k n -> k a n"))
                ob = pa.tile([128, CH, c.AUX1], FP8, tag="ob")
                for i in range(CH):
                    ps = pp.tile([128, c.C1 + 8], F32,
                                 tag="A" if i % 2 == 0 else "L")
                    nc.tensor.matmul(out=ps[:], lhsT=xt[:, i, :], rhs=w1s[:],
                                     start=True, stop=True)
                    obv = ob[:, i, 0:4 * c.HW1].rearrange("p (h q) -> p h q",
                                                          q=c.HW1)
                    psv = ps[:, 0:c.C1].rearrange("p (h q) -> p h q", q=c.HID)
                    if i % 2 == 0:
                        nc.scalar.activation(out=obv[:, :, 0:c.HID], in_=psv,
                                             func=AF.Copy)
                        nc.vector.tensor_copy(
                            out=ob[:, i, 4 * c.HW1:4 * c.HW1 + 16].bitcast(BF),
                            in_=ps[:, c.C1:c.C1 + 8])
                        nc.vector.tensor_scalar(
                            out=obv[:, :, c.HID:c.HW1], in0=psv[:, :, 0:1],
                            scalar1=0.0, scalar2=1.0, op0=AL.mult, op1=AL.add)
                    else:
                        nc.vector.tensor_copy(out=obv[:, :, 0:c.HID], in_=psv)
                        nc.scalar.activation(
                            out=ob[:, i, 4 * c.HW1:4 * c.HW1 + 16].bitcast(BF),
                            in_=ps[:, c.C1:c.C1 + 8], func=AF.Copy)
                        nc.vector.tensor_scalar(
                            out=obv[:, :, c.HID:c.HW1], in0=psv[:, :, 0:1],
                            scalar1=0.0, scalar2=1.0, op0=AL.mult, op1=AL.add)
                r0 = t0 * 128
                nc.sync.dma_start(
                    out=t1loc[r0:r0 + CH * 128, 0:c.AUX1].rearrange(
                        "(a p) q -> p a q", p=128), in_=ob[:])
            pre_mT = {}
            for b in (range(min(2, c.NBLK)) if stage >= "B" else []):
                dTp = pg.tile([128, K * 128], BF, tag="dT", name=f"dTp{b}")
                nc.sync.dma_start(
                    out=dTp[:],
                    in_=dstrelT_d[b:b + 1, :].to_broadcast([128, K * 128]))
                mTp = pg.tile([128, K, 128], BF, tag="mT", name=f"mTp{b}")
                nc.vector.tensor_tensor(
                    out=mTp[:],
                    in0=iotap_s[:, :, None].to_broadcast([128, K, 128]),
                    in1=dTp[:].rearrange("p (k s) -> p k s", k=K),
                    op=AL.is_equal)
                pre_mT[b] = mTp
            nc.gpsimd.collective_compute(
                kind="AllGather", op=AL.bypass,
                replica_groups=[list(range(c.NCORES))],
                ins=[t1loc[:, :]], outs=[t1[:, :]])

            # ================= phase B: layer-1 edge aggregation =============
            for b in (range(c.NBLK) if stage >= "B" else []):
                hg = pg.tile([128, K, c.ROW1], FP8, tag="hg", bufs=5)
                nc.gpsimd.dma_gather(
                    out_ap=hg[:, 0:kA, :], in_ap=t1[0:c.SPLIT, :],
                    idxs_ap=idxA_s[:, b, :], num_idxs=kA * 128,
                    num_idxs_reg=kA * 128, elem_size=c.ROW1, single_packet=False)
                nc.gpsimd.dma_gather(
                    out_ap=hg[:, kA:K, :], in_ap=t1[c.SPLIT:c.RTOT, :],
                    idxs_ap=idxB_s[:, b, :], num_idxs=kB * 128,
                    num_idxs_reg=kB * 128, elem_size=c.ROW1, single_packet=False)

                # transposed one-hot mask (dst-major) for alpha_dst broadcast
                if b in pre_mT:
                    maskT = pre_mT[b]
                else:
                    dT = pg.tile([128, K * 128], BF, tag="dT")
                    nc.sync.dma_start(
                        out=dT[:],
                        in_=dstrelT_d[b:b + 1, :].to_broadcast([128, K * 128]))
                    maskT = pg.tile([128, K, 128], BF, tag="mT")
                    nc.vector.tensor_tensor(
                        out=maskT[:],
                        in0=iotap_s[:, :, None].to_broadcast([128, K, 128]),
                        in1=dT[:].rearrange("p (k s) -> p k s", k=K),
                        op=AL.is_equal)
                # block's own alpha_dst rows (on-the-fly from own x tile)
                xo = pa.tile([128, c.IN_C], FP8, tag="xo")
                nc.sync.dma_start(out=xo[:], in_=x_own_d[b, :, :])
                pso = pp.tile([128, c.C1 + 8], F32, tag="L")
                nc.tensor.matmul(out=pso[:], lhsT=xo[:], rhs=w1s[:],
                                 start=True, stop=True)
                adb = sm.tile([128, 4], BF, tag="adb")
                nc.scalar.activation(out=adb[:], in_=pso[:, c.C1 + 4:c.C1 + 8],
                                     func=AF.Copy)
                als = sm.tile([128, 8], F32, tag="als")
                nc.scalar.activation(out=als[:], in_=pso[:, c.C1:c.C1 + 8],
                                     func=AF.Copy)
                psL = pp.tile([128, c.C1 + 8], F32, tag="L")
                for j in range(K):
                    nc.tensor.matmul(out=psL[:, j * 4:(j + 1) * 4],
                                     lhsT=maskT[:, j, :],
                                     rhs=adb[:],
                                     start=True, stop=True)

                z = sm.tile([128, K, c.HEADS], F32, tag="z")
                nc.vector.tensor_tensor(
                    out=z[:],
                    in0=hg[:, :, 4 * c.HW1:4 * c.HW1 + 8].bitcast(BF),
                    in1=psL[:, 0:K * 4].rearrange("p (k h) -> p k h", k=K),
                    op=AL.add)
                lr = sm.tile([128, K, c.HEADS], F32, tag="lr")
                nc.vector.scalar_tensor_tensor(
                    out=lr[:], in0=z[:], scalar=c.NEG, in1=z[:],
                    op0=AL.mult, op1=AL.max)
                eeb = sm.tile([128, K, c.HEADS], BF, tag="eeb")
                nc.scalar.activation(out=eeb[:], in_=lr[:], func=AF.Exp)

                mask = pg.tile([128, K, 128], BF, tag="mask", bufs=2)
                nc.vector.tensor_tensor(
                    out=mask[:],
                    in0=iota_s[:, None, :].to_broadcast([128, K, 128]),
                    in1=dstrel_s[:, b, :, None].to_broadcast([128, K, 128]),
                    op=AL.is_equal)

                v = pg.tile([128, K, 4 * c.HW1], BF, tag="v", bufs=2)
                nc.vector.tensor_tensor(
                    out=v[:].rearrange("p k (h q) -> p k h q", h=c.HEADS),
                    in0=hg[:, :, 0:4 * c.HW1].rearrange("p k (h q) -> p k h q",
                                                        h=c.HEADS),
                    in1=eeb[:, :, :, None].to_broadcast([128, K, c.HEADS, c.HW1]),
                    op=AL.mult)

                ps = pp.tile([128, 4 * c.HW1], F32, tag="B")
                for j in range(K):
                    nc.tensor.matmul(out=ps[:], lhsT=mask[:, j, :],
                                     rhs=v[:, j, :],
                                     start=(j == 0), stop=(j == K - 1))

                # dense self-loop contribution (own rows, partition = dst)
                zs = sm.tile([128, c.HEADS], F32, tag="zs")
                nc.vector.tensor_tensor(
                    out=zs[:], in0=als[:, 0:4], in1=als[:, 4:8], op=AL.add)
                lrs = sm.tile([128, c.HEADS], F32, tag="lrs")
                nc.vector.scalar_tensor_tensor(
                    out=lrs[:], in0=zs[:], scalar=c.NEG, in1=zs[:],
                    op0=AL.mult, op1=AL.max)
                ees = sm.tile([128, c.HEADS], BF, tag="ees")
                nc.scalar.activation(out=ees[:], in_=lrs[:], func=AF.Exp)
                vself = sm.tile([128, 4 * c.HW1], BF, tag="vself")
                vsv = vself[:].rearrange("p (h q) -> p h q", q=c.HW1)
                nc.vector.tensor_tensor(
                    out=vsv[:, :, 0:c.HID],
                    in0=pso[:, 0:c.C1].rearrange("p (h q) -> p h q", q=c.HID),
                    in1=ees[:, :, None].to_broadcast([128, c.HEADS, c.HID]),
                    op=AL.mult)
                nc.vector.tensor_copy(out=vsv[:, :, c.HID:c.HW1],
                                      in_=ees[:, :, None])
                nc.vector.tensor_tensor(out=ps[:], in0=ps[:], in1=vself[:],
                                        op=AL.add)

                den = sm.tile([128, c.HEADS], F32, tag="den")
                nc.vector.tensor_scalar(
                    out=den[:, :, None],
                    in0=ps[:].rearrange("p (h q) -> p h q",
                                        q=c.HW1)[:, :, c.HID:c.HW1],
                    scalar1=1e-16, scalar2=None, op0=AL.add)
                rec = sm.tile([128, c.HEADS], F32, tag="rec")
                nc.vector.reciprocal(out=rec[:], in_=den[:])
                h1b = sm.tile([128, c.C1], BF, tag="h1b")
                for hh in range(c.HEADS):
                    nc.scalar.activation(
                        out=h1b[:, hh * c.HID:(hh + 1) * c.HID],
                        in_=ps[:, hh * c.HW1:hh * c.HW1 + c.HID],
                        func=AF.Relu, scale=rec[:, hh:hh + 1])
                nc.sync.dma_start(out=h1d[b * 128:(b + 1) * 128, :], in_=h1b[:])
                if debug:
                    h1dbgf = sm.tile([128, c.C1], F32, tag="h1dbgf")
                    nc.vector.tensor_copy(out=h1dbgf[:], in_=h1b[:])
                    nc.sync.dma_start(out=h1dbg_d[b * 128:(b + 1) * 128, :],
                                      in_=h1dbgf[:])

                # ---- fused phase C: h2_pre = relu(h1) @ W2aug ----
                if stage >= "C":
                    psc = pp.tile([128, c.C1 + 8], F32, tag="A")
                    for kh in range(KH):
                        ht = pa.tile([128, 128], BF, tag="ht")
                        nc.sync.dma_start(
                            out=ht[:], in_=h1d[b * 128:(b + 1) * 128,
                                               kh * 128:(kh + 1) * 128],
                            transpose=True)
                        nc.tensor.matmul(out=psc[:, 0:72], lhsT=ht[:],
                                         rhs=w2s[:, kh, :],
                                         start=(kh == 0), stop=(kh == KH - 1))
                    hc = pa.tile([128, c.ROW2], BF, tag="hc")
                    nc.vector.memset(hc[:, 67:c.ROW2], 0.0)
                    nc.vector.memset(hc[:, c.OUT_C:c.OUT_C + 1], 1.0)
                    nc.vector.tensor_copy(out=hc[:, 0:c.OUT_C],
                                          in_=psc[:, 0:c.OUT_C])
                    nc.vector.tensor_copy(out=hc[:, 65:67],
                                          in_=psc[:, c.OUT_C:c.OUT_C + 2])
                    nc.sync.dma_start(out=cc3in[b * 128:(b + 1) * 128, :],
                                      in_=hc[:])
                    if debug:
                        h2f = pa.tile([128, 72], F32, tag="h2f")
                        nc.vector.tensor_copy(out=h2f[:], in_=psc[:, 0:72])
                        nc.sync.dma_start(out=h2dbg_d[b * 128:(b + 1) * 128, :],
                                          in_=h2f[:])

            # ================= phase D: allgather + repack ===================
            if stage >= "D":
                nc.gpsimd.collective_compute(
                    kind="AllGather", op=AL.bypass,
                    replica_groups=[list(range(c.NCORES))],
                    ins=[cc3in[:, :]], outs=[cc3[:, :]])

            # block-own alpha_dst2 column, resident for phase E
            a2dS = cpool.tile([128, c.NBLK], BF)
            if stage >= "D":
                nc.sync.dma_start(
                    out=a2dS[:],
                    in_=cc3in[0:c.NDP, 66:67].rearrange(
                        "(b p) q -> p (b q)", p=128))
            else:
                nc.vector.memset(a2dS[:], 0.0)

            # ================= phase E: layer-2 edge aggregation =============
            pacc = cpool.tile([128, c.OUT_C], F32)
            nc.vector.memset(pacc[:], 0.0)
            for b in (range(c.NBLK) if stage >= "E" else []):
                hg2 = pe2.tile([128, K, c.ROW2], BF, tag="hg2", bufs=4)
                nc.gpsimd.dma_gather(
                    out_ap=hg2[:, 0:kA, :], in_ap=cc3[:, :],
                    idxs_ap=idxA_s[:, b, :], num_idxs=kA * 128,
                    num_idxs_reg=kA * 128, elem_size=c.ROW2, single_packet=False)
                nc.gpsimd.dma_gather(
                    out_ap=hg2[:, kA:K, :], in_ap=cc3[c.SPLIT:c.RTOT, :],
                    idxs_ap=idxB_s[:, b, :], num_idxs=kB * 128,
                    num_idxs_reg=kB * 128, elem_size=c.ROW2, single_packet=False)

                dT2 = pe2.tile([128, K * 128], BF, tag="dT2")
                nc.sync.dma_start(
                    out=dT2[:], in_=dstrelT_d[b:b + 1, :].to_broadcast([128, K * 128]))
                maskT2 = pe2.tile([128, K, 128], BF, tag="mT2")
                nc.vector.tensor_tensor(
                    out=maskT2[:],
                    in0=iotap_s[:, :, None].to_broadcast([128, K, 128]),
                    in1=dT2[:].rearrange("p (k s) -> p k s", k=K),
                    op=AL.is_equal)
                psL2 = pp.tile([128, K * 4], F32, tag="L")
                for j in range(K):
                    nc.tensor.matmul(out=psL2[:, j:j + 1],
                                     lhsT=maskT2[:, j, :],
                                     rhs=a2dS[:, b:b + 1],
                                     start=True, stop=True)

                z2 = sm.tile([128, K], F32, tag="z2")
                nc.vector.tensor_tensor(
                    out=z2[:],
                    in0=hg2[:, :, 65:66].rearrange("p k q -> p (k q)"),
                    in1=psL2[:, 0:K], op=AL.add)
                lr2 = sm.tile([128, K], F32, tag="lr2")
                nc.vector.scalar_tensor_tensor(
                    out=lr2[:], in0=z2[:], scalar=c.NEG, in1=z2[:],
                    op0=AL.mult, op1=AL.max)
                ee2 = sm.tile([128, K], BF, tag="ee2")
                nc.scalar.activation(out=ee2[:], in_=lr2[:], func=AF.Exp)

                mask2 = pe2.tile([128, K, 128], BF, tag="mask2")
                nc.vector.tensor_tensor(
                    out=mask2[:],
                    in0=iota_s[:, None, :].to_broadcast([128, K, 128]),
                    in1=dstrel_s[:, b, :, None].to_broadcast([128, K, 128]),
                    op=AL.is_equal)

                v2 = pe2.tile([128, K, c.OUT_C + 1], BF, tag="v2", bufs=2)
                nc.vector.tensor_tensor(
                    out=v2[:], in0=hg2[:, :, 0:c.OUT_C + 1],
                    in1=ee2[:, :, None].to_broadcast([128, K, c.OUT_C + 1]),
                    op=AL.mult)

                ps2 = pp.tile([128, c.OUT_C + 1], F32, tag="E")
                for j in range(K):
                    nc.tensor.matmul(out=ps2[:], lhsT=mask2[:, j, :],
                                     rhs=v2[:, j, :],
                                     start=(j == 0), stop=(j == K - 1))

                own2 = pe2.tile([128, 67], BF, tag="own2")
                nc.sync.dma_start(out=own2[:],
                                  in_=cc3in[b * 128:(b + 1) * 128, 0:67])
                z2s = sm.tile([128, 1], F32, tag="z2s")
                nc.vector.tensor_tensor(out=z2s[:], in0=own2[:, 65:66],
                                        in1=own2[:, 66:67], op=AL.add)
                lr2s = sm.tile([128, 1], F32, tag="lr2s")
                nc.vector.scalar_tensor_tensor(
                    out=lr2s[:], in0=z2s[:], scalar=c.NEG, in1=z2s[:],
                    op0=AL.mult, op1=AL.max)
                ee2s = sm.tile([128, 1], BF, tag="ee2s")
                nc.scalar.activation(out=ee2s[:], in_=lr2s[:], func=AF.Exp)
                v2s = sm.tile([128, c.OUT_C + 1], BF, tag="v2s")
                nc.vector.tensor_tensor(
                    out=v2s[:], in0=own2[:, 0:c.OUT_C + 1],
                    in1=ee2s[:, 0:1].to_broadcast([128, c.OUT_C + 1]),
                    op=AL.mult)
                nc.vector.tensor_tensor(out=ps2[:], in0=ps2[:], in1=v2s[:],
                                        op=AL.add)

                den2 = sm.tile([128, 1], F32, tag="den2")
                nc.vector.tensor_scalar(
                    out=den2[:], in0=ps2[:, c.OUT_C:c.OUT_C + 1], scalar1=1e-16,
                    scalar2=None, op0=AL.add)
                rec2 = sm.tile([128, 1], F32, tag="rec2")
                nc.vector.reciprocal(out=rec2[:], in_=den2[:])
                o2r = sm.tile([128, c.OUT_C], F32, tag="o2r")
                nc.scalar.activation(out=o2r[:], in_=ps2[:, 0:c.OUT_C],
                                     func=AF.Relu, scale=rec2[:])
                nc.vector.tensor_tensor(out=pacc[:], in0=pacc[:], in1=o2r[:],
                                        op=AL.add)

            # ================= phase F: pool partial =========================
            psf = pp.tile([c.OUT_C + 1, 1], F32, tag="E")
            nc.tensor.matmul(out=psf[0:c.OUT_C, :], lhsT=pacc[:], rhs=ones_s[:],
                             start=True, stop=True)
            pf = sm.tile([c.OUT_C, 1], F32, tag="pf")
            nc.vector.tensor_copy(out=pf[:], in_=psf[0:c.OUT_C, :])
            nc.sync.dma_start(out=pool_d[:, :], in_=pf[:])

    nc.compile()
    legalize_waits(nc)
    return nc


def legalize_waits(nc):
    """Walrus encodes at most ONE sync wait per instruction on this toolchain.
    Hoist excess waits onto same-engine NoOps inserted before the instruction."""
    for fn in nc.m.functions:
        for bb in fn.blocks:
            insts = list(bb.instructions)
            out = []
            changed = False
            for inst in insts:
                si = inst.sync_info
                if si is not None and si.on_wait and len(si.on_wait) > 1:
                    waits = list(si.on_wait)
                    for w in waits[:-1]:
                        nop = mybir.InstNoOp(
                            name=nc.get_next_instruction_name(), ins=[], outs=[])
                        nop.engine = inst.engine
                        nop.sync_info = mybir.SyncInfo(on_wait=[w], on_update=[])
                        nc.register_instruction(nop)
                        out.append(nop)
                    inst.sync_info = mybir.SyncInfo(
                        on_wait=waits[-1:], on_update=list(si.on_update))
                    changed = True
                out.append(inst)
            if changed:
                bb.instructions.clear()
                bb.instructions.extend(out)


def host_finish(cfg, pools, fc_w, fc_b):
    c = cfg
    tot = np.zeros(c.OUT_C, np.float64)
    for p in pools:
        tot += p[:, 0].astype(np.float64)
    pooled = (tot / c.N).astype(np.float32)
    logits = pooled @ np.asarray(fc_w, np.float32) + np.asarray(fc_b, np.float32)
    m = logits.max()
    ls = logits - (m + np.log(np.exp(logits - m).sum()))
    return ls.reshape(1, c.NCLS).astype(np.float32)


_BUILD_CACHE = {}


def run(cfg, inputs, debug=False, trace=False, **run_kwargs):
    in_maps, meta = host_prep(
        cfg, inputs["x"], inputs["edge_index"], inputs["W1"], inputs["att_src1"],
        inputs["att_dst1"], inputs["b1"], inputs["W2"], inputs["att_src2"],
        inputs["att_dst2"], inputs["b2"])
    stage = os.environ.get("KSTAGE", "F")
    key = (cfg.N, cfg.E, meta["kA"], meta["kB"], debug, stage)
    if key not in _BUILD_CACHE:
        _BUILD_CACHE[key] = build(cfg, meta["kA"], meta["kB"], debug=debug,
                                  stage=stage)
    nc = _BUILD_CACHE[key]
    res = bass_utils.run_bass_kernel_spmd(
        nc, in_maps, core_ids=list(range(cfg.NCORES)), trace=trace, **run_kwargs)
    out = host_finish(cfg, [r["pool64"] for r in res.results],
                      inputs["fc_w"], inputs["fc_b"])
    return out, res


def kernel(**inputs):
    cfg = Cfg()
    out, _ = run(cfg, inputs)
    return out


# revision 24
# speedup vs baseline: 1.3678x; 1.0015x over previous
"""GAT (2-layer graph attention network) Bass kernel for 8 Trainium2 NeuronCores.

Strategy (per spec sharding hint): edges are partitioned by destination-node
block so segment-softmax/sum stay core-local; each core owns N/8 destination
nodes. Layer-1 node features (h = x @ W1 plus attention alphas via an
augmented weight matrix) are computed replicated on every core into a DRAM
table with 512-byte rows: 256 fp8(e4m3) h channels + 4 bf16 alpha_src + 4
bf16 alpha_dst. Per-edge source rows arrive via one SWDGE gather per region
(A/B split for the int16 index range). The per-edge alpha_dst values are NOT
gathered: they are broadcast from the block's own alpha rows through a
transposed one-hot mask on the tensor engine (K tiny matmuls), which removes
one 256-B gather per edge versus the previous build. The segment-softmax +
weighted aggregation are fused into PE matmuls with a one-hot {edge x dst}
mask; PSUM accumulates numerator and denominator together. Layer-2
pre-features are exchanged with one AllGather; layer-2 aggregation repeats
the same structure with 256-B bf16 rows. Global mean-pool partials are
reduced on host along with the tiny fc + log_softmax head.
"""
import os
import sys
import types
import math

import numpy as np
import ml_dtypes


def _setup_paths():
    for p in ("/opt/trn_rl_repo", "/root/.axon_site/_ro/trn_rl_repo"):
        if os.path.isdir(p) and p not in sys.path:
            sys.path.insert(0, p)
    try:
        import concourse.bass  # noqa: F401
    except ImportError as e:
        raise RuntimeError(f"concourse not importable: {e}")


_setup_paths()

import concourse.bass as bass  # noqa: E402
import concourse.mybir as mybir  # noqa: E402
import concourse.tile as tile  # noqa: E402
from concourse import bacc, bass_utils  # noqa: E402

bf16 = ml_dtypes.bfloat16
BF = mybir.dt.bfloat16
F32 = mybir.dt.float32
I16 = mybir.dt.int16
FP8 = mybir.dt.float8e4
AL = mybir.AluOpType
AF = mybir.ActivationFunctionType


class Cfg:
    def __init__(self, N=50000, E=800000, IN_C=128, HID=64, OUT_C=64, HEADS=4,
                 NCLS=40, NEG=0.2, NCORES=8):
        self.N, self.E = N, E
        self.IN_C, self.HID, self.OUT_C, self.HEADS = IN_C, HID, OUT_C, HEADS
        self.NCLS, self.NEG, self.NCORES = NCLS, NEG, NCORES
        assert N % NCORES == 0
        self.NB = N // NCORES                      # owned real nodes per core
        self.NBLK = math.ceil(self.NB / 128)       # dst blocks per core
        self.NDP = self.NBLK * 128                 # padded owned rows per core
        self.RTOT = self.NDP * NCORES              # global padded row space
        assert self.RTOT % 128 == 0
        self.NT1 = self.RTOT // 128                # phase-A node tiles
        # A/B gather split (int16 row-index limit), multiple of 128
        self.SPLIT = min(32768, (self.RTOT // 2 + 127) // 128 * 128)
        assert self.SPLIT % 128 == 0 and self.SPLIT < 32768 + 1
        self.C1 = HEADS * HID                      # 256 layer-1 channels
        self.ROW1 = 512                            # table1 row bytes (fp8 units)
        self.AUX1 = 276                            # used bytes per table1 row
        self.HW1 = self.HID + 1                    # 65: head block + ones col
        self.ROW2 = 128                            # table3 cols (256B rows)
        assert self.OUT_C + 2 <= self.ROW2

    def row_of(self, v):
        return self.NDP * (v // self.NB) + (v % self.NB)


def _pack_idx(vals_2d):
    """vals_2d [G, n] -> dma_gather index layout [G, 128, n//16] int16.

    Index i lives at [i % 16, i // 16]; the 16-row group is replicated 8x
    across the 128 partitions.
    """
    G, n = vals_2d.shape
    assert n % 16 == 0
    a = vals_2d.reshape(G, n // 16, 16).transpose(0, 2, 1)   # [G, 16, n/16]
    return np.tile(a, (1, 8, 1)).astype(np.int16)            # [G, 128, n/16]


def host_prep(cfg, x, edge_index, W1, att_src1, att_dst1, b1, W2, att_src2,
              att_dst2, b2):
    """Build per-core in_maps (everything except the graph-independent consts)."""
    c = cfg
    # self-loops are handled densely per block on-device (not slotted)
    src = np.asarray(edge_index[0], dtype=np.int64)
    dst = np.asarray(edge_index[1], dtype=np.int64)
    EE = src.shape[0]

    core = dst // c.NB
    # load-balanced permutation of each core's dst nodes into blocks so that
    # per-block A/B-region edge counts are even (minimizes gather slot count K)
    t0core = src // c.NB                           # provisional (pre-perm) side
    rowmap = np.zeros(c.N, np.int64)               # node -> global padded row
    for ci in range(c.NCORES):
        sel = core == ci
        dloc = dst[sel] - ci * c.NB
        srcA = (c.row_of(src[sel]) < c.SPLIT)      # approx region (pre-perm)
        degA = np.bincount(dloc[srcA], minlength=c.NB).astype(np.int64)
        degT = np.bincount(dloc, minlength=c.NB).astype(np.int64)
        degB = degT - degA
        order_d = np.argsort(-degT, kind="stable")
        cnt = np.zeros(c.NBLK, np.int64)
        lA = np.zeros(c.NBLK, np.float64)
        lB = np.zeros(c.NBLK, np.float64)
        perm = np.zeros(c.NB, np.int64)
        for v in order_d:
            score = np.maximum(lA + degA[v], lB + degB[v]) + 1e9 * (cnt >= 128)
            j = int(np.argmin(score))
            perm[v] = j * 128 + cnt[j]
            cnt[j] += 1
            lA[j] += degA[v]
            lB[j] += degB[v]
        rowmap[ci * c.NB:(ci + 1) * c.NB] = ci * c.NDP + perm
    drow = rowmap[dst]
    blk = (drow - core * c.NDP) // 128
    din = (drow - core * c.NDP) % 128              # dst index within block
    srow = rowmap[src]
    isB = (srow >= c.SPLIT).astype(np.int64)

    gid = (core * c.NBLK + blk) * 2 + isB          # group id (A/B separate)
    order = np.argsort(gid, kind="stable")
    gid_s = gid[order]
    counts = np.bincount(gid_s, minlength=c.NCORES * c.NBLK * 2)
    nA = counts[0::2].reshape(c.NCORES, c.NBLK)
    nB = counts[1::2].reshape(c.NCORES, c.NBLK)
    kA = max(1, int(math.ceil(nA.max() / 128)))
    kB = max(1, int(math.ceil(nB.max() / 128)))
    K = kA + kB

    # rank within group
    starts = np.zeros_like(counts)
    starts[1:] = np.cumsum(counts)[:-1]
    rank = np.arange(EE) - starts[gid_s]

    # destination slot within the (core, blk) slot array of length K*128
    slot = np.where(isB[order] == 0, rank, kA * 128 + rank)
    cg = core[order] * c.NBLK + blk[order]          # [EE] group (core, blk)

    srow_slot = np.zeros((c.NCORES * c.NBLK, K * 128), np.int64)
    srow_slot[:, kA * 128:] = c.SPLIT               # B-region pad -> idx 0
    din_slot = np.full((c.NCORES * c.NBLK, K * 128), 128.0, np.float32)
    srow_slot[cg, slot] = srow[order]
    din_slot[cg, slot] = din[order]

    srow_slot = srow_slot.reshape(c.NCORES, c.NBLK, K * 128)
    din_slot = din_slot.reshape(c.NCORES, c.NBLK, K * 128)

    # augmented weights
    W1 = np.asarray(W1, np.float32)
    a_s1 = np.asarray(att_src1, np.float32).reshape(c.HEADS, c.HID)
    a_d1 = np.asarray(att_dst1, np.float32).reshape(c.HEADS, c.HID)
    W1r = W1.reshape(c.IN_C, c.HEADS, c.HID)
    Wa_s = np.einsum("khc,hc->kh", W1r, a_s1)       # [IN_C, HEADS]
    Wa_d = np.einsum("khc,hc->kh", W1r, a_d1)
    w1aug = np.zeros((c.IN_C, c.C1 + 8), np.float32)
    w1aug[:, :c.C1] = W1
    w1aug[:, c.C1:c.C1 + c.HEADS] = Wa_s
    w1aug[:, c.C1 + 4:c.C1 + 4 + c.HEADS] = Wa_d

    W2 = np.asarray(W2, np.float32)
    a_s2 = np.asarray(att_src2, np.float32).reshape(c.OUT_C)
    a_d2 = np.asarray(att_dst2, np.float32).reshape(c.OUT_C)
    w2aug = np.zeros((c.C1, 72), np.float32)
    w2aug[:, :c.OUT_C] = W2
    w2aug[:, c.OUT_C] = W2 @ a_s2
    w2aug[:, c.OUT_C + 1] = W2 @ a_d2

    assert np.allclose(np.asarray(b1), 0) and np.allclose(np.asarray(b2), 0), \
        "nonzero biases not folded in this build"

    # padded, row-mapped, transposed x tiles
    x = np.asarray(x, np.float32)
    x_pad = np.zeros((c.RTOT, c.IN_C), np.float32)
    x_pad[rowmap] = x
    xT = x_pad.reshape(c.NT1, 128, c.IN_C).transpose(0, 2, 1)  # [t, k, n]
    xT = np.ascontiguousarray(xT).astype(ml_dtypes.float8_e4m3fn)

    iota = np.broadcast_to(np.arange(128, dtype=np.float32),
                           (128, 128)).astype(bf16).copy()
    iota_p = np.arange(128, dtype=np.float32).reshape(128, 1).astype(bf16)
    ones = np.ones((128, 1), np.float32)

    in_maps = []
    meta = dict(kA=kA, kB=kB, K=K)
    for ci in range(c.NCORES):
        idxA = _pack_idx(srow_slot[ci, :, :kA * 128].copy())         # [NBLK,128,kA*8]
        idxB = _pack_idx(srow_slot[ci, :, kA * 128:] - c.SPLIT)
        dr = din_slot[ci].reshape(c.NBLK, K, 128).transpose(2, 0, 1)  # [128,NBLK,K]
        drT = din_slot[ci].reshape(c.NBLK, K * 128)                   # [NBLK,K*128]
        xo = xT[ci * c.NBLK:(ci + 1) * c.NBLK]                       # own tiles
        in_maps.append({
            "x_own": np.ascontiguousarray(xo),
            "w1aug": w1aug.astype(bf16),
            "w2aug": np.ascontiguousarray(w2aug.astype(bf16).reshape(c.C1 // 128, 128, 72).transpose(1, 0, 2)),
            "idxA": np.ascontiguousarray(idxA.transpose(1, 0, 2)),   # [128,NBLK,kA*8]
            "idxB": np.ascontiguousarray(idxB.transpose(1, 0, 2)),
            "dstrel": np.ascontiguousarray(dr).astype(bf16),
            "dstrelT": np.ascontiguousarray(drT).astype(bf16),
            "iota": iota,
            "iota_p": iota_p,
            "ones": ones,
        })
    return in_maps, meta


def build(cfg, kA, kB, core_id_split=None, debug=False, stage="F"):
    """stage: truncate program after phase A/B/C/D/E/F (for HW bisection)."""
    c = cfg
    K = kA + kB
    KH = c.C1 // 128                      # k-halves for layer-2 contraction
    nc = bacc.Bacc("TRN2", target_bir_lowering=False, debug=False,
                   num_devices=c.NCORES)

    # ---- IO ----
    x_own_d = nc.dram_tensor("x_own", [c.NBLK, 128, c.IN_C], FP8, kind="ExternalInput").ap()
    w1_d = nc.dram_tensor("w1aug", [c.IN_C, c.C1 + 8], BF, kind="ExternalInput").ap()
    w2_d = nc.dram_tensor("w2aug", [128, KH, 72], BF, kind="ExternalInput").ap()
    idxA_d = nc.dram_tensor("idxA", [128, c.NBLK, kA * 8], I16, kind="ExternalInput").ap()
    idxB_d = nc.dram_tensor("idxB", [128, c.NBLK, kB * 8], I16, kind="ExternalInput").ap()
    dstrel_d = nc.dram_tensor("dstrel", [128, c.NBLK, K], BF, kind="ExternalInput").ap()
    dstrelT_d = nc.dram_tensor("dstrelT", [c.NBLK, K * 128], BF, kind="ExternalInput").ap()
    iota_d = nc.dram_tensor("iota", [128, 128], BF, kind="ExternalInput").ap()
    iotap_d = nc.dram_tensor("iota_p", [128, 1], BF, kind="ExternalInput").ap()
    ones_d = nc.dram_tensor("ones", [128, 1], F32, kind="ExternalInput").ap()
    pool_d = nc.dram_tensor("pool64", [c.OUT_C, 1], F32, kind="ExternalOutput").ap()
    if debug:
        h1dbg_d = nc.dram_tensor("h1dbg", [c.NDP, c.C1], F32, kind="ExternalOutput").ap()
        h2dbg_d = nc.dram_tensor("h2dbg", [c.NDP, 72], F32, kind="ExternalOutput").ap()

    # ---- internal DRAM ----
    nsplit_t = c.SPLIT // 128
    t1loc = nc.dram_tensor("t1loc", [c.NDP, c.ROW1], FP8, kind="Internal").ap()
    t1 = nc.dram_tensor("t1", [c.RTOT, c.ROW1], FP8, kind="Internal",
                        addr_space="Shared").ap()
    h1d = nc.dram_tensor("h1d", [c.NDP, c.C1], BF, kind="Internal").ap()
    cc3in = nc.dram_tensor("cc3in", [c.NDP, c.ROW2], BF, kind="Internal").ap()
    cc3 = nc.dram_tensor("cc3", [c.RTOT, c.ROW2], BF, kind="Internal",
                         addr_space="Shared").ap()

    with tile.TileContext(nc) as tc:
        with tc.tile_pool(name="const", bufs=1) as cpool, \
             tc.tile_pool(name="pa", bufs=4) as pa, \
             tc.tile_pool(name="pp", bufs=2, space="PSUM") as pp, \
             tc.tile_pool(name="pg", bufs=3) as pg, \
             tc.tile_pool(name="pe2", bufs=3) as pe2, \
             tc.tile_pool(name="sm", bufs=3) as sm:

            # constants resident
            w1s = cpool.tile_from(w1_d)                     # [128, C1+8]
            w2s = cpool.tile_from(w2_d)                     # [128, KH, 72]
            iota_s = cpool.tile_from(iota_d)
            iotap_s = cpool.tile_from(iotap_d)
            ones_s = cpool.tile_from(ones_d)
            idxA_s = cpool.tile_from(idxA_d)
            idxB_s = cpool.tile_from(idxB_d)
            dstrel_s = cpool.tile_from(dstrel_d)

            # ========== phase A: own-strip h table + AllGather ==========
            CH = 7
            for t0 in range(0, c.NBLK, CH):
                xt = pa.tile([128, CH, c.IN_C], FP8, tag="xt")
                nc.sync.dma_start(
                    out=xt[:], in_=x_own_d[t0:t0 + CH, :, :].rearrange("a k n -> k a n"))
                ob = pa.tile([128, CH, c.AUX1], FP8, tag="ob")
                for i in range(CH):
                    ps = pp.tile([128, c.C1 + 8], F32,
                                 tag="A" if i % 2 == 0 else "L")
                    nc.tensor.matmul(out=ps[:], lhsT=xt[:, i, :], rhs=w1s[:],
                                     start=True, stop=True)
                    obv = ob[:, i, 0:4 * c.HW1].rearrange("p (h q) -> p h q",
                                                          q=c.HW1)
                    psv = ps[:, 0:c.C1].rearrange("p (h q) -> p h q", q=c.HID)
                    if i % 2 == 0:
                        nc.scalar.activation(out=obv[:, :, 0:c.HID], in_=psv,
                                             func=AF.Copy)
                        nc.vector.tensor_copy(
                            out=ob[:, i, 4 * c.HW1:4 * c.HW1 + 16].bitcast(BF),
                            in_=ps[:, c.C1:c.C1 + 8])
                        nc.vector.tensor_scalar(
                            out=obv[:, :, c.HID:c.HW1], in0=psv[:, :, 0:1],
                            scalar1=0.0, scalar2=1.0, op0=AL.mult, op1=AL.add)
                    else:
                        nc.vector.tensor_copy(out=obv[:, :, 0:c.HID], in_=psv)
                        nc.scalar.activation(
                            out=ob[:, i, 4 * c.HW1:4 * c.HW1 + 16].bitcast(BF),
                            in_=ps[:, c.C1:c.C1 + 8], func=AF.Copy)
                        nc.vector.tensor_scalar(
                            out=obv[:, :, c.HID:c.HW1], in0=psv[:, :, 0:1],
                            scalar1=0.0, scalar2=1.0, op0=AL.mult, op1=AL.add)
                r0 = t0 * 128
                nc.sync.dma_start(
                    out=t1loc[r0:r0 + CH * 128, 0:c.AUX1].rearrange(
                        "(a p) q -> p a q", p=128), in_=ob[:])
            pre_mT = {}
            for b in (range(min(2, c.NBLK)) if stage >= "B" else []):
                dTp = pg.tile([128, K * 128], BF, tag="dT", name=f"dTp{b}")
                nc.sync.dma_start(
                    out=dTp[:],
                    in_=dstrelT_d[b:b + 1, :].to_broadcast([128, K * 128]))
                mTp = pg.tile([128, K, 128], BF, tag="mT", name=f"mTp{b}")
                nc.vector.tensor_tensor(
                    out=mTp[:],
                    in0=iotap_s[:, :, None].to_broadcast([128, K, 128]),
                    in1=dTp[:].rearrange("p (k s) -> p k s", k=K),
                    op=AL.is_equal)
                pre_mT[b] = mTp
            nc.gpsimd.collective_compute(
                kind="AllGather", op=AL.bypass,
                replica_groups=[list(range(c.NCORES))],
                ins=[t1loc[:, :]], outs=[t1[:, :]])

            # ================= phase B: layer-1 edge aggregation =============
            for b in (range(c.NBLK) if stage >= "B" else []):
                hg = pg.tile([128, K, c.ROW1], FP8, tag="hg", bufs=5)
                nc.gpsimd.dma_gather(
                    out_ap=hg[:, 0:kA, :], in_ap=t1[0:c.SPLIT, :],
                    idxs_ap=idxA_s[:, b, :], num_idxs=kA * 128,
                    num_idxs_reg=kA * 128, elem_size=c.ROW1, single_packet=False)
                nc.gpsimd.dma_gather(
                    out_ap=hg[:, kA:K, :], in_ap=t1[c.SPLIT:c.RTOT, :],
                    idxs_ap=idxB_s[:, b, :], num_idxs=kB * 128,
                    num_idxs_reg=kB * 128, elem_size=c.ROW1, single_packet=False)

                # transposed one-hot mask (dst-major) for alpha_dst broadcast
                if b in pre_mT:
                    maskT = pre_mT[b]
                else:
                    dT = pg.tile([128, K * 128], BF, tag="dT")
                    nc.sync.dma_start(
                        out=dT[:],
                        in_=dstrelT_d[b:b + 1, :].to_broadcast([128, K * 128]))
                    maskT = pg.tile([128, K, 128], BF, tag="mT")
                    nc.vector.tensor_tensor(
                        out=maskT[:],
                        in0=iotap_s[:, :, None].to_broadcast([128, K, 128]),
                        in1=dT[:].rearrange("p (k s) -> p k s", k=K),
                        op=AL.is_equal)
                # block's own alpha_dst rows (on-the-fly from own x tile)
                xo = pa.tile([128, c.IN_C], FP8, tag="xo")
                nc.sync.dma_start(out=xo[:], in_=x_own_d[b, :, :])
                pso = pp.tile([128, c.C1 + 8], F32, tag="L")
                nc.tensor.matmul(out=pso[:], lhsT=xo[:], rhs=w1s[:],
                                 start=True, stop=True)
                adb = sm.tile([128, 4], BF, tag="adb")
                nc.scalar.activation(out=adb[:], in_=pso[:, c.C1 + 4:c.C1 + 8],
                                     func=AF.Copy)
                als = sm.tile([128, 8], F32, tag="als")
                nc.scalar.activation(out=als[:], in_=pso[:, c.C1:c.C1 + 8],
                                     func=AF.Copy)
                psL = pp.tile([128, c.C1 + 8], F32, tag="L")
                for j in range(K):
                    nc.tensor.matmul(out=psL[:, j * 4:(j + 1) * 4],
                                     lhsT=maskT[:, j, :],
                                     rhs=adb[:],
                                     start=True, stop=True)

                z = sm.tile([128, K, c.HEADS], F32, tag="z")
                nc.vector.tensor_tensor(
                    out=z[:],
                    in0=hg[:, :, 4 * c.HW1:4 * c.HW1 + 8].bitcast(BF),
                    in1=psL[:, 0:K * 4].rearrange("p (k h) -> p k h", k=K),
                    op=AL.add)
                lr = sm.tile([128, K, c.HEADS], F32, tag="lr")
                nc.vector.scalar_tensor_tensor(
                    out=lr[:], in0=z[:], scalar=c.NEG, in1=z[:],
                    op0=AL.mult, op1=AL.max)
                eeb = sm.tile([128, K, c.HEADS], BF, tag="eeb")
                nc.scalar.activation(out=eeb[:], in_=lr[:], func=AF.Exp)

                mask = pg.tile([128, K, 128], BF, tag="mask", bufs=2)
                nc.vector.tensor_tensor(
                    out=mask[:],
                    in0=iota_s[:, None, :].to_broadcast([128, K, 128]),
                    in1=dstrel_s[:, b, :, None].to_broadcast([128, K, 128]),
                    op=AL.is_equal)

                v = pg.tile([128, K, 4 * c.HW1], BF, tag="v", bufs=2)
                nc.vector.tensor_tensor(
                    out=v[:].rearrange("p k (h q) -> p k h q", h=c.HEADS),
                    in0=hg[:, :, 0:4 * c.HW1].rearrange("p k (h q) -> p k h q",
                                                        h=c.HEADS),
                    in1=eeb[:, :, :, None].to_broadcast([128, K, c.HEADS, c.HW1]),
                    op=AL.mult)

                ps = pp.tile([128, 4 * c.HW1], F32, tag="B")
                for j in range(K):
                    nc.tensor.matmul(out=ps[:], lhsT=mask[:, j, :],
                                     rhs=v[:, j, :],
                                     start=(j == 0), stop=(j == K - 1))

                # dense self-loop contribution (own rows, partition = dst)
                zs = sm.tile([128, c.HEADS], F32, tag="zs")
                nc.vector.tensor_tensor(
                    out=zs[:], in0=als[:, 0:4], in1=als[:, 4:8], op=AL.add)
                lrs = sm.tile([128, c.HEADS], F32, tag="lrs")
                nc.vector.scalar_tensor_tensor(
                    out=lrs[:], in0=zs[:], scalar=c.NEG, in1=zs[:],
                    op0=AL.mult, op1=AL.max)
                ees = sm.tile([128, c.HEADS], BF, tag="ees")
                nc.scalar.activation(out=ees[:], in_=lrs[:], func=AF.Exp)
                vself = sm.tile([128, 4 * c.HW1], BF, tag="vself")
                vsv = vself[:].rearrange("p (h q) -> p h q", q=c.HW1)
                nc.vector.tensor_tensor(
                    out=vsv[:, :, 0:c.HID],
                    in0=pso[:, 0:c.C1].rearrange("p (h q) -> p h q", q=c.HID),
                    in1=ees[:, :, None].to_broadcast([128, c.HEADS, c.HID]),
                    op=AL.mult)
                nc.vector.tensor_copy(out=vsv[:, :, c.HID:c.HW1],
                                      in_=ees[:, :, None])
                nc.vector.tensor_tensor(out=ps[:], in0=ps[:], in1=vself[:],
                                        op=AL.add)

                den = sm.tile([128, c.HEADS], F32, tag="den")
                nc.vector.tensor_scalar(
                    out=den[:, :, None],
                    in0=ps[:].rearrange("p (h q) -> p h q",
                                        q=c.HW1)[:, :, c.HID:c.HW1],
                    scalar1=1e-16, scalar2=None, op0=AL.add)
                rec = sm.tile([128, c.HEADS], F32, tag="rec")
                nc.vector.reciprocal(out=rec[:], in_=den[:])
                h1b = sm.tile([128, c.C1], BF, tag="h1b")
                for hh in range(c.HEADS):
                    nc.scalar.activation(
                        out=h1b[:, hh * c.HID:(hh + 1) * c.HID],
                        in_=ps[:, hh * c.HW1:hh * c.HW1 + c.HID],
                        func=AF.Relu, scale=rec[:, hh:hh + 1])
                nc.sync.dma_start(out=h1d[b * 128:(b + 1) * 128, :], in_=h1b[:])
                if debug:
                    h1dbgf = sm.tile([128, c.C1], F32, tag="h1dbgf")
                    nc.vector.tensor_copy(out=h1dbgf[:], in_=h1b[:])
                    nc.sync.dma_start(out=h1dbg_d[b * 128:(b + 1) * 128, :],
                                      in_=h1dbgf[:])

                # ---- fused phase C: h2_pre = relu(h1) @ W2aug ----
                if stage >= "C":
                    psc = pp.tile([128, c.C1 + 8], F32, tag="A")
                    for kh in range(KH):
                        ht = pa.tile([128, 128], BF, tag="ht")
                        nc.sync.dma_start(
                            out=ht[:], in_=h1d[b * 128:(b + 1) * 128,
                                               kh * 128:(kh + 1) * 128],
                            transpose=True)
                        nc.tensor.matmul(out=psc[:, 0:72], lhsT=ht[:],
                                         rhs=w2s[:, kh, :],
                                         start=(kh == 0), stop=(kh == KH - 1))
                    hc = pa.tile([128, c.ROW2], BF, tag="hc")
                    nc.vector.memset(hc[:, 67:c.ROW2], 0.0)
                    nc.vector.memset(hc[:, c.OUT_C:c.OUT_C + 1], 1.0)
                    nc.vector.tensor_copy(out=hc[:, 0:c.OUT_C],
                                          in_=psc[:, 0:c.OUT_C])
                    nc.vector.tensor_copy(out=hc[:, 65:67],
                                          in_=psc[:, c.OUT_C:c.OUT_C + 2])
                    nc.sync.dma_start(out=cc3in[b * 128:(b + 1) * 128, :],
                                      in_=hc[:])
                    if debug:
                        h2f = pa.tile([128, 72], F32, tag="h2f")
                        nc.vector.tensor_copy(out=h2f[:], in_=psc[:, 0:72])
                        nc.sync.dma_start(out=h2dbg_d[b * 128:(b + 1) * 128, :],
                                          in_=h2f[:])

            # block-own alpha_dst2 column, resident for phase E (local read;
            # issued before the collective so it overlaps the exchange)
            a2dS = cpool.tile([128, c.NBLK], BF)
            pre_mT2 = {}
            if stage >= "D":
                nc.sync.dma_start(
                    out=a2dS[:],
                    in_=cc3in[0:c.NDP, 66:67].rearrange(
                        "(b p) q -> p (b q)", p=128))
                for b in range(min(2, c.NBLK)):
                    dT2p = pe2.tile([128, K * 128], BF, tag="dT2",
                                    name=f"dT2p{b}")
                    nc.sync.dma_start(
                        out=dT2p[:],
                        in_=dstrelT_d[b:b + 1, :].to_broadcast([128, K * 128]))
                    mT2p = pe2.tile([128, K, 128], BF, tag="mT2",
                                    name=f"mT2p{b}")
                    nc.vector.tensor_tensor(
                        out=mT2p[:],
                        in0=iotap_s[:, :, None].to_broadcast([128, K, 128]),
                        in1=dT2p[:].rearrange("p (k s) -> p k s", k=K),
                        op=AL.is_equal)
                    pre_mT2[b] = mT2p
                nc.gpsimd.collective_compute(
                    kind="AllGather", op=AL.bypass,
                    replica_groups=[list(range(c.NCORES))],
                    ins=[cc3in[:, :]], outs=[cc3[:, :]])
            else:
                nc.vector.memset(a2dS[:], 0.0)

            # ================= phase E: layer-2 edge aggregation =============
            pacc = cpool.tile([128, c.OUT_C], F32)
            nc.vector.memset(pacc[:], 0.0)
            for b in (range(c.NBLK) if stage >= "E" else []):
                hg2 = pe2.tile([128, K, c.ROW2], BF, tag="hg2", bufs=4)
                nc.gpsimd.dma_gather(
                    out_ap=hg2[:, 0:kA, :], in_ap=cc3[:, :],
                    idxs_ap=idxA_s[:, b, :], num_idxs=kA * 128,
                    num_idxs_reg=kA * 128, elem_size=c.ROW2, single_packet=False)
                nc.gpsimd.dma_gather(
                    out_ap=hg2[:, kA:K, :], in_ap=cc3[c.SPLIT:c.RTOT, :],
                    idxs_ap=idxB_s[:, b, :], num_idxs=kB * 128,
                    num_idxs_reg=kB * 128, elem_size=c.ROW2, single_packet=False)

                if b in pre_mT2:
                    maskT2 = pre_mT2[b]
                else:
                    dT2 = pe2.tile([128, K * 128], BF, tag="dT2")
                    nc.sync.dma_start(
                        out=dT2[:],
                        in_=dstrelT_d[b:b + 1, :].to_broadcast([128, K * 128]))
                    maskT2 = pe2.tile([128, K, 128], BF, tag="mT2")
                    nc.vector.tensor_tensor(
                        out=maskT2[:],
                        in0=iotap_s[:, :, None].to_broadcast([128, K, 128]),
                        in1=dT2[:].rearrange("p (k s) -> p k s", k=K),
                        op=AL.is_equal)
                psL2 = pp.tile([128, K * 4], F32, tag="L")
                for j in range(K):
                    nc.tensor.matmul(out=psL2[:, j:j + 1],
                                     lhsT=maskT2[:, j, :],
                                     rhs=a2dS[:, b:b + 1],
                                     start=True, stop=True)

                z2 = sm.tile([128, K], F32, tag="z2")
                nc.vector.tensor_tensor(
                    out=z2[:],
                    in0=hg2[:, :, 65:66].rearrange("p k q -> p (k q)"),
                    in1=psL2[:, 0:K], op=AL.add)
                lr2 = sm.tile([128, K], F32, tag="lr2")
                nc.vector.scalar_tensor_tensor(
                    out=lr2[:], in0=z2[:], scalar=c.NEG, in1=z2[:],
                    op0=AL.mult, op1=AL.max)
                ee2 = sm.tile([128, K], BF, tag="ee2")
                nc.scalar.activation(out=ee2[:], in_=lr2[:], func=AF.Exp)

                mask2 = pe2.tile([128, K, 128], BF, tag="mask2")
                nc.vector.tensor_tensor(
                    out=mask2[:],
                    in0=iota_s[:, None, :].to_broadcast([128, K, 128]),
                    in1=dstrel_s[:, b, :, None].to_broadcast([128, K, 128]),
                    op=AL.is_equal)

                v2 = pe2.tile([128, K, c.OUT_C + 1], BF, tag="v2", bufs=2)
                nc.vector.tensor_tensor(
                    out=v2[:], in0=hg2[:, :, 0:c.OUT_C + 1],
                    in1=ee2[:, :, None].to_broadcast([128, K, c.OUT_C + 1]),
                    op=AL.mult)

                ps2 = pp.tile([128, c.OUT_C + 1], F32, tag="E")
                for j in range(K):
                    nc.tensor.matmul(out=ps2[:], lhsT=mask2[:, j, :],
                                     rhs=v2[:, j, :],
                                     start=(j == 0), stop=(j == K - 1))

                own2 = pe2.tile([128, 67], BF, tag="own2")
                nc.sync.dma_start(out=own2[:],
                                  in_=cc3in[b * 128:(b + 1) * 128, 0:67])
                z2s = sm.tile([128, 1], F32, tag="z2s")
                nc.vector.tensor_tensor(out=z2s[:], in0=own2[:, 65:66],
                                        in1=own2[:, 66:67], op=AL.add)
                lr2s = sm.tile([128, 1], F32, tag="lr2s")
                nc.vector.scalar_tensor_tensor(
                    out=lr2s[:], in0=z2s[:], scalar=c.NEG, in1=z2s[:],
                    op0=AL.mult, op1=AL.max)
                ee2s = sm.tile([128, 1], BF, tag="ee2s")
                nc.scalar.activation(out=ee2s[:], in_=lr2s[:], func=AF.Exp)
                v2s = sm.tile([128, c.OUT_C + 1], BF, tag="v2s")
                nc.vector.tensor_tensor(
                    out=v2s[:], in0=own2[:, 0:c.OUT_C + 1],
                    in1=ee2s[:, 0:1].to_broadcast([128, c.OUT_C + 1]),
                    op=AL.mult)
                nc.vector.tensor_tensor(out=ps2[:], in0=ps2[:], in1=v2s[:],
                                        op=AL.add)

                den2 = sm.tile([128, 1], F32, tag="den2")
                nc.vector.tensor_scalar(
                    out=den2[:], in0=ps2[:, c.OUT_C:c.OUT_C + 1], scalar1=1e-16,
                    scalar2=None, op0=AL.add)
                rec2 = sm.tile([128, 1], F32, tag="rec2")
                nc.vector.reciprocal(out=rec2[:], in_=den2[:])
                o2r = sm.tile([128, c.OUT_C], F32, tag="o2r")
                nc.scalar.activation(out=o2r[:], in_=ps2[:, 0:c.OUT_C],
                                     func=AF.Relu, scale=rec2[:])
                nc.vector.tensor_tensor(out=pacc[:], in0=pacc[:], in1=o2r[:],
                                        op=AL.add)

            # ================= phase F: pool partial =========================
            psf = pp.tile([c.OUT_C + 1, 1], F32, tag="E")
            nc.tensor.matmul(out=psf[0:c.OUT_C, :], lhsT=pacc[:], rhs=ones_s[:],
                             start=True, stop=True)
            pf = sm.tile([c.OUT_C, 1], F32, tag="pf")
            nc.vector.tensor_copy(out=pf[:], in_=psf[0:c.OUT_C, :])
            nc.sync.dma_start(out=pool_d[:, :], in_=pf[:])

    nc.compile()
    legalize_waits(nc)
    return nc


def legalize_waits(nc):
    """Walrus encodes at most ONE sync wait per instruction on this toolchain.
    Hoist excess waits onto same-engine NoOps inserted before the instruction."""
    for fn in nc.m.functions:
        for bb in fn.blocks:
            insts = list(bb.instructions)
            out = []
            changed = False
            for inst in insts:
                si = inst.sync_info
                if si is not None and si.on_wait and len(si.on_wait) > 1:
                    waits = list(si.on_wait)
                    for w in waits[:-1]:
                        nop = mybir.InstNoOp(
                            name=nc.get_next_instruction_name(), ins=[], outs=[])
                        nop.engine = inst.engine
                        nop.sync_info = mybir.SyncInfo(on_wait=[w], on_update=[])
                        nc.register_instruction(nop)
                        out.append(nop)
                    inst.sync_info = mybir.SyncInfo(
                        on_wait=waits[-1:], on_update=list(si.on_update))
                    changed = True
                out.append(inst)
            if changed:
                bb.instructions.clear()
                bb.instructions.extend(out)


def host_finish(cfg, pools, fc_w, fc_b):
    c = cfg
    tot = np.zeros(c.OUT_C, np.float64)
    for p in pools:
        tot += p[:, 0].astype(np.float64)
    pooled = (tot / c.N).astype(np.float32)
    logits = pooled @ np.asarray(fc_w, np.float32) + np.asarray(fc_b, np.float32)
    m = logits.max()
    ls = logits - (m + np.log(np.exp(logits - m).sum()))
    return ls.reshape(1, c.NCLS).astype(np.float32)


_BUILD_CACHE = {}


def run(cfg, inputs, debug=False, trace=False, **run_kwargs):
    in_maps, meta = host_prep(
        cfg, inputs["x"], inputs["edge_index"], inputs["W1"], inputs["att_src1"],
        inputs["att_dst1"], inputs["b1"], inputs["W2"], inputs["att_src2"],
        inputs["att_dst2"], inputs["b2"])
    stage = os.environ.get("KSTAGE", "F")
    key = (cfg.N, cfg.E, meta["kA"], meta["kB"], debug, stage)
    if key not in _BUILD_CACHE:
        _BUILD_CACHE[key] = build(cfg, meta["kA"], meta["kB"], debug=debug,
                                  stage=stage)
    nc = _BUILD_CACHE[key]
    res = bass_utils.run_bass_kernel_spmd(
        nc, in_maps, core_ids=list(range(cfg.NCORES)), trace=trace, **run_kwargs)
    out = host_finish(cfg, [r["pool64"] for r in res.results],
                      inputs["fc_w"], inputs["fc_b"])
    return out, res


def kernel(**inputs):
    cfg = Cfg()
    out, _ = run(cfg, inputs)
    return out
